# revision 1
# baseline (speedup 1.0000x reference)
"""Trainium2 Bass kernel for a transformer decoder layer (self-attn +
cross-attn + FFN), fp8-e4m3 DoubleRow edition.

Sharding: 8 cores = 4 batches x 2 halves, no collectives. Core h of a batch
owns the interleaved query tiles {h, h+2, ..., h+14} (causal load balance) and
computes the FULL K/V projections for its batch locally (cheaper than the
pair-exchange collective at fp8 speeds).

Numerics: all matmuls run in fp8-e4m3 with DoubleRow perf mode (2 contraction
rows per partition).  Weights are pre-scaled x32 host-side so they sit in
e4m3's normal range; every PSUM drain folds the 1/32 back in.  Three
refinements keep absmax rel err ~3e-3 (gate is 2e-2):
  - FFN: both matmuls use hi+lo fp8 splits of activations AND weights
    (3 DoubleRow matmuls per logical matmul = fp16-level accuracy at 2x
    fp16 speed).
  - Early causal tokens (global positions 0..255, each core's local q-tile
    u=0) see few keys, so fp8 noise doesn't average out: their Q/K/V/E values
    are computed via the same hi+lo corrected path.
  - LayerNorm gammas/betas are folded into the next matmul's weights where
    possible (WQ2, W_ff1) and the residual carriers keep f16 precision.

Causal masking is via per-core precomputed [128 x 512] mask rows (tri/ones/
zeros blocks depending on core half), applied to each self-attn score tile,
so the SPMD program is uniform across cores.

Stage order: A1 (self K/V/Q + early) -> B (self-attn, prefetching A2 inputs)
-> A2 (cross K/V) -> T1 (n1 transpose) -> C0 (Q2) -> C (cross-attn) ->
T2 (n2 transpose hi/lo) -> D (FFN, token-chunked, streamed W_ff1).
"""

from contextlib import ExitStack

import ml_dtypes
import numpy as np

import concourse.bass as bass
import concourse.mybir as mybir
import concourse.tile as tile
from concourse import bacc
from concourse.bass_utils import run_bass_kernel_spmd
from concourse.masks import make_identity

f32 = mybir.dt.float32
f16 = mybir.dt.float16
f8 = mybir.dt.float8e4

P = 128
D = 1024          # d_model
S = 2048          # kv sequence length
NQ = 1024         # query tokens per core
DFF = 4096
DTI = D // P      # 8 d-model partition tiles
KTI = S // P      # 16 kv token tiles
QTI = NQ // P     # 8 query tiles
FTI = DFF // P    # 32 d_ff tiles
ACT = mybir.ActivationFunctionType
ALU = mybir.AluOpType
DR = mybir.MatmulPerfMode.DoubleRow
N_CORES = 8
WS = 32.0         # host-side weight pre-scale
IWS = 1.0 / WS
SCALE = 1.0 / 32.0  # 1/sqrt(D) softmax scale
E4NP = ml_dtypes.float8_e4m3


def build_nc():
    nc = bacc.Bacc("TRN2", target_bir_lowering=False, debug=False,
                   num_devices=N_CORES)

    def dp(name, shape, dt, out=False):
        return nc.declare_dram_parameter(name, shape, dt, isOutput=out)

    xq8_d = dp("xq8", [P, DTI, NQ], f8)
    xq0lo_d = dp("xq0lo", [P, DTI, P], f8)
    xkv8_d = dp("xkv8", [P, DTI, S], f8)
    xkvelo_d = dp("xkvelo", [P, DTI, 2 * P], f8)
    z8_d = dp("z8", [P, DTI, S], f8)
    yres_d = dp("yres", [P, QTI, D], f16)
    w_d = {n: dp(n, [P, DTI, D], f8)
           for n in ["wq1", "wk1", "wv1", "wq2", "wk2", "wv2",
                     "wq1lo", "wk1lo", "wv1lo"]}
    wf1_d = dp("wf1", [FTI * P, 2, DTI, P], f8)   # hi/lo interleaved
    wf2h_d = dp("wf2h", [P, FTI, D], f8)
    wf2l_d = dp("wf2l", [P, FTI, D], f8)
    bf1_d = dp("bf1", [P, FTI], f32)
    qb2_d = dp("qb2", [P, DTI], f32)
    mask_d = dp("maskblk", [P, DTI, 512], f8)
    v16_d = {n: dp(n, [D], f16) for n in ["g1", "be1", "g2", "b2r"]}
    v32_d = {n: dp(n, [D], f32) for n in ["g3", "be3"]}
    out_d = dp("out", [NQ, D], f32, out=True)

    def bc(ap):  # broadcast a [n] dram vector across 128 partitions
        return bass.AP(tensor=ap.tensor, offset=ap.offset,
                       ap=[[0, P]] + [list(x) for x in ap.ap])

    with tile.TileContext(nc) as tc, ExitStack() as top:
        const = top.enter_context(tc.tile_pool(name="const", bufs=1))
        dramp = top.enter_context(tc.tile_pool(name="dramp", bufs=1,
                                               space="DRAM"))
        ident = const.tile([P, P], f16, name="ident", tag="ident")
        make_identity(nc, ident)
        masks = const.tile([P, DTI, 512], f8, name="masks", tag="masks")
        ones8t = const.tile([P, 2, 16], f8, name="ones8", tag="ones8")
        nc.vector.memset(ones8t, 1.0)
        ones8 = ones8t[:, :, 0:1]  # outer step 16B: dual-fp8 ldweights rule
        eps = const.tile([P, 1], f32, name="eps", tag="eps")
        nc.vector.memset(eps, 1e-5)
        bf1sb = const.tile([P, FTI], f32, name="bf1sb", tag="bf1sb")
        qb2sb = const.tile([P, DTI], f32, name="qb2sb", tag="qb2sb")

        def vload(name, dt, dram):
            return const.tile([P, D], dt, name=f"{name}b", tag=f"{name}b")

        g1b = vload("g1", f16, v16_d)
        b1b = vload("be1", f16, v16_d)
        g2b = vload("g2", f16, v16_d)
        b2rb = vload("b2r", f16, v16_d)
        g3b = vload("g3", f32, v32_d)
        b3b = vload("be3", f32, v32_d)

        def load_consts():
            # deferred off-critical-path constant loads
            nc.sync.dma_start(out=masks, in_=mask_d.ap())
            nc.sync.dma_start(out=bf1sb, in_=bf1_d.ap())
            nc.sync.dma_start(out=qb2sb, in_=qb2_d.ap())
            for t, nm, dd in [(g1b, "g1", v16_d), (b1b, "be1", v16_d),
                              (g2b, "g2", v16_d), (b2rb, "b2r", v16_d),
                              (g3b, "g3", v32_d), (b3b, "be3", v32_d)]:
                nc.sync.dma_start(out=t, in_=bc(dd[nm].ap()))

        # ---- persistent pools; LIFO per side ----
        # left: y1, n1, [zpB/wpB], [n1T], [n2T]
        # right: yres, kvp, earlyp | y2r, n2, kv2p, qT2p
        y1p = tc.alloc_tile_pool(name="y1p", bufs=1)
        y1 = y1p.tile([P, QTI, D], f16, name="y1", tag="y1")
        n1p = tc.alloc_tile_pool(name="n1p", bufs=1)
        n1 = n1p.tile([P, QTI, D], f16, name="n1", tag="n1")

        yresp = tc.alloc_tile_pool(name="yresp", bufs=1, side="right")
        yres = yresp.tile([P, QTI, D], f16, name="yres", tag="yres")
        kvp = tc.alloc_tile_pool(name="kvp", bufs=1, side="right")
        kT = kvp.tile([P, DTI, S], f8, name="kT", tag="kT")
        v = kvp.tile([P, KTI, D], f8, name="v", tag="v")
        qT = kvp.tile([P, DTI, NQ], f8, name="qT", tag="qT")
        earlyp = tc.alloc_tile_pool(name="earlyp", bufs=1, side="right")
        keT = [earlyp.tile([P, DTI, 2 * P], f8, name=f"keT{x}", tag=f"keT{x}")
               for x in range(2)]  # hi, lo
        qeT = [earlyp.tile([P, DTI, P], f8, name=f"qeT{x}", tag=f"qeT{x}")
               for x in range(2)]
        ve = [earlyp.tile([P, 2, D], f8, name=f"ve{x}", tag=f"ve{x}")
              for x in range(2)]

        def dr_acc(ps, terms, rhs_sl, lhs_sl):
            """Accumulate sum of DoubleRow products into psum region ps.
            terms: list of (lhsT_tile, rhs_tile); contraction over DTI//2
            k-tile pairs per term. rhs_sl/lhs_sl: fn(tile, g) -> AP."""
            n = len(terms) * (DTI // 2)
            i = 0
            for lt, rt in terms:
                for g in range(DTI // 2):
                    nc.tensor.matmul(ps, lhsT=lhs_sl(lt, g),
                                     rhs=rhs_sl(rt, g), perf_mode=DR,
                                     start=(i == 0), stop=(i == n - 1))
                    i += 1

        def split3(pool, ps, scale, bias, func, hi_out, lo_out, eng, n):
            """3-op hi/lo drain: t16 = func(scale*ps + bias); hi = q8(t16);
            lo = q8(t16 - hi)."""
            t16 = pool.tile([P, n], f16, name="t16", tag="t16", bufs=3)
            nc.scalar.activation(out=t16, in_=ps, func=func, bias=bias,
                                 scale=scale)
            if eng == 0:
                nc.vector.tensor_copy(out=hi_out, in_=t16)
                nc.gpsimd.tensor_sub(lo_out, t16, hi_out)
            else:
                nc.gpsimd.tensor_copy(out=hi_out, in_=t16)
                nc.vector.tensor_sub(lo_out, t16, hi_out)

        # ==================== stage A1: self-attn projections =============
        with ExitStack() as stA:
            wpA = stA.enter_context(tc.tile_pool(name="wpA", bufs=1))
            xpA = stA.enter_context(tc.tile_pool(name="xpA", bufs=1))
            psA = stA.enter_context(tc.tile_pool(name="psA", bufs=3,
                                                 space="PSUM"))
            psE = stA.enter_context(tc.tile_pool(name="psE", bufs=2,
                                                 space="PSUM"))
            drp = stA.enter_context(tc.tile_pool(name="drpA", bufs=1))

            def wload(tag, name):
                t = wpA.tile([P, DTI, D], f8, name=name, tag=tag)
                nc.sync.dma_start(out=t, in_=w_d[name].ap())
                return t

            wk1 = wload("wA0", "wk1")
            xkv8 = xpA.tile([P, DTI, S], f8, name="xkv8", tag="xkv8")
            nc.sync.dma_start(out=xkv8, in_=xkv8_d.ap())
            wv1 = wload("wA1", "wv1")
            wk1lo = wload("wA2", "wk1lo")
            wv1lo = wload("wA3", "wv1lo")
            xkvelo = xpA.tile([P, DTI, 2 * P], f8, name="xkvelo",
                              tag="xkvelo")
            nc.sync.dma_start(out=xkvelo, in_=xkvelo_d.ap())
            xq8 = xpA.tile([P, DTI, NQ], f8, name="xq8", tag="xq8")
            nc.sync.dma_start(out=xq8, in_=xq8_d.ap())
            xq0lo = xpA.tile([P, DTI, P], f8, name="xq0lo", tag="xq0lo")
            nc.sync.dma_start(out=xq0lo, in_=xq0lo_d.ap())
            nc.sync.dma_start(out=yres, in_=yres_d.ap())
            load_consts()

            # K1: kT[:, j, :] = (wk1.T @ xkv)/32, d_out on partitions
            for j in range(DTI):
                for th in range(2):
                    ps = psA.tile([P, 1024], f32, name="psp", tag="psp")
                    for sub in range(2):
                        tsl = slice(th * 1024 + sub * 512,
                                    th * 1024 + sub * 512 + 512)
                        dr_acc(ps[:, sub * 512:sub * 512 + 512],
                               [(wk1, xkv8)],
                               lambda t, g, tsl=tsl: t[:, 2 * g:2 * g + 2, tsl],
                               lambda t, g, j=j: t[:, 2 * g:2 * g + 2,
                                                   j * P:(j + 1) * P])
                    osl = kT[:, j, th * 1024:(th + 1) * 1024]
                    if (j + th) % 2 == 0:
                        nc.scalar.activation(out=osl, in_=ps, func=ACT.Copy,
                                             scale=IWS)
                    else:
                        nc.vector.tensor_scalar_mul(osl, ps, IWS)

            # early K (tokens 0:256), hi+lo corrected
            for j in range(DTI):
                ps = psE.tile([P, 512], f32, name="pse", tag="pse")
                dr_acc(ps[:, 0:256],
                       [(wk1, xkv8), (wk1, xkvelo), (wk1lo, xkv8)],
                       lambda t, g: (t[:, 2 * g:2 * g + 2, 0:256]
                                     if t is xkv8 else
                                     t[:, 2 * g:2 * g + 2, :]),
                       lambda t, g, j=j: t[:, 2 * g:2 * g + 2,
                                           j * P:(j + 1) * P])
                split3(drp, ps[:, 0:256], IWS, 0.0, ACT.Copy,
                       keT[0][:, j, :], keT[1][:, j, :], j % 2, 256)

            # V1: v[:, t, :] = (xkv.T @ wv1)/32, tokens on partitions
            for t in range(KTI):
                ps = psA.tile([P, 1024], f32, name="psp", tag="psp")
                for half in range(2):
                    dr_acc(ps[:, half * 512:half * 512 + 512],
                           [(xkv8, wv1)],
                           lambda tt, g, half=half: tt[:, 2 * g:2 * g + 2,
                                                       half * 512:half * 512 + 512],
                           lambda tt, g, t=t: tt[:, 2 * g:2 * g + 2,
                                                 t * P:(t + 1) * P])
                osl = v[:, t, :]
                if t % 2 == 0:
                    nc.scalar.activation(out=osl, in_=ps, func=ACT.Copy,
                                         scale=IWS)
                else:
                    nc.vector.tensor_scalar_mul(osl, ps, IWS)

            # early V (k-tiles 0..1), hi+lo corrected
            for t in range(2):
                for half in range(2):
                    ps = psE.tile([P, 512], f32, name="pse", tag="pse")
                    hsl = slice(half * 512, half * 512 + 512)
                    dr_acc(ps,
                           [(xkv8, wv1), (xkvelo, wv1), (xkv8, wv1lo)],
                           lambda tt, g, hsl=hsl: tt[:, 2 * g:2 * g + 2, hsl],
                           lambda tt, g, t=t: tt[:, 2 * g:2 * g + 2,
                                                 t * P:(t + 1) * P])
                    split3(drp, ps, IWS, 0.0, ACT.Copy,
                           ve[0][:, t, hsl], ve[1][:, t, hsl],
                           (t + half) % 2, 512)

            # Q1 (weights reuse the K1 buffers)
            wq1 = wload("wA0", "wq1")
            wq1lo = wload("wA2", "wq1lo")
            for j in range(DTI):
                ps = psA.tile([P, 1024], f32, name="psp", tag="psp")
                for sub in range(2):
                    dr_acc(ps[:, sub * 512:sub * 512 + 512],
                           [(wq1, xq8)],
                           lambda t, g, sub=sub: t[:, 2 * g:2 * g + 2,
                                                   sub * 512:sub * 512 + 512],
                           lambda t, g, j=j: t[:, 2 * g:2 * g + 2,
                                               j * P:(j + 1) * P])
                osl = qT[:, j, :]
                if j % 2 == 0:
                    nc.scalar.activation(out=osl, in_=ps, func=ACT.Copy,
                                         scale=IWS)
                else:
                    nc.vector.tensor_scalar_mul(osl, ps, IWS)
            # early Q (own u=0 tile)
            for j in range(DTI):
                ps = psE.tile([P, 512], f32, name="pse", tag="pse")
                dr_acc(ps[:, 0:P],
                       [(wq1, xq8), (wq1, xq0lo), (wq1lo, xq8)],
                       lambda t, g: (t[:, 2 * g:2 * g + 2, 0:P]
                                     if t is xq8 else
                                     t[:, 2 * g:2 * g + 2, :]),
                       lambda t, g, j=j: t[:, 2 * g:2 * g + 2,
                                           j * P:(j + 1) * P])
                split3(drp, ps[:, 0:P], IWS, 0.0, ACT.Copy,
                       qeT[0][:, j, :], qeT[1][:, j, :], j % 2, P)

        # ==================== attention helper ============================
        def attention(stk, tagp, qTt, kTt, vt, resid_sl, gb, bb, yout, nout,
                      masked):
            pss = stk.enter_context(tc.tile_pool(name=f"{tagp}pss", bufs=3,
                                                 space="PSUM"))
            pso = stk.enter_context(tc.tile_pool(name=f"{tagp}pso", bufs=2,
                                                 space="PSUM"))
            psd = stk.enter_context(tc.tile_pool(name=f"{tagp}psd", bufs=1,
                                                 space="PSUM"))
            ep = stk.enter_context(tc.tile_pool(name=f"{tagp}ep", bufs=2))
            lnp = stk.enter_context(tc.tile_pool(name=f"{tagp}lnp", bufs=4))
            for c in range(2):
                e = ep.tile([P, KTI, 512], f8, name="e", tag="e")
                vis = range(8 * (c + 1)) if masked else range(KTI)
                nvis = len(vis)
                ee = None
                if masked and c == 0:
                    # early corrected scores/E for q-tile u=0, k-tiles 0..1
                    # (emitted first so its long drain chain overlaps the
                    # main score tiles)
                    ee = lnp.tile([P, 2, 2, P], f8, name="ee", tag="ee",
                                  bufs=1)
                    for t in range(2):
                        ps = pss.tile([P, 512], f32, name="ps_s", tag="ps_s")
                        dr_acc(ps[:, 0:P],
                               [(keT[0], qeT[0]), (keT[0], qeT[1]),
                                (keT[1], qeT[0])],
                               lambda tt, g: tt[:, 2 * g:2 * g + 2, :],
                               lambda tt, g, t=t: tt[:, 2 * g:2 * g + 2,
                                                     t * P:(t + 1) * P])
                        tm = lnp.tile([P, P], f16, name="etm", tag="etm",
                                      bufs=2)
                        nc.scalar.activation(out=tm, in_=ps[:, 0:P],
                                             func=ACT.Exp, scale=SCALE)
                        nc.vector.tensor_mul(tm, tm, masks[:, t, 0:P])
                        nc.vector.tensor_copy(out=ee[:, 0, t, :], in_=tm)
                        nc.gpsimd.tensor_sub(ee[:, 1, t, :], tm,
                                             ee[:, 0, t, :])
                for t in vis:
                    ps = pss.tile([P, 512], f32, name="ps_s", tag="ps_s")
                    dr_acc(ps, [(kTt, qTt)],
                           lambda tt, g, c=c: tt[:, 2 * g:2 * g + 2,
                                                 c * 512:c * 512 + 512],
                           lambda tt, g, t=t: tt[:, 2 * g:2 * g + 2,
                                                 t * P:(t + 1) * P])
                    nc.scalar.activation(out=e[:, t, :], in_=ps,
                                         func=ACT.Exp, scale=SCALE)
                    if masked and t >= 8 * c:
                        r = t - 8 * c
                        esl = e[:, t, :]
                        if r % 2 == 0:
                            nc.vector.tensor_mul(esl, esl, masks[:, r, :])
                        else:
                            nc.gpsimd.tensor_mul(esl, esl, masks[:, r, :])
                # denominators: ones^T @ E -> [1, 512]
                pd = psd.tile([1, 512], f32, name="pd", tag="pd")
                for i in range(nvis // 2):
                    nc.tensor.matmul(pd, lhsT=ones8,
                                     rhs=e[:, 2 * i:2 * i + 2, :],
                                     perf_mode=DR, start=(i == 0),
                                     stop=(i == nvis // 2 - 1))
                dsb = lnp.tile([1, 512], f32, name="dsb", tag="dsb")
                nc.vector.tensor_copy(out=dsb, in_=pd)
                dscr = dramp.tile([512], f32, name="dscr",
                                  tag=f"{tagp}dscr{c}")
                nc.sync.dma_start(out=dscr, in_=dsb)
                if ee is not None:
                    pde = psd.tile([1, 512], f32, name="pde", tag="pd")
                    for hl in range(2):
                        nc.tensor.matmul(pde[:, 0:P], lhsT=ones8,
                                         rhs=ee[:, hl, :, :], perf_mode=DR,
                                         start=(hl == 0), stop=(hl == 1))
                    dsbe = lnp.tile([1, P], f32, name="dsbe", tag="dsbe")
                    nc.vector.tensor_copy(out=dsbe, in_=pde[:, 0:P])
                    nc.sync.dma_start(out=dscr[0:P], in_=dsbe)
                dT = lnp.tile([P, 4], f32, name="dT", tag="dT")
                nc.sync.dma_start(out=dT,
                                  in_=dscr.rearrange("(a p) -> p a", p=P))
                recT = lnp.tile([P, 4], f32, name="recT", tag="recT")
                nc.vector.reciprocal(recT, dT)
                u4order = [1, 2, 3, 0] if ee is not None else range(4)
                for u4 in u4order:
                    u = c * 4 + u4
                    po = pso.tile([P, 1024], f32, name="po", tag="po")
                    if ee is not None and u == 0:
                        for half in range(2):
                            hsl = slice(half * 512, half * 512 + 512)
                            for ti, (el, vl) in enumerate(
                                    [(0, 0), (1, 0), (0, 1)]):
                                nc.tensor.matmul(
                                    po[:, hsl], lhsT=ee[:, el, :, :],
                                    rhs=ve[vl][:, :, hsl], perf_mode=DR,
                                    start=(ti == 0), stop=(ti == 2))
                    else:
                        np_ = (u + 1) if masked else 8
                        for half in range(2):
                            hsl = slice(half * 512, half * 512 + 512)
                            for i in range(np_):
                                nc.tensor.matmul(
                                    po[:, hsl],
                                    lhsT=e[:, 2 * i:2 * i + 2,
                                           u4 * P:(u4 + 1) * P],
                                    rhs=vt[:, 2 * i:2 * i + 2, hsl],
                                    perf_mode=DR, start=(i == 0),
                                    stop=(i == np_ - 1))
                    xr = lnp.tile([P, D], f16, name="xr", tag="xr", bufs=2)
                    nc.scalar.activation(out=xr, in_=po, func=ACT.Copy,
                                         scale=recT[:, u4:u4 + 1])
                    nc.vector.tensor_add(xr, xr, resid_sl(u))
                    # LN core + affine
                    stats = lnp.tile([P, 2, 6], f32, name="stats",
                                     tag="stats")
                    nc.vector.bn_stats(out=stats[:, 0, :], in_=xr[:, 0:512])
                    nc.vector.bn_stats(out=stats[:, 1, :], in_=xr[:, 512:])
                    mv = lnp.tile([P, 2], f32, name="mv", tag="mv")
                    nc.vector.bn_aggr(out=mv, in_=stats)
                    std = lnp.tile([P, 1], f32, name="std", tag="std")
                    nc.scalar.activation(out=std, in_=mv[:, 1:2],
                                         func=ACT.Sqrt, bias=eps, scale=1.0)
                    rstd = lnp.tile([P, 1], f32, name="rstd", tag="rstd")
                    nc.vector.reciprocal(rstd, std)
                    nsl = nout[:, u, :]
                    nc.vector.tensor_scalar(out=nsl, in0=xr,
                                            scalar1=mv[:, 0:1], scalar2=rstd,
                                            op0=ALU.subtract, op1=ALU.mult)
                    t1 = lnp.tile([P, D], f16, name="lt0", tag="lt0", bufs=2)
                    nc.vector.tensor_mul(t1, nsl, gb)
                    nc.gpsimd.tensor_add(yout[:, u, :], t1, bb)

        # ==================== stage B: self-attention + LN1 ===============
        with ExitStack() as stB:
            # prefetch stage-A2 inputs while attention runs
            zpB = stB.enter_context(tc.tile_pool(name="zpB", bufs=1))
            z8 = zpB.tile([P, DTI, S], f8, name="z8", tag="z8")
            nc.sync.dma_start(out=z8, in_=z8_d.ap())
            wk2 = zpB.tile([P, DTI, D], f8, name="wk2", tag="wk2")
            nc.sync.dma_start(out=wk2, in_=w_d["wk2"].ap())
            wv2 = zpB.tile([P, DTI, D], f8, name="wv2", tag="wv2")
            nc.sync.dma_start(out=wv2, in_=w_d["wv2"].ap())
            with ExitStack() as stB2:
                attention(stB2, "sa_", qT, kT, v, lambda u: yres[:, u, :],
                          g1b, b1b, y1, n1, masked=True)
            earlyp.release()
            kvp.release()
            yresp.release()

            # ================ stage A2: cross-attn K/V ====================
            y2rp = tc.alloc_tile_pool(name="y2rp", bufs=1, side="right")
            y2r = y2rp.tile([P, QTI, D], f16, name="y2r", tag="y2r")
            n2p = tc.alloc_tile_pool(name="n2p", bufs=1, side="right")
            n2 = n2p.tile([P, QTI, D], f16, name="n2", tag="n2")
            kv2p = tc.alloc_tile_pool(name="kv2p", bufs=1, side="right")
            kT2 = kv2p.tile([P, DTI, S], f8, name="kT2", tag="kT2")
            v2 = kv2p.tile([P, KTI, D], f8, name="v2", tag="v2")
            with ExitStack() as stA2:
                psA2 = stA2.enter_context(tc.tile_pool(name="psA2", bufs=3,
                                                       space="PSUM"))
                for j in range(DTI):
                    for th in range(2):
                        ps = psA2.tile([P, 1024], f32, name="psp2",
                                       tag="psp2")
                        for sub in range(2):
                            tsl = slice(th * 1024 + sub * 512,
                                        th * 1024 + sub * 512 + 512)
                            dr_acc(ps[:, sub * 512:sub * 512 + 512],
                                   [(wk2, z8)],
                                   lambda t, g, tsl=tsl: t[:, 2 * g:2 * g + 2,
                                                           tsl],
                                   lambda t, g, j=j: t[:, 2 * g:2 * g + 2,
                                                       j * P:(j + 1) * P])
                        osl = kT2[:, j, th * 1024:(th + 1) * 1024]
                        if (j + th) % 2 == 0:
                            nc.scalar.activation(out=osl, in_=ps,
                                                 func=ACT.Copy, scale=IWS)
                        else:
                            nc.vector.tensor_scalar_mul(osl, ps, IWS)
                for t in range(KTI):
                    ps = psA2.tile([P, 1024], f32, name="psp2", tag="psp2")
                    for half in range(2):
                        dr_acc(ps[:, half * 512:half * 512 + 512],
                               [(z8, wv2)],
                               lambda tt, g, half=half: tt[:, 2 * g:2 * g + 2,
                                                           half * 512:half * 512 + 512],
                               lambda tt, g, t=t: tt[:, 2 * g:2 * g + 2,
                                                     t * P:(t + 1) * P])
                    osl = v2[:, t, :]
                    if t % 2 == 0:
                        nc.scalar.activation(out=osl, in_=ps, func=ACT.Copy,
                                             scale=IWS)
                    else:
                        nc.vector.tensor_scalar_mul(osl, ps, IWS)

        # ==================== stage T1: n1 -> n1T (fp8) ===================
        n1Tp = tc.alloc_tile_pool(name="n1Tp", bufs=1)
        n1T = n1Tp.tile([P, DTI, NQ], f8, name="n1T", tag="n1T")
        with ExitStack() as stT1:
            pst = stT1.enter_context(tc.tile_pool(name="pst1", bufs=4,
                                                  space="PSUM"))
            for i in range(DTI):
                for c in range(2):
                    pt = pst.tile([P, 512], f16, name="pt", tag="pt")
                    for u4 in range(4):
                        nc.tensor.transpose(
                            pt[:, u4 * P:(u4 + 1) * P],
                            in_=n1[:, c * 4 + u4, i * P:(i + 1) * P],
                            identity=ident)
                    osl = n1T[:, i, c * 512:c * 512 + 512]
                    if (i + c) % 2 == 0:
                        nc.scalar.activation(out=osl, in_=pt, func=ACT.Copy,
                                             scale=1.0)
                    else:
                        nc.vector.tensor_copy(out=osl, in_=pt)

        # ==================== stage C0: Q2 projection =====================
        qT2p = tc.alloc_tile_pool(name="qT2p", bufs=1, side="right")
        qT2 = qT2p.tile([P, DTI, NQ], f8, name="qT2", tag="qT2")
        with ExitStack() as stC0:
            wpC = stC0.enter_context(tc.tile_pool(name="wpC", bufs=1))
            psC = stC0.enter_context(tc.tile_pool(name="psC", bufs=3,
                                                  space="PSUM"))
            wq2 = wpC.tile([P, DTI, D], f8, name="wq2", tag="wq2")
            nc.sync.dma_start(out=wq2, in_=w_d["wq2"].ap())
            for j in range(DTI):
                ps = psC.tile([P, 1024], f32, name="psq2", tag="psq2")
                for sub in range(2):
                    dr_acc(ps[:, sub * 512:sub * 512 + 512],
                           [(wq2, n1T)],
                           lambda t, g, sub=sub: t[:, 2 * g:2 * g + 2,
                                                   sub * 512:sub * 512 + 512],
                           lambda t, g, j=j: t[:, 2 * g:2 * g + 2,
                                               j * P:(j + 1) * P])
                if j % 2 == 0:
                    nc.scalar.activation(out=qT2[:, j, :], in_=ps,
                                         func=ACT.Identity,
                                         bias=qb2sb[:, j:j + 1], scale=IWS)
                else:
                    nc.vector.tensor_scalar(out=qT2[:, j, :], in0=ps,
                                            scalar1=IWS,
                                            scalar2=qb2sb[:, j:j + 1],
                                            op0=ALU.mult, op1=ALU.add)
        n1Tp.release()
        n1p.release()

        # ==================== stage C: cross-attention + LN2 ==============
        with ExitStack() as stC:
            attention(stC, "ca_", qT2, kT2, v2, lambda u: y1[:, u, :],
                      g2b, b2rb, y2r, n2, masked=False)
        qT2p.release()
        kv2p.release()
        y1p.release()

        # ==================== stage T2: n2 -> n2T hi/lo (fp8) =============
        n2Tp = tc.alloc_tile_pool(name="n2Tp", bufs=1)
        n2T = [n2Tp.tile([P, DTI, NQ], f8, name=f"n2T{x}", tag=f"n2T{x}")
               for x in range(2)]
        with ExitStack() as stT2:
            pst = stT2.enter_context(tc.tile_pool(name="pst2", bufs=4,
                                                  space="PSUM"))
            for i in range(DTI):
                for c in range(2):
                    pt = pst.tile([P, 512], f16, name="pt2", tag="pt2")
                    for u4 in range(4):
                        nc.tensor.transpose(
                            pt[:, u4 * P:(u4 + 1) * P],
                            in_=n2[:, c * 4 + u4, i * P:(i + 1) * P],
                            identity=ident)
                    csl = slice(c * 512, c * 512 + 512)
                    if (i + c) % 2 == 0:
                        nc.scalar.activation(out=n2T[0][:, i, csl], in_=pt,
                                             func=ACT.Copy, scale=1.0)
                    else:
                        nc.vector.tensor_copy(out=n2T[0][:, i, csl], in_=pt)
                    nc.vector.tensor_sub(n2T[1][:, i, csl], pt,
                                         n2T[0][:, i, csl])
        n2p.release()

        # ==================== stage D: FFN + LN3 + output =================
        with ExitStack() as stD:
            wf2p = stD.enter_context(tc.tile_pool(name="wf2p", bufs=1))
            wf1p = stD.enter_context(tc.tile_pool(name="wf1p", bufs=3))
            hp = stD.enter_context(tc.tile_pool(name="hp", bufs=1))
            psH = stD.enter_context(tc.tile_pool(name="psH", bufs=4,
                                                 space="PSUM"))
            psF = stD.enter_context(tc.tile_pool(name="psF", bufs=2,
                                                 space="PSUM"))
            drp = stD.enter_context(tc.tile_pool(name="drpD", bufs=1))
            lnp = stD.enter_context(tc.tile_pool(name="lnpD", bufs=4))
            outp = stD.enter_context(tc.tile_pool(name="outp", bufs=2))
            wf2h = wf2p.tile([P, FTI, D], f8, name="wf2h", tag="wf2h")
            nc.sync.dma_start(out=wf2h, in_=wf2h_d.ap())
            wf2l = wf2p.tile([P, FTI, D], f8, name="wf2l", tag="wf2l")
            nc.sync.dma_start(out=wf2l, in_=wf2l_d.ap())
            for c in range(2):
                csl = slice(c * 512, c * 512 + 512)
                hh = hp.tile([P, FTI, 512], f8, name="hh", tag="hh")
                hl = hp.tile([P, FTI, 512], f8, name="hl", tag="hl")
                for s in range(FTI):
                    w1t = wf1p.tile([P, 2, DTI, P], f8, name="w1t",
                                    tag="w1t")
                    nc.sync.dma_start(out=w1t,
                                      in_=wf1_d.ap()[s * P:(s + 1) * P])
                    ps = psH.tile([P, 512], f32, name="ph", tag="ph")
                    i = 0
                    for wi, xi in [(0, 0), (0, 1), (1, 0)]:
                        for g in range(DTI // 2):
                            nc.tensor.matmul(
                                ps,
                                lhsT=w1t[:, wi, 2 * g:2 * g + 2, :],
                                rhs=n2T[xi][:, 2 * g:2 * g + 2, csl],
                                perf_mode=DR, start=(i == 0),
                                stop=(i == 3 * DTI // 2 - 1))
                            i += 1
                    split3(drp, ps, IWS, bf1sb[:, s:s + 1], ACT.Relu,
                           hh[:, s, :], hl[:, s, :], s % 2, 512)
                for u4 in range(4):
                    u = c * 4 + u4
                    pf = psF.tile([P, 1024], f32, name="pf", tag="pf")
                    usl = slice(u4 * P, (u4 + 1) * P)
                    for half in range(2):
                        hsl = slice(half * 512, half * 512 + 512)
                        i = 0
                        for ha, wb in [(hh, wf2h), (hl, wf2h), (hh, wf2l)]:
                            for sp in range(FTI // 2):
                                nc.tensor.matmul(
                                    pf[:, hsl],
                                    lhsT=ha[:, 2 * sp:2 * sp + 2, usl],
                                    rhs=wb[:, 2 * sp:2 * sp + 2, hsl],
                                    perf_mode=DR, start=(i == 0),
                                    stop=(i == 3 * FTI // 2 - 1))
                                i += 1
                    xr = lnp.tile([P, D], f16, name="xr3", tag="xr3",
                                  bufs=2)
                    nc.vector.tensor_scalar_mul(xr, pf, IWS)
                    nc.vector.tensor_add(xr, xr, y2r[:, u, :])
                    stats = lnp.tile([P, 2, 6], f32, name="st3", tag="st3")
                    nc.vector.bn_stats(out=stats[:, 0, :], in_=xr[:, 0:512])
                    nc.vector.bn_stats(out=stats[:, 1, :], in_=xr[:, 512:])
                    mv = lnp.tile([P, 2], f32, name="mv3", tag="mv3")
                    nc.vector.bn_aggr(out=mv, in_=stats)
                    std = lnp.tile([P, 1], f32, name="std3", tag="std3")
                    nc.scalar.activation(out=std, in_=mv[:, 1:2],
                                         func=ACT.Sqrt, bias=eps, scale=1.0)
                    rstd = lnp.tile([P, 1], f32, name="rstd3", tag="rstd3")
                    nc.vector.reciprocal(rstd, std)
                    n3 = lnp.tile([P, D], f16, name="n3", tag="n3", bufs=2)
                    nc.vector.tensor_scalar(out=n3, in0=xr,
                                            scalar1=mv[:, 0:1], scalar2=rstd,
                                            op0=ALU.subtract, op1=ALU.mult)
                    t1 = lnp.tile([P, D], f16, name="t13", tag="t13",
                                  bufs=2)
                    nc.vector.tensor_mul(t1, n3, g3b)
                    y3 = outp.tile([P, D], f32, name="y3", tag="y3")
                    nc.vector.tensor_add(y3, t1, b3b)
                    nc.sync.dma_start(out=out_d.ap()[u * P:(u + 1) * P, :],
                                      in_=y3)
        n2Tp.release()
        y2rp.release()

    nc.compile()
    return nc


_CACHE = {}


def _get_nc():
    if "nc" not in _CACHE:
        _CACHE["nc"] = build_nc()
    return _CACHE["nc"]


def _q_indices(h):
    """Interleaved q-tile ownership: core-half h owns global tiles h, h+2..."""
    tiles = np.arange(h, 2 * QTI, 2)
    return (tiles[:, None] * P + np.arange(P)[None, :]).reshape(-1)


def _q8(x):
    return np.asarray(x, np.float32).astype(E4NP)


def _q8f(x):
    return _q8(x).astype(np.float32)


def _pack_dT(m):
    """[D, n] (d-major) -> [128, DTI, n] (partition, k-tile, col)."""
    return np.ascontiguousarray(
        m.reshape(DTI, P, -1).transpose(1, 0, 2))


def _hilo(m):
    hi = _q8(m)
    lo = _q8(np.asarray(m, np.float32) - hi.astype(np.float32))
    return hi, lo


def _prep_shared(inp):
    """Weight/vector arrays shared by all cores (host-side prep)."""
    f = lambda k: np.asarray(inp[k], np.float32)
    sh = {}
    for nm, key in [("wq1", "WQ1"), ("wk1", "WK1"), ("wv1", "WV1"),
                    ("wk2", "WK2"), ("wv2", "WV2")]:
        hi, lo = _hilo(WS * f(key))
        sh[nm] = _pack_dT(hi)
        if nm in ("wq1", "wk1", "wv1"):
            sh[nm + "lo"] = _pack_dT(lo)
    # wq2 with LN1 gamma folded; bias = be1 @ WQ2
    wq2p = WS * (f("g1")[:, None] * f("WQ2"))
    sh["wq2"] = _pack_dT(_q8(wq2p))
    sh["qb2"] = np.ascontiguousarray(
        (f("be1") @ f("WQ2")).reshape(DTI, P).T).astype(np.float32)
    # FFN weights: W1 with LN2 gamma folded, hi+lo interleaved; W2 hi+lo
    w1p = WS * (f("g2")[:, None] * f("W_ff1"))
    w1h, w1l = _hilo(w1p)
    w1h = w1h.reshape(DTI, P, FTI, P).transpose(2, 1, 0, 3)
    w1l = w1l.reshape(DTI, P, FTI, P).transpose(2, 1, 0, 3)
    sh["wf1"] = np.ascontiguousarray(
        np.stack([w1h, w1l], axis=2)).reshape(FTI * P, 2, DTI, P)
    w2h, w2l = _hilo(WS * f("W_ff2"))
    sh["wf2h"] = np.ascontiguousarray(
        w2h.reshape(FTI, P, D).transpose(1, 0, 2))
    sh["wf2l"] = np.ascontiguousarray(
        w2l.reshape(FTI, P, D).transpose(1, 0, 2))
    bh = f("be2") @ f("W_ff1") + f("b_ff1")
    sh["bf1"] = np.ascontiguousarray(bh.reshape(FTI, P).T).astype(np.float32)
    sh["g1"] = f("g1").astype(np.float16)
    sh["be1"] = f("be1").astype(np.float16)
    sh["g2"] = f("g2").astype(np.float16)
    sh["b2r"] = (f("be2") + f("b_ff2")).astype(np.float16)
    sh["g3"] = f("g3")
    sh["be3"] = f("be3")
    return sh


def _mask_blocks(h):
    """[128, 8, 512] fp8: row r multiplies score tile t=8c+r (k-partition,
    512 q columns = 4 col-blocks u4).  Block (r, u4) vs q-tile g=8c+2u4+h:
    g>t -> ones, g<t -> zeros, g==t -> causal tri (krow <= qcol)."""
    tri = (np.arange(P)[:, None] <= np.arange(P)[None, :]).astype(np.float32)
    blocks = np.empty((DTI, P, 512), np.float32)
    for r in range(DTI):
        for u4 in range(4):
            cmp = 2 * u4 + h - r
            blk = tri if cmp == 0 else (1.0 if cmp > 0 else 0.0)
            blocks[r, :, u4 * P:(u4 + 1) * P] = blk
    return np.ascontiguousarray(blocks.transpose(1, 0, 2)).astype(E4NP)


def _prep_core(c, y, Z, shared):
    b, h = c // 2, c % 2
    qi = _q_indices(h)
    yb16 = y[b].astype(np.float16)          # [S, D]
    yq16 = yb16[qi]                         # [NQ, D] own queries
    xkvT = yb16.T.astype(np.float32)        # [D, S]
    xqT = yq16.T.astype(np.float32)         # [D, NQ]
    zT = Z[b].astype(np.float16).T.astype(np.float32)
    m = {
        "xq8": _pack_dT(_q8(xqT)),
        "xq0lo": _pack_dT(_q8(xqT[:, 0:P] - _q8f(xqT[:, 0:P]))),
        "xkv8": _pack_dT(_q8(xkvT)),
        "xkvelo": _pack_dT(_q8(xkvT[:, 0:2 * P] - _q8f(xkvT[:, 0:2 * P]))),
        "z8": _pack_dT(_q8(zT)),
        "yres": np.ascontiguousarray(
            yq16.reshape(QTI, P, D).transpose(1, 0, 2)),
        "maskblk": _mask_blocks(h),
    }
    m.update(shared)
    return m


def kernel(**inputs):
    inp = {k: np.asarray(v) for k, v in inputs.items()}
    y = inp["y"].astype(np.float32)
    Z = inp["Z"].astype(np.float32)
    shared = _prep_shared(inp)
    in_maps = [_prep_core(c, y, Z, shared) for c in range(N_CORES)]
    res = run_bass_kernel_spmd(_get_nc(), in_maps, list(range(N_CORES)))
    out = np.zeros((4, 2048, 1024), np.float32)
    for c in range(N_CORES):
        b, h = c // 2, c % 2
        out[b, _q_indices(h)] = res.results[c]["out"]
    return out



# revision 6
# speedup vs baseline: 1.0365x; 1.0365x over previous
"""Trainium2 Bass kernel for a transformer decoder layer (self-attn +
cross-attn + FFN), fp8-e4m3 DoubleRow edition.

Sharding: 8 cores = 4 batches x 2 halves, no collectives. Core h of a batch
owns the interleaved query tiles {h, h+2, ..., h+14} (causal load balance) and
computes the FULL K/V projections for its batch locally (cheaper than the
pair-exchange collective at fp8 speeds).

Numerics: all matmuls run in fp8-e4m3 with DoubleRow perf mode (2 contraction
rows per partition).  Weights are pre-scaled x32 host-side so they sit in
e4m3's normal range; every PSUM drain folds the 1/32 back in.  Three
refinements keep absmax rel err ~3e-3 (gate is 2e-2):
  - FFN: both matmuls use hi+lo fp8 splits of activations AND weights
    (3 DoubleRow matmuls per logical matmul = fp16-level accuracy at 2x
    fp16 speed).
  - Early causal tokens (global positions 0..255, each core's local q-tile
    u=0) see few keys, so fp8 noise doesn't average out: their Q/K/V/E values
    are computed via the same hi+lo corrected path.
  - LayerNorm gammas/betas are folded into the next matmul's weights where
    possible (WQ2, W_ff1) and the residual carriers keep f16 precision.

Causal masking is via per-core precomputed [128 x 512] mask rows (tri/ones/
zeros blocks depending on core half), applied to each self-attn score tile,
so the SPMD program is uniform across cores.

Stage order: A1 (self K/V/Q + early) -> B (self-attn, prefetching A2 inputs)
-> A2 (cross K/V) -> T1 (n1 transpose) -> C0 (Q2) -> C (cross-attn) ->
T2 (n2 transpose hi/lo) -> D (FFN, token-chunked, streamed W_ff1).
"""

from contextlib import ExitStack

import ml_dtypes
import numpy as np

import concourse.bass as bass
import concourse.mybir as mybir
import concourse.tile as tile
from concourse import bacc
from concourse.bass_utils import run_bass_kernel_spmd
from concourse.masks import make_identity

f32 = mybir.dt.float32
f16 = mybir.dt.float16
f8 = mybir.dt.float8e4

P = 128
D = 1024          # d_model
S = 2048          # kv sequence length
NQ = 1024         # query tokens per core
DFF = 4096
DTI = D // P      # 8 d-model partition tiles
KTI = S // P      # 16 kv token tiles
QTI = NQ // P     # 8 query tiles
FTI = DFF // P    # 32 d_ff tiles
ACT = mybir.ActivationFunctionType
ALU = mybir.AluOpType
DR = mybir.MatmulPerfMode.DoubleRow
N_CORES = 8
WS = 32.0         # host-side weight pre-scale
IWS = 1.0 / WS
SCALE = 1.0 / 32.0  # 1/sqrt(D) softmax scale
E4NP = ml_dtypes.float8_e4m3


def build_nc():
    nc = bacc.Bacc("TRN2", target_bir_lowering=False, debug=False,
                   num_devices=N_CORES)

    def dp(name, shape, dt, out=False):
        return nc.declare_dram_parameter(name, shape, dt, isOutput=out)

    xq8_d = dp("xq8", [P, DTI, NQ], f8)
    xq0lo_d = dp("xq0lo", [P, DTI, P], f8)
    xkv8_d = dp("xkv8", [P, DTI, S], f8)
    xkvelo_d = dp("xkvelo", [P, DTI, 2 * P], f8)
    z8_d = dp("z8", [P, DTI, S], f8)
    yres_d = dp("yres", [P, QTI, D], f16)
    w_d = {n: dp(n, [P, DTI, D], f8)
           for n in ["wq1", "wk1", "wv1", "wq2", "wk2", "wv2",
                     "wq1lo", "wk1lo", "wv1lo"]}
    wf1_d = dp("wf1", [FTI * P, 2, DTI, P], f8)   # hi/lo interleaved
    wf2h_d = dp("wf2h", [P, FTI, D], f8)
    wf2l_d = dp("wf2l", [P, FTI, D], f8)
    bf1_d = dp("bf1", [P, FTI], f32)
    qb2_d = dp("qb2", [P, DTI], f32)
    mask_d = dp("maskblk", [P, DTI, P], f8)
    v16_d = {n: dp(n, [D], f16) for n in ["g1", "be1", "g2", "b2r"]}
    v32_d = {n: dp(n, [D], f32) for n in ["g3", "be3"]}
    out_d = dp("out", [NQ, D], f32, out=True)

    def bc(ap):  # broadcast a [n] dram vector across 128 partitions
        return bass.AP(tensor=ap.tensor, offset=ap.offset,
                       ap=[[0, P]] + [list(x) for x in ap.ap])

    with tile.TileContext(nc) as tc, ExitStack() as top:
        const = top.enter_context(tc.tile_pool(name="const", bufs=1))
        ident = const.tile([P, P], f16, name="ident", tag="ident")
        make_identity(nc, ident)
        masks = const.tile([P, DTI, P], f8, name="masks", tag="masks")
        ones8t = const.tile([P, 2, 16], f8, name="ones8", tag="ones8")
        nc.vector.memset(ones8t, 1.0)
        ones8 = ones8t[:, :, 0:1]  # outer step 16B: dual-fp8 ldweights rule
        eps = const.tile([P, 1], f32, name="eps", tag="eps")
        nc.vector.memset(eps, 1e-5)
        bf1sb = const.tile([P, FTI], f32, name="bf1sb", tag="bf1sb")
        qb2sb = const.tile([P, DTI], f32, name="qb2sb", tag="qb2sb")

        def vload(name, dt, dram):
            return const.tile([P, D], dt, name=f"{name}b", tag=f"{name}b")

        g1b = vload("g1", f16, v16_d)
        b1b = vload("be1", f16, v16_d)
        g2b = vload("g2", f16, v16_d)
        b2rb = vload("b2r", f16, v16_d)
        g3b = vload("g3", f32, v32_d)
        b3b = vload("be3", f32, v32_d)

        def load_consts():
            # deferred off-critical-path constant loads
            nc.sync.dma_start(out=masks, in_=mask_d.ap())
            nc.sync.dma_start(out=bf1sb, in_=bf1_d.ap())
            nc.sync.dma_start(out=qb2sb, in_=qb2_d.ap())
            for t, nm, dd in [(g1b, "g1", v16_d), (b1b, "be1", v16_d),
                              (g2b, "g2", v16_d), (b2rb, "b2r", v16_d),
                              (g3b, "g3", v32_d), (b3b, "be3", v32_d)]:
                nc.sync.dma_start(out=t, in_=bc(dd[nm].ap()))

        # ---- persistent pools; LIFO per side ----
        # left: y1, n1, [zpB/wpB], [n1T], [n2T]
        # right: yres, kvp, earlyp | y2r, n2, kv2p, qT2p
        y1p = tc.alloc_tile_pool(name="y1p", bufs=1)
        y1 = y1p.tile([P, QTI, D], f16, name="y1", tag="y1")
        n1p = tc.alloc_tile_pool(name="n1p", bufs=1)
        n1 = n1p.tile([P, QTI, D], f16, name="n1", tag="n1")

        yresp = tc.alloc_tile_pool(name="yresp", bufs=1, side="right")
        yres = yresp.tile([P, QTI, D], f16, name="yres", tag="yres")
        kvp = tc.alloc_tile_pool(name="kvp", bufs=1, side="right")
        kT = kvp.tile([P, DTI, S], f8, name="kT", tag="kT")
        v = kvp.tile([P, KTI, D], f8, name="v", tag="v")
        qT = kvp.tile([P, DTI, NQ], f8, name="qT", tag="qT")
        earlyp = tc.alloc_tile_pool(name="earlyp", bufs=1, side="right")
        keT = [earlyp.tile([P, DTI, 2 * P], f8, name=f"keT{x}", tag=f"keT{x}")
               for x in range(2)]  # hi, lo
        qeT = [earlyp.tile([P, DTI, P], f8, name=f"qeT{x}", tag=f"qeT{x}")
               for x in range(2)]
        ve = [earlyp.tile([P, 2, D], f8, name=f"ve{x}", tag=f"ve{x}")
              for x in range(2)]

        def dr_acc(ps, terms, rhs_sl, lhs_sl):
            """Accumulate sum of DoubleRow products into psum region ps.
            terms: list of (lhsT_tile, rhs_tile); contraction over DTI//2
            k-tile pairs per term. rhs_sl/lhs_sl: fn(tile, g) -> AP."""
            n = len(terms) * (DTI // 2)
            i = 0
            for lt, rt in terms:
                for g in range(DTI // 2):
                    nc.tensor.matmul(ps, lhsT=lhs_sl(lt, g),
                                     rhs=rhs_sl(rt, g), perf_mode=DR,
                                     start=(i == 0), stop=(i == n - 1))
                    i += 1

        def split3(pool, ps, scale, bias, func, hi_out, lo_out, eng, n):
            """3-op hi/lo drain: t16 = func(scale*ps + bias); hi = q8(t16);
            lo = q8(t16 - hi)."""
            t16 = pool.tile([P, n], f16, name="t16", tag="t16", bufs=3)
            nc.scalar.activation(out=t16, in_=ps, func=func, bias=bias,
                                 scale=scale)
            if eng == 0:
                nc.vector.tensor_copy(out=hi_out, in_=t16)
                nc.gpsimd.tensor_sub(lo_out, t16, hi_out)
            else:
                nc.gpsimd.tensor_copy(out=hi_out, in_=t16)
                nc.vector.tensor_sub(lo_out, t16, hi_out)

        # ==================== stage A1: self-attn projections =============
        with ExitStack() as stA:
            wpA = stA.enter_context(tc.tile_pool(name="wpA", bufs=1))
            xpA = stA.enter_context(tc.tile_pool(name="xpA", bufs=1))
            psA = stA.enter_context(tc.tile_pool(name="psA", bufs=3,
                                                 space="PSUM"))
            psE = stA.enter_context(tc.tile_pool(name="psE", bufs=2,
                                                 space="PSUM"))
            drp = stA.enter_context(tc.tile_pool(name="drpA", bufs=1))

            def wload(tag, name):
                t = wpA.tile([P, DTI, D], f8, name=name, tag=tag)
                nc.sync.dma_start(out=t, in_=w_d[name].ap())
                return t

            wk1 = wload("wA0", "wk1")
            xkv8 = xpA.tile([P, DTI, S], f8, name="xkv8", tag="xkv8")
            nc.sync.dma_start(out=xkv8, in_=xkv8_d.ap())
            wv1 = wload("wA1", "wv1")
            wk1lo = wload("wA2", "wk1lo")
            wv1lo = wload("wA3", "wv1lo")
            xkvelo = xpA.tile([P, DTI, 2 * P], f8, name="xkvelo",
                              tag="xkvelo")
            nc.sync.dma_start(out=xkvelo, in_=xkvelo_d.ap())
            xq8 = xpA.tile([P, DTI, NQ], f8, name="xq8", tag="xq8")
            nc.sync.dma_start(out=xq8, in_=xq8_d.ap())
            xq0lo = xpA.tile([P, DTI, P], f8, name="xq0lo", tag="xq0lo")
            nc.sync.dma_start(out=xq0lo, in_=xq0lo_d.ap())
            nc.sync.dma_start(out=yres, in_=yres_d.ap())
            load_consts()

            # K1: kT[:, j, :] = (wk1.T @ xkv)/32, d_out on partitions
            for j in range(DTI):
                for th in range(2):
                    ps = psA.tile([P, 1024], f32, name="psp", tag="psp")
                    for sub in range(2):
                        tsl = slice(th * 1024 + sub * 512,
                                    th * 1024 + sub * 512 + 512)
                        dr_acc(ps[:, sub * 512:sub * 512 + 512],
                               [(wk1, xkv8)],
                               lambda t, g, tsl=tsl: t[:, 2 * g:2 * g + 2, tsl],
                               lambda t, g, j=j: t[:, 2 * g:2 * g + 2,
                                                   j * P:(j + 1) * P])
                    osl = kT[:, j, th * 1024:(th + 1) * 1024]
                    if (j + th) % 2 == 0:
                        nc.scalar.activation(out=osl, in_=ps, func=ACT.Copy,
                                             scale=IWS)
                    else:
                        nc.vector.tensor_scalar_mul(osl, ps, IWS)

            # early K (tokens 0:256), hi+lo corrected
            for j in range(DTI):
                ps = psE.tile([P, 512], f32, name="pse", tag="pse")
                dr_acc(ps[:, 0:256],
                       [(wk1, xkv8), (wk1, xkvelo), (wk1lo, xkv8)],
                       lambda t, g: (t[:, 2 * g:2 * g + 2, 0:256]
                                     if t is xkv8 else
                                     t[:, 2 * g:2 * g + 2, :]),
                       lambda t, g, j=j: t[:, 2 * g:2 * g + 2,
                                           j * P:(j + 1) * P])
                split3(drp, ps[:, 0:256], IWS, 0.0, ACT.Copy,
                       keT[0][:, j, :], keT[1][:, j, :], j % 2, 256)

            # V1: v[:, t, :] = (xkv.T @ wv1)/32, tokens on partitions
            for t in range(KTI):
                ps = psA.tile([P, 1024], f32, name="psp", tag="psp")
                for half in range(2):
                    dr_acc(ps[:, half * 512:half * 512 + 512],
                           [(xkv8, wv1)],
                           lambda tt, g, half=half: tt[:, 2 * g:2 * g + 2,
                                                       half * 512:half * 512 + 512],
                           lambda tt, g, t=t: tt[:, 2 * g:2 * g + 2,
                                                 t * P:(t + 1) * P])
                osl = v[:, t, :]
                if t % 2 == 0:
                    nc.scalar.activation(out=osl, in_=ps, func=ACT.Copy,
                                         scale=IWS)
                else:
                    nc.vector.tensor_scalar_mul(osl, ps, IWS)

            # early V (k-tiles 0..1), hi+lo corrected
            for t in range(2):
                for half in range(2):
                    ps = psE.tile([P, 512], f32, name="pse", tag="pse")
                    hsl = slice(half * 512, half * 512 + 512)
                    dr_acc(ps,
                           [(xkv8, wv1), (xkvelo, wv1), (xkv8, wv1lo)],
                           lambda tt, g, hsl=hsl: tt[:, 2 * g:2 * g + 2, hsl],
                           lambda tt, g, t=t: tt[:, 2 * g:2 * g + 2,
                                                 t * P:(t + 1) * P])
                    split3(drp, ps, IWS, 0.0, ACT.Copy,
                           ve[0][:, t, hsl], ve[1][:, t, hsl],
                           (t + half) % 2, 512)

            # Q1 (weights reuse the K1 buffers)
            wq1 = wload("wA0", "wq1")
            wq1lo = wload("wA2", "wq1lo")
            for j in range(DTI):
                ps = psA.tile([P, 1024], f32, name="psp", tag="psp")
                for sub in range(2):
                    dr_acc(ps[:, sub * 512:sub * 512 + 512],
                           [(wq1, xq8)],
                           lambda t, g, sub=sub: t[:, 2 * g:2 * g + 2,
                                                   sub * 512:sub * 512 + 512],
                           lambda t, g, j=j: t[:, 2 * g:2 * g + 2,
                                               j * P:(j + 1) * P])
                osl = qT[:, j, :]
                if j % 2 == 0:
                    nc.scalar.activation(out=osl, in_=ps, func=ACT.Copy,
                                         scale=IWS)
                else:
                    nc.vector.tensor_scalar_mul(osl, ps, IWS)
            # early Q (own u=0 tile)
            for j in range(DTI):
                ps = psE.tile([P, 512], f32, name="pse", tag="pse")
                dr_acc(ps[:, 0:P],
                       [(wq1, xq8), (wq1, xq0lo), (wq1lo, xq8)],
                       lambda t, g: (t[:, 2 * g:2 * g + 2, 0:P]
                                     if t is xq8 else
                                     t[:, 2 * g:2 * g + 2, :]),
                       lambda t, g, j=j: t[:, 2 * g:2 * g + 2,
                                           j * P:(j + 1) * P])
                split3(drp, ps[:, 0:P], IWS, 0.0, ACT.Copy,
                       qeT[0][:, j, :], qeT[1][:, j, :], j % 2, P)

        # ==================== attention helper ============================
        def attention(stk, tagp, qTt, kTt, vt, resid_sl, gb, bb, yout, nout,
                      masked):
            pss = stk.enter_context(tc.tile_pool(name=f"{tagp}pss", bufs=2,
                                                 space="PSUM"))
            pso = stk.enter_context(tc.tile_pool(name=f"{tagp}pso", bufs=2,
                                                 space="PSUM"))
            ep = stk.enter_context(tc.tile_pool(name=f"{tagp}ep", bufs=2))
            lnp = stk.enter_context(tc.tile_pool(name=f"{tagp}lnp", bufs=4))
            for c in range(2):
                e = ep.tile([P, KTI, 512], f8, name="e", tag="e")
                nvis = 8 * (c + 1) if masked else KTI
                ee = None
                if masked and c == 0:
                    # early corrected scores/E for q-tile u=0, k-tiles 0..1
                    # (emitted first so its long drain chain overlaps the
                    # main score tiles)
                    ee = lnp.tile([P, 2, 2, P], f8, name="ee", tag="ee",
                                  bufs=1)
                    for t in range(2):
                        ps = pss.tile([P, 1024], f32, name="ps_s", tag="ps_s")
                        dr_acc(ps[:, 0:P],
                               [(keT[0], qeT[0]), (keT[0], qeT[1]),
                                (keT[1], qeT[0])],
                               lambda tt, g: tt[:, 2 * g:2 * g + 2, :],
                               lambda tt, g, t=t: tt[:, 2 * g:2 * g + 2,
                                                     t * P:(t + 1) * P])
                        tm = lnp.tile([P, P], f16, name="etm", tag="etm",
                                      bufs=2)
                        nc.scalar.activation(out=tm, in_=ps[:, 0:P],
                                             func=ACT.Exp, scale=SCALE)
                        nc.vector.tensor_mul(tm, tm, masks[:, t, :])
                        nc.vector.tensor_copy(out=ee[:, 0, t, :], in_=tm)
                        nc.gpsimd.tensor_sub(ee[:, 1, t, :], tm,
                                             ee[:, 0, t, :])
                # scores + exp, two k-tiles per psum tile / exp instruction;
                # causal mask only touches the boundary q-block of each tile
                # (hidden non-boundary blocks are never read downstream)
                for dual in range(nvis // 2):
                    ps = pss.tile([P, 1024], f32, name="ps_s", tag="ps_s")
                    for k in range(2):
                        t = 2 * dual + k
                        dr_acc(ps[:, k * 512:k * 512 + 512], [(kTt, qTt)],
                               lambda tt, g, c=c: tt[:, 2 * g:2 * g + 2,
                                                     c * 512:c * 512 + 512],
                               lambda tt, g, t=t: tt[:, 2 * g:2 * g + 2,
                                                     t * P:(t + 1) * P])
                    nc.scalar.activation(out=e[:, 2 * dual:2 * dual + 2, :],
                                         in_=ps, func=ACT.Exp, scale=SCALE)
                    if masked:
                        for k in range(2):
                            t = 2 * dual + k
                            if t < 8 * c:
                                continue
                            r = t - 8 * c
                            u4b = r // 2
                            esl = e[:, t, u4b * P:(u4b + 1) * P]
                            if r % 2 == 0:
                                nc.vector.tensor_mul(esl, esl, masks[:, r, :])
                            else:
                                nc.gpsimd.tensor_mul(esl, esl, masks[:, r, :])
                # denominators: E^T @ ones -> [128 q, 1] per u4 column of a
                # psum tile (q on partitions; no DRAM transpose round-trip)
                pd = pso.tile([P, 1024], f32, name="pd", tag="po")
                for u4 in range(4):
                    if ee is not None and u4 == 0:
                        for hl in range(2):
                            nc.tensor.matmul(pd[:, 0:1],
                                             lhsT=ee[:, hl, :, :],
                                             rhs=ones8, perf_mode=DR,
                                             start=(hl == 0), stop=(hl == 1))
                        continue
                    np_ = (4 * c + u4 + 1) if masked else 8
                    for i in range(np_):
                        nc.tensor.matmul(
                            pd[:, u4:u4 + 1],
                            lhsT=e[:, 2 * i:2 * i + 2, u4 * P:(u4 + 1) * P],
                            rhs=ones8, perf_mode=DR,
                            start=(i == 0), stop=(i == np_ - 1))
                recT = lnp.tile([P, 4], f32, name="recT", tag="recT")
                nc.vector.reciprocal(recT, pd[:, 0:4])
                u4order = [1, 2, 3, 0] if ee is not None else range(4)
                for u4 in u4order:
                    u = c * 4 + u4
                    po = pso.tile([P, 1024], f32, name="po", tag="po")
                    if ee is not None and u == 0:
                        for half in range(2):
                            hsl = slice(half * 512, half * 512 + 512)
                            for ti, (el, vl) in enumerate(
                                    [(0, 0), (1, 0), (0, 1)]):
                                nc.tensor.matmul(
                                    po[:, hsl], lhsT=ee[:, el, :, :],
                                    rhs=ve[vl][:, :, hsl], perf_mode=DR,
                                    start=(ti == 0), stop=(ti == 2))
                    else:
                        np_ = (u + 1) if masked else 8
                        for half in range(2):
                            hsl = slice(half * 512, half * 512 + 512)
                            for i in range(np_):
                                nc.tensor.matmul(
                                    po[:, hsl],
                                    lhsT=e[:, 2 * i:2 * i + 2,
                                           u4 * P:(u4 + 1) * P],
                                    rhs=vt[:, 2 * i:2 * i + 2, hsl],
                                    perf_mode=DR, start=(i == 0),
                                    stop=(i == np_ - 1))
                    xr = lnp.tile([P, D], f16, name="xr", tag="xr", bufs=2)
                    nc.scalar.activation(out=xr, in_=po, func=ACT.Copy,
                                         scale=recT[:, u4:u4 + 1])
                    nc.vector.tensor_add(xr, xr, resid_sl(u))
                    # LN core + affine
                    stats = lnp.tile([P, 2, 6], f32, name="stats",
                                     tag="stats")
                    nc.vector.bn_stats(out=stats[:, 0, :], in_=xr[:, 0:512])
                    nc.vector.bn_stats(out=stats[:, 1, :], in_=xr[:, 512:])
                    mv = lnp.tile([P, 2], f32, name="mv", tag="mv")
                    nc.vector.bn_aggr(out=mv, in_=stats)
                    std = lnp.tile([P, 1], f32, name="std", tag="std")
                    nc.scalar.activation(out=std, in_=mv[:, 1:2],
                                         func=ACT.Sqrt, bias=eps, scale=1.0)
                    rstd = lnp.tile([P, 1], f32, name="rstd", tag="rstd")
                    nc.vector.reciprocal(rstd, std)
                    nsl = nout[:, u, :]
                    nc.vector.tensor_scalar(out=nsl, in0=xr,
                                            scalar1=mv[:, 0:1], scalar2=rstd,
                                            op0=ALU.subtract, op1=ALU.mult)
                    t1 = lnp.tile([P, D], f16, name="lt0", tag="lt0", bufs=2)
                    nc.vector.tensor_mul(t1, nsl, gb)
                    nc.gpsimd.tensor_add(yout[:, u, :], t1, bb)

        # ==================== stage B: self-attention + LN1 ===============
        with ExitStack() as stB:
            # prefetch stage-A2 inputs while attention runs
            zpB = stB.enter_context(tc.tile_pool(name="zpB", bufs=1))
            z8 = zpB.tile([P, DTI, S], f8, name="z8", tag="z8")
            nc.sync.dma_start(out=z8, in_=z8_d.ap())
            wk2 = zpB.tile([P, DTI, D], f8, name="wk2", tag="wk2")
            nc.sync.dma_start(out=wk2, in_=w_d["wk2"].ap())
            wv2 = zpB.tile([P, DTI, D], f8, name="wv2", tag="wv2")
            nc.sync.dma_start(out=wv2, in_=w_d["wv2"].ap())
            with ExitStack() as stB2:
                attention(stB2, "sa_", qT, kT, v, lambda u: yres[:, u, :],
                          g1b, b1b, y1, n1, masked=True)
            earlyp.release()
            kvp.release()
            yresp.release()

            # ================ stage A2: cross-attn K/V ====================
            y2rp = tc.alloc_tile_pool(name="y2rp", bufs=1, side="right")
            y2r = y2rp.tile([P, QTI, D], f16, name="y2r", tag="y2r")
            n2p = tc.alloc_tile_pool(name="n2p", bufs=1, side="right")
            n2 = n2p.tile([P, QTI, D], f16, name="n2", tag="n2")
            kv2p = tc.alloc_tile_pool(name="kv2p", bufs=1, side="right")
            kT2 = kv2p.tile([P, DTI, S], f8, name="kT2", tag="kT2")
            v2 = kv2p.tile([P, KTI, D], f8, name="v2", tag="v2")
            with ExitStack() as stA2:
                psA2 = stA2.enter_context(tc.tile_pool(name="psA2", bufs=3,
                                                       space="PSUM"))
                for j in range(DTI):
                    for th in range(2):
                        ps = psA2.tile([P, 1024], f32, name="psp2",
                                       tag="psp2")
                        for sub in range(2):
                            tsl = slice(th * 1024 + sub * 512,
                                        th * 1024 + sub * 512 + 512)
                            dr_acc(ps[:, sub * 512:sub * 512 + 512],
                                   [(wk2, z8)],
                                   lambda t, g, tsl=tsl: t[:, 2 * g:2 * g + 2,
                                                           tsl],
                                   lambda t, g, j=j: t[:, 2 * g:2 * g + 2,
                                                       j * P:(j + 1) * P])
                        osl = kT2[:, j, th * 1024:(th + 1) * 1024]
                        if (j + th) % 2 == 0:
                            nc.scalar.activation(out=osl, in_=ps,
                                                 func=ACT.Copy, scale=IWS)
                        else:
                            nc.vector.tensor_scalar_mul(osl, ps, IWS)
                for t in range(KTI):
                    ps = psA2.tile([P, 1024], f32, name="psp2", tag="psp2")
                    for half in range(2):
                        dr_acc(ps[:, half * 512:half * 512 + 512],
                               [(z8, wv2)],
                               lambda tt, g, half=half: tt[:, 2 * g:2 * g + 2,
                                                           half * 512:half * 512 + 512],
                               lambda tt, g, t=t: tt[:, 2 * g:2 * g + 2,
                                                     t * P:(t + 1) * P])
                    osl = v2[:, t, :]
                    if t % 2 == 0:
                        nc.scalar.activation(out=osl, in_=ps, func=ACT.Copy,
                                             scale=IWS)
                    else:
                        nc.vector.tensor_scalar_mul(osl, ps, IWS)

        # ==================== stage T1: n1 -> n1T (fp8) ===================
        n1Tp = tc.alloc_tile_pool(name="n1Tp", bufs=1)
        n1T = n1Tp.tile([P, DTI, NQ], f8, name="n1T", tag="n1T")
        with ExitStack() as stT1:
            pst = stT1.enter_context(tc.tile_pool(name="pst1", bufs=4,
                                                  space="PSUM"))
            for i in range(DTI):
                for c in range(2):
                    pt = pst.tile([P, 512], f16, name="pt", tag="pt")
                    for u4 in range(4):
                        nc.tensor.transpose(
                            pt[:, u4 * P:(u4 + 1) * P],
                            in_=n1[:, c * 4 + u4, i * P:(i + 1) * P],
                            identity=ident)
                    osl = n1T[:, i, c * 512:c * 512 + 512]
                    if (i + c) % 2 == 0:
                        nc.scalar.activation(out=osl, in_=pt, func=ACT.Copy,
                                             scale=1.0)
                    else:
                        nc.vector.tensor_copy(out=osl, in_=pt)

        # ==================== stage C0: Q2 projection =====================
        qT2p = tc.alloc_tile_pool(name="qT2p", bufs=1, side="right")
        qT2 = qT2p.tile([P, DTI, NQ], f8, name="qT2", tag="qT2")
        with ExitStack() as stC0:
            wpC = stC0.enter_context(tc.tile_pool(name="wpC", bufs=1))
            psC = stC0.enter_context(tc.tile_pool(name="psC", bufs=3,
                                                  space="PSUM"))
            wq2 = wpC.tile([P, DTI, D], f8, name="wq2", tag="wq2")
            nc.sync.dma_start(out=wq2, in_=w_d["wq2"].ap())
            for j in range(DTI):
                ps = psC.tile([P, 1024], f32, name="psq2", tag="psq2")
                for sub in range(2):
                    dr_acc(ps[:, sub * 512:sub * 512 + 512],
                           [(wq2, n1T)],
                           lambda t, g, sub=sub: t[:, 2 * g:2 * g + 2,
                                                   sub * 512:sub * 512 + 512],
                           lambda t, g, j=j: t[:, 2 * g:2 * g + 2,
                                               j * P:(j + 1) * P])
                if j % 2 == 0:
                    nc.scalar.activation(out=qT2[:, j, :], in_=ps,
                                         func=ACT.Identity,
                                         bias=qb2sb[:, j:j + 1], scale=IWS)
                else:
                    nc.vector.tensor_scalar(out=qT2[:, j, :], in0=ps,
                                            scalar1=IWS,
                                            scalar2=qb2sb[:, j:j + 1],
                                            op0=ALU.mult, op1=ALU.add)
        n1Tp.release()
        n1p.release()

        # ==================== stage C: cross-attention + LN2 ==============
        with ExitStack() as stC:
            attention(stC, "ca_", qT2, kT2, v2, lambda u: y1[:, u, :],
                      g2b, b2rb, y2r, n2, masked=False)
        qT2p.release()
        kv2p.release()
        y1p.release()

        # ==================== stage T2: n2 -> n2T hi/lo (fp8) =============
        n2Tp = tc.alloc_tile_pool(name="n2Tp", bufs=1)
        n2T = [n2Tp.tile([P, DTI, NQ], f8, name=f"n2T{x}", tag=f"n2T{x}")
               for x in range(2)]
        with ExitStack() as stT2:
            pst = stT2.enter_context(tc.tile_pool(name="pst2", bufs=4,
                                                  space="PSUM"))
            for i in range(DTI):
                for c in range(2):
                    pt = pst.tile([P, 512], f16, name="pt2", tag="pt2")
                    for u4 in range(4):
                        nc.tensor.transpose(
                            pt[:, u4 * P:(u4 + 1) * P],
                            in_=n2[:, c * 4 + u4, i * P:(i + 1) * P],
                            identity=ident)
                    csl = slice(c * 512, c * 512 + 512)
                    if (i + c) % 2 == 0:
                        nc.scalar.activation(out=n2T[0][:, i, csl], in_=pt,
                                             func=ACT.Copy, scale=1.0)
                    else:
                        nc.vector.tensor_copy(out=n2T[0][:, i, csl], in_=pt)
                    nc.vector.tensor_sub(n2T[1][:, i, csl], pt,
                                         n2T[0][:, i, csl])
        n2p.release()

        # ==================== stage D: FFN + LN3 + output =================
        with ExitStack() as stD:
            wf2p = stD.enter_context(tc.tile_pool(name="wf2p", bufs=1))
            wf1p = stD.enter_context(tc.tile_pool(name="wf1p", bufs=3))
            hp = stD.enter_context(tc.tile_pool(name="hp", bufs=1))
            psH = stD.enter_context(tc.tile_pool(name="psH", bufs=4,
                                                 space="PSUM"))
            psF = stD.enter_context(tc.tile_pool(name="psF", bufs=2,
                                                 space="PSUM"))
            drp = stD.enter_context(tc.tile_pool(name="drpD", bufs=1))
            lnp = stD.enter_context(tc.tile_pool(name="lnpD", bufs=4))
            outp = stD.enter_context(tc.tile_pool(name="outp", bufs=2))
            wf2h = wf2p.tile([P, FTI, D], f8, name="wf2h", tag="wf2h")
            nc.sync.dma_start(out=wf2h, in_=wf2h_d.ap())
            wf2l = wf2p.tile([P, FTI, D], f8, name="wf2l", tag="wf2l")
            nc.sync.dma_start(out=wf2l, in_=wf2l_d.ap())
            for c in range(2):
                csl = slice(c * 512, c * 512 + 512)
                hh = hp.tile([P, FTI, 512], f8, name="hh", tag="hh")
                hl = hp.tile([P, FTI, 512], f8, name="hl", tag="hl")
                for s in range(FTI):
                    w1t = wf1p.tile([P, 2, DTI, P], f8, name="w1t",
                                    tag="w1t")
                    nc.sync.dma_start(out=w1t,
                                      in_=wf1_d.ap()[s * P:(s + 1) * P])
                    ps = psH.tile([P, 512], f32, name="ph", tag="ph")
                    i = 0
                    for wi, xi in [(0, 0), (0, 1), (1, 0)]:
                        for g in range(DTI // 2):
                            nc.tensor.matmul(
                                ps,
                                lhsT=w1t[:, wi, 2 * g:2 * g + 2, :],
                                rhs=n2T[xi][:, 2 * g:2 * g + 2, csl],
                                perf_mode=DR, start=(i == 0),
                                stop=(i == 3 * DTI // 2 - 1))
                            i += 1
                    split3(drp, ps, IWS, bf1sb[:, s:s + 1], ACT.Relu,
                           hh[:, s, :], hl[:, s, :], s % 2, 512)
                for u4 in range(4):
                    u = c * 4 + u4
                    pf = psF.tile([P, 1024], f32, name="pf", tag="pf")
                    usl = slice(u4 * P, (u4 + 1) * P)
                    for half in range(2):
                        hsl = slice(half * 512, half * 512 + 512)
                        i = 0
                        for ha, wb in [(hh, wf2h), (hl, wf2h), (hh, wf2l)]:
                            for sp in range(FTI // 2):
                                nc.tensor.matmul(
                                    pf[:, hsl],
                                    lhsT=ha[:, 2 * sp:2 * sp + 2, usl],
                                    rhs=wb[:, 2 * sp:2 * sp + 2, hsl],
                                    perf_mode=DR, start=(i == 0),
                                    stop=(i == 3 * FTI // 2 - 1))
                                i += 1
                    xr = lnp.tile([P, D], f16, name="xr3", tag="xr3",
                                  bufs=2)
                    nc.vector.tensor_scalar_mul(xr, pf, IWS)
                    nc.vector.tensor_add(xr, xr, y2r[:, u, :])
                    stats = lnp.tile([P, 2, 6], f32, name="st3", tag="st3")
                    nc.vector.bn_stats(out=stats[:, 0, :], in_=xr[:, 0:512])
                    nc.vector.bn_stats(out=stats[:, 1, :], in_=xr[:, 512:])
                    mv = lnp.tile([P, 2], f32, name="mv3", tag="mv3")
                    nc.vector.bn_aggr(out=mv, in_=stats)
                    std = lnp.tile([P, 1], f32, name="std3", tag="std3")
                    nc.scalar.activation(out=std, in_=mv[:, 1:2],
                                         func=ACT.Sqrt, bias=eps, scale=1.0)
                    rstd = lnp.tile([P, 1], f32, name="rstd3", tag="rstd3")
                    nc.vector.reciprocal(rstd, std)
                    n3 = lnp.tile([P, D], f16, name="n3", tag="n3", bufs=2)
                    nc.vector.tensor_scalar(out=n3, in0=xr,
                                            scalar1=mv[:, 0:1], scalar2=rstd,
                                            op0=ALU.subtract, op1=ALU.mult)
                    t1 = lnp.tile([P, D], f16, name="t13", tag="t13",
                                  bufs=2)
                    nc.vector.tensor_mul(t1, n3, g3b)
                    y3 = outp.tile([P, D], f32, name="y3", tag="y3")
                    nc.vector.tensor_add(y3, t1, b3b)
                    nc.sync.dma_start(out=out_d.ap()[u * P:(u + 1) * P, :],
                                      in_=y3)
        n2Tp.release()
        y2rp.release()

    nc.compile()
    return nc


_CACHE = {}


def _get_nc():
    if "nc" not in _CACHE:
        _CACHE["nc"] = build_nc()
    return _CACHE["nc"]


def _q_indices(h):
    """Interleaved q-tile ownership: core-half h owns global tiles h, h+2..."""
    tiles = np.arange(h, 2 * QTI, 2)
    return (tiles[:, None] * P + np.arange(P)[None, :]).reshape(-1)


def _q8(x):
    return np.asarray(x, np.float32).astype(E4NP)


def _q8f(x):
    return _q8(x).astype(np.float32)


def _pack_dT(m):
    """[D, n] (d-major) -> [128, DTI, n] (partition, k-tile, col)."""
    return np.ascontiguousarray(
        m.reshape(DTI, P, -1).transpose(1, 0, 2))


def _hilo(m):
    hi = _q8(m)
    lo = _q8(np.asarray(m, np.float32) - hi.astype(np.float32))
    return hi, lo


def _prep_shared(inp):
    """Weight/vector arrays shared by all cores (host-side prep)."""
    f = lambda k: np.asarray(inp[k], np.float32)
    sh = {}
    for nm, key in [("wq1", "WQ1"), ("wk1", "WK1"), ("wv1", "WV1"),
                    ("wk2", "WK2"), ("wv2", "WV2")]:
        hi, lo = _hilo(WS * f(key))
        sh[nm] = _pack_dT(hi)
        if nm in ("wq1", "wk1", "wv1"):
            sh[nm + "lo"] = _pack_dT(lo)
    # wq2 with LN1 gamma folded; bias = be1 @ WQ2
    wq2p = WS * (f("g1")[:, None] * f("WQ2"))
    sh["wq2"] = _pack_dT(_q8(wq2p))
    sh["qb2"] = np.ascontiguousarray(
        (f("be1") @ f("WQ2")).reshape(DTI, P).T).astype(np.float32)
    # FFN weights: W1 with LN2 gamma folded, hi+lo interleaved; W2 hi+lo
    w1p = WS * (f("g2")[:, None] * f("W_ff1"))
    w1h, w1l = _hilo(w1p)
    w1h = w1h.reshape(DTI, P, FTI, P).transpose(2, 1, 0, 3)
    w1l = w1l.reshape(DTI, P, FTI, P).transpose(2, 1, 0, 3)
    sh["wf1"] = np.ascontiguousarray(
        np.stack([w1h, w1l], axis=2)).reshape(FTI * P, 2, DTI, P)
    w2h, w2l = _hilo(WS * f("W_ff2"))
    sh["wf2h"] = np.ascontiguousarray(
        w2h.reshape(FTI, P, D).transpose(1, 0, 2))
    sh["wf2l"] = np.ascontiguousarray(
        w2l.reshape(FTI, P, D).transpose(1, 0, 2))
    bh = f("be2") @ f("W_ff1") + f("b_ff1")
    sh["bf1"] = np.ascontiguousarray(bh.reshape(FTI, P).T).astype(np.float32)
    sh["g1"] = f("g1").astype(np.float16)
    sh["be1"] = f("be1").astype(np.float16)
    sh["g2"] = f("g2").astype(np.float16)
    sh["b2r"] = (f("be2") + f("b_ff2")).astype(np.float16)
    sh["g3"] = f("g3")
    sh["be3"] = f("be3")
    return sh


def _mask_blocks(h):
    """[128, 8, 128] fp8: boundary mask for self-attn score tile r=t-8c,
    applied to its q-block u4b=r//2 (the only block where the causal
    frontier can land).  r even: tri (h=0) / ones (h=1); r odd: zeros
    (h=0) / tri (h=1).  Hidden non-boundary blocks are never read."""
    tri = (np.arange(P)[:, None] <= np.arange(P)[None, :]).astype(np.float32)
    blocks = np.empty((DTI, P, P), np.float32)
    for r in range(DTI):
        cmp = 2 * (r // 2) + h - r
        blocks[r] = tri if cmp == 0 else (1.0 if cmp > 0 else 0.0)
    return np.ascontiguousarray(blocks.transpose(1, 0, 2)).astype(E4NP)


def _prep_core(c, y, Z, shared):
    b, h = c // 2, c % 2
    qi = _q_indices(h)
    yb16 = y[b].astype(np.float16)          # [S, D]
    yq16 = yb16[qi]                         # [NQ, D] own queries
    xkvT = yb16.T.astype(np.float32)        # [D, S]
    xqT = yq16.T.astype(np.float32)         # [D, NQ]
    zT = Z[b].astype(np.float16).T.astype(np.float32)
    m = {
        "xq8": _pack_dT(_q8(xqT)),
        "xq0lo": _pack_dT(_q8(xqT[:, 0:P] - _q8f(xqT[:, 0:P]))),
        "xkv8": _pack_dT(_q8(xkvT)),
        "xkvelo": _pack_dT(_q8(xkvT[:, 0:2 * P] - _q8f(xkvT[:, 0:2 * P]))),
        "z8": _pack_dT(_q8(zT)),
        "yres": np.ascontiguousarray(
            yq16.reshape(QTI, P, D).transpose(1, 0, 2)),
        "maskblk": _mask_blocks(h),
    }
    m.update(shared)
    return m


def kernel(**inputs):
    inp = {k: np.asarray(v) for k, v in inputs.items()}
    y = inp["y"].astype(np.float32)
    Z = inp["Z"].astype(np.float32)
    shared = _prep_shared(inp)
    in_maps = [_prep_core(c, y, Z, shared) for c in range(N_CORES)]
    res = run_bass_kernel_spmd(_get_nc(), in_maps, list(range(N_CORES)))
    out = np.zeros((4, 2048, 1024), np.float32)
    for c in range(N_CORES):
        b, h = c // 2, c % 2
        out[b, _q_indices(h)] = res.results[c]["out"]
    return out



# revision 39
# speedup vs baseline: 1.0848x; 1.0466x over previous
"""Trainium2 Bass kernel for a transformer decoder layer (self-attn +
cross-attn + FFN), fp8-e4m3 DoubleRow edition.

Sharding: 8 cores = 4 batches x 2 halves, no collectives. Core h of a batch
owns the interleaved query tiles {h, h+2, ..., h+14} (causal load balance) and
computes the FULL K/V projections for its batch locally (cheaper than the
pair-exchange collective at fp8 speeds).

Numerics: all matmuls run in fp8-e4m3 with DoubleRow perf mode (2 contraction
rows per partition).  Weights are pre-scaled x32 host-side so they sit in
e4m3's normal range; every PSUM drain folds the 1/32 back in.  Three
refinements keep absmax rel err ~3e-3 (gate is 2e-2):
  - FFN: both matmuls use hi+lo fp8 splits of activations AND weights
    (3 DoubleRow matmuls per logical matmul = fp16-level accuracy at 2x
    fp16 speed).
  - Early causal tokens (global positions 0..255, each core's local q-tile
    u=0) see few keys, so fp8 noise doesn't average out: their Q/K/V/E values
    are computed via the same hi+lo corrected path.
  - LayerNorm gammas/betas are folded into the next matmul's weights where
    possible (WQ2, W_ff1) and the residual carriers keep f16 precision.

Causal masking is via per-core precomputed [128 x 512] mask rows (tri/ones/
zeros blocks depending on core half), applied to each self-attn score tile,
so the SPMD program is uniform across cores.

Stage order: A1 (self K/V/Q + early) -> B (self-attn, prefetching A2 inputs)
-> A2 (cross K/V) -> T1 (n1 transpose) -> C0 (Q2) -> C (cross-attn) ->
T2 (n2 transpose hi/lo) -> D (FFN, token-chunked, streamed W_ff1).
"""

from contextlib import ExitStack

import ml_dtypes
import numpy as np

import concourse.bass as bass
import concourse.mybir as mybir
import concourse.tile as tile
from concourse import bacc
from concourse.bass_utils import run_bass_kernel_spmd
from concourse.masks import make_identity

f32 = mybir.dt.float32
f16 = mybir.dt.float16
f8 = mybir.dt.float8e4

P = 128
D = 1024          # d_model
S = 2048          # kv sequence length
NQ = 1024         # query tokens per core
DFF = 4096
DTI = D // P      # 8 d-model partition tiles
KTI = S // P      # 16 kv token tiles
QTI = NQ // P     # 8 query tiles
FTI = DFF // P    # 32 d_ff tiles
ACT = mybir.ActivationFunctionType
ALU = mybir.AluOpType
DR = mybir.MatmulPerfMode.DoubleRow
N_CORES = 8
WS = 32.0         # host-side weight pre-scale
IWS = 1.0 / WS
SCALE = 1.0 / 32.0  # 1/sqrt(D) softmax scale
E4NP = ml_dtypes.float8_e4m3


def build_nc():
    nc = bacc.Bacc("TRN2", target_bir_lowering=False, debug=False,
                   num_devices=N_CORES)

    def dp(name, shape, dt, out=False):
        return nc.declare_dram_parameter(name, shape, dt, isOutput=out)

    xq8_d = dp("xq8", [P, DTI, NQ], f8)
    xq0lo_d = dp("xq0lo", [P, DTI, P], f8)
    xkv8_d = dp("xkv8", [P, DTI, S], f8)
    xkvelo_d = dp("xkvelo", [P, DTI, 2 * P], f8)
    z8_d = dp("z8", [P, DTI, S], f8)
    yres_d = dp("yres", [P, QTI, D], f16)
    w_d = {n: dp(n, [P, DTI, D], f8)
           for n in ["wq1", "wk1", "wv1", "wq2", "wk2", "wv2",
                     "wq1lo", "wk1lo", "wv1lo"]}
    wf1_d = dp("wf1", [FTI * P, 2, DTI, P], f8)   # hi/lo interleaved
    wf2h_d = dp("wf2h", [P, FTI, D], f8)
    wf2l_d = dp("wf2l", [P, FTI, D], f8)
    bf1_d = dp("bf1", [P, FTI], f32)
    qb2_d = dp("qb2", [P, DTI], f32)
    mask_d = dp("maskblk", [P, DTI, P], f8)
    v16_d = {n: dp(n, [D], f16) for n in ["g1", "be1", "g2", "b2r"]}
    v32_d = {n: dp(n, [D], f32) for n in ["g3", "be3"]}
    out_d = dp("out", [NQ, D], f32, out=True)

    def bc(ap):  # broadcast a [n] dram vector across 128 partitions
        return bass.AP(tensor=ap.tensor, offset=ap.offset,
                       ap=[[0, P]] + [list(x) for x in ap.ap])

    with tile.TileContext(nc) as tc, ExitStack() as top:
        const = top.enter_context(tc.tile_pool(name="const", bufs=1))
        # one explicit act-table load (natural_log_exp_and_others: exp, ln,
        # copy, identity, relu, square) so every activation in the kernel is
        # servable without another table swap, regardless of how the
        # scheduler interleaves exp/rstd chains
        nc.scalar.add_instruction(mybir.InstLoadActFuncSet(
            name=f"I-{nc.next_id()}", act_func_set_id=6))
        ident = const.tile([P, P], f16, name="ident", tag="ident")
        make_identity(nc, ident)
        masks = const.tile([P, DTI, P], f8, name="masks", tag="masks")
        ones8t = const.tile([P, 2, 16], f8, name="ones8", tag="ones8")
        nc.vector.memset(ones8t, 1.0)
        ones8 = ones8t[:, :, 0:1]  # outer step 16B: dual-fp8 ldweights rule
        eps = const.tile([P, 1], f32, name="eps", tag="eps")
        nc.vector.memset(eps, 1e-5)
        bf1sb = const.tile([P, FTI], f32, name="bf1sb", tag="bf1sb")
        qb2sb = const.tile([P, DTI], f32, name="qb2sb", tag="qb2sb")

        def vload(name, dt, dram):
            return const.tile([P, D], dt, name=f"{name}b", tag=f"{name}b")

        g1b = vload("g1", f16, v16_d)
        b1b = vload("be1", f16, v16_d)
        g2b = vload("g2", f16, v16_d)
        b2rb = vload("b2r", f16, v16_d)

        def load_consts():
            # deferred off-critical-path constant loads
            nc.sync.dma_start(out=masks, in_=mask_d.ap())
            nc.sync.dma_start(out=bf1sb, in_=bf1_d.ap())
            nc.sync.dma_start(out=qb2sb, in_=qb2_d.ap())
            for t, nm, dd in [(g1b, "g1", v16_d), (b1b, "be1", v16_d),
                              (g2b, "g2", v16_d), (b2rb, "b2r", v16_d)]:
                nc.sync.dma_start(out=t, in_=bc(dd[nm].ap()))

        # ---- persistent pools; LIFO per side ----
        # left: kv2p (bottom; dies after cross), y1, n1, [zpB/wpB], [n1T],
        #       [n2T]
        # right: yres, kvp, earlyp | y2r, n2, qT2p
        kv2p = tc.alloc_tile_pool(name="kv2p", bufs=1)
        kT2 = kv2p.tile([P, DTI, S], f8, name="kT2", tag="kT2")
        v2 = kv2p.tile([P, KTI, D], f8, name="v2", tag="v2")
        y1p = tc.alloc_tile_pool(name="y1p", bufs=1)
        y1 = y1p.tile([P, QTI, D], f16, name="y1", tag="y1")
        n1p = tc.alloc_tile_pool(name="n1p", bufs=1)
        n1 = n1p.tile([P, QTI, D], f16, name="n1", tag="n1")

        yresp = tc.alloc_tile_pool(name="yresp", bufs=1, side="right")
        yres = yresp.tile([P, QTI, D], f16, name="yres", tag="yres")
        kvp = tc.alloc_tile_pool(name="kvp", bufs=1, side="right")
        kT = kvp.tile([P, DTI, S], f8, name="kT", tag="kT")
        v = kvp.tile([P, KTI, D], f8, name="v", tag="v")
        qT = kvp.tile([P, DTI, NQ], f8, name="qT", tag="qT")
        earlyp = tc.alloc_tile_pool(name="earlyp", bufs=1, side="right")
        keT = [earlyp.tile([P, DTI, 2 * P], f8, name=f"keT{x}", tag=f"keT{x}")
               for x in range(2)]  # hi, lo
        qeT = [earlyp.tile([P, DTI, P], f8, name=f"qeT{x}", tag=f"qeT{x}")
               for x in range(2)]
        ve = [earlyp.tile([P, 2, D], f8, name=f"ve{x}", tag=f"ve{x}")
              for x in range(2)]

        def dr_acc(ps, terms, rhs_sl, lhs_sl):
            """Accumulate sum of DoubleRow products into psum region ps.
            terms: list of (lhsT_tile, rhs_tile); contraction over DTI//2
            k-tile pairs per term. rhs_sl/lhs_sl: fn(tile, g) -> AP."""
            n = len(terms) * (DTI // 2)
            i = 0
            for lt, rt in terms:
                for g in range(DTI // 2):
                    nc.tensor.matmul(ps, lhsT=lhs_sl(lt, g),
                                     rhs=rhs_sl(rt, g), perf_mode=DR,
                                     start=(i == 0), stop=(i == n - 1))
                    i += 1

        def split3(pool, ps, scale, bias, func, hi_out, lo_out, eng, n):
            """3-op hi/lo drain: t16 = func(scale*ps + bias); hi = q8(t16);
            lo = q8(t16 - hi)."""
            t16 = pool.tile([P, n], f16, name="t16", tag="t16", bufs=3)
            nc.scalar.activation(out=t16, in_=ps, func=func, bias=bias,
                                 scale=scale)
            if eng == 0:
                nc.vector.tensor_copy(out=hi_out, in_=t16)
                nc.gpsimd.tensor_sub(lo_out, t16, hi_out)
            else:
                nc.gpsimd.tensor_copy(out=hi_out, in_=t16)
                nc.vector.tensor_sub(lo_out, t16, hi_out)

        # ==================== stage A1: self-attn projections =============
        with ExitStack() as stA:
            wpA = stA.enter_context(tc.tile_pool(name="wpA", bufs=1))
            xpA = stA.enter_context(tc.tile_pool(name="xpA", bufs=1))
            psA = stA.enter_context(tc.tile_pool(name="psA", bufs=3,
                                                 space="PSUM"))
            psE = stA.enter_context(tc.tile_pool(name="psE", bufs=2,
                                                 space="PSUM"))
            drp = stA.enter_context(tc.tile_pool(name="drpA", bufs=1))

            def wload(tag, name):
                t = wpA.tile([P, DTI, D], f8, name=name, tag=tag)
                nc.sync.dma_start(out=t, in_=w_d[name].ap())
                return t

            # first loads chunked so K1 (j=0, th=0) can start after ~1/4 of
            # the wk1+xkv8 bytes have landed
            wk1 = wpA.tile([P, DTI, D], f8, name="wk1", tag="wA0")
            xkv8 = xpA.tile([P, DTI, S], f8, name="xkv8", tag="xkv8")
            nc.sync.dma_start(out=wk1[:, :, 0:512],
                              in_=w_d["wk1"].ap()[:, :, 0:512])
            nc.sync.dma_start(out=xkv8[:, :, 0:1024],
                              in_=xkv8_d.ap()[:, :, 0:1024])
            nc.sync.dma_start(out=wk1[:, :, 512:1024],
                              in_=w_d["wk1"].ap()[:, :, 512:1024])
            nc.sync.dma_start(out=xkv8[:, :, 1024:2048],
                              in_=xkv8_d.ap()[:, :, 1024:2048])
            wv1 = wload("wA1", "wv1")
            wk1lo = wload("wA2", "wk1lo")
            wv1lo = wload("wA3", "wv1lo")
            xkvelo = xpA.tile([P, DTI, 2 * P], f8, name="xkvelo",
                              tag="xkvelo")
            nc.sync.dma_start(out=xkvelo, in_=xkvelo_d.ap())
            xq8 = xpA.tile([P, DTI, NQ], f8, name="xq8", tag="xq8")
            nc.sync.dma_start(out=xq8, in_=xq8_d.ap())
            xq0lo = xpA.tile([P, DTI, P], f8, name="xq0lo", tag="xq0lo")
            nc.sync.dma_start(out=xq0lo, in_=xq0lo_d.ap())
            nc.sync.dma_start(out=yres, in_=yres_d.ap())
            load_consts()

            # K1: kT[:, j, :] = (wk1.T @ xkv)/32, d_out on partitions
            for j in range(DTI):
                for th in range(2):
                    ps = psA.tile([P, 1024], f32, name="psp", tag="psp")
                    for sub in range(2):
                        tsl = slice(th * 1024 + sub * 512,
                                    th * 1024 + sub * 512 + 512)
                        dr_acc(ps[:, sub * 512:sub * 512 + 512],
                               [(wk1, xkv8)],
                               lambda t, g, tsl=tsl: t[:, 2 * g:2 * g + 2, tsl],
                               lambda t, g, j=j: t[:, 2 * g:2 * g + 2,
                                                   j * P:(j + 1) * P])
                    osl = kT[:, j, th * 1024:(th + 1) * 1024]
                    if (j + th) % 2 == 0:
                        nc.scalar.activation(out=osl, in_=ps, func=ACT.Copy,
                                             scale=IWS)
                    else:
                        nc.vector.tensor_scalar_mul(osl, ps, IWS)

            # early K (tokens 0:256), hi+lo corrected
            for j in range(DTI):
                ps = psE.tile([P, 512], f32, name="pse", tag="pse")
                dr_acc(ps[:, 0:256],
                       [(wk1, xkv8), (wk1, xkvelo), (wk1lo, xkv8)],
                       lambda t, g: (t[:, 2 * g:2 * g + 2, 0:256]
                                     if t is xkv8 else
                                     t[:, 2 * g:2 * g + 2, :]),
                       lambda t, g, j=j: t[:, 2 * g:2 * g + 2,
                                           j * P:(j + 1) * P])
                split3(drp, ps[:, 0:256], IWS, 0.0, ACT.Copy,
                       keT[0][:, j, :], keT[1][:, j, :], j % 2, 256)

            # V1: v[:, t, :] = (xkv.T @ wv1)/32, tokens on partitions
            for t in range(KTI):
                ps = psA.tile([P, 1024], f32, name="psp", tag="psp")
                for half in range(2):
                    dr_acc(ps[:, half * 512:half * 512 + 512],
                           [(xkv8, wv1)],
                           lambda tt, g, half=half: tt[:, 2 * g:2 * g + 2,
                                                       half * 512:half * 512 + 512],
                           lambda tt, g, t=t: tt[:, 2 * g:2 * g + 2,
                                                 t * P:(t + 1) * P])
                osl = v[:, t, :]
                if t % 2 == 0:
                    nc.scalar.activation(out=osl, in_=ps, func=ACT.Copy,
                                         scale=IWS)
                else:
                    nc.vector.tensor_scalar_mul(osl, ps, IWS)

            # early V (k-tiles 0..1), hi+lo corrected
            for t in range(2):
                for half in range(2):
                    ps = psE.tile([P, 512], f32, name="pse", tag="pse")
                    hsl = slice(half * 512, half * 512 + 512)
                    dr_acc(ps,
                           [(xkv8, wv1), (xkvelo, wv1), (xkv8, wv1lo)],
                           lambda tt, g, hsl=hsl: tt[:, 2 * g:2 * g + 2, hsl],
                           lambda tt, g, t=t: tt[:, 2 * g:2 * g + 2,
                                                 t * P:(t + 1) * P])
                    split3(drp, ps, IWS, 0.0, ACT.Copy,
                           ve[0][:, t, hsl], ve[1][:, t, hsl],
                           (t + half) % 2, 512)

            # Q1 (weights reuse the K1 buffers)
            wq1 = wload("wA0", "wq1")
            wq1lo = wload("wA2", "wq1lo")
            for j in range(DTI):
                ps = psA.tile([P, 1024], f32, name="psp", tag="psp")
                for sub in range(2):
                    dr_acc(ps[:, sub * 512:sub * 512 + 512],
                           [(wq1, xq8)],
                           lambda t, g, sub=sub: t[:, 2 * g:2 * g + 2,
                                                   sub * 512:sub * 512 + 512],
                           lambda t, g, j=j: t[:, 2 * g:2 * g + 2,
                                               j * P:(j + 1) * P])
                osl = qT[:, j, :]
                if j % 2 == 0:
                    nc.scalar.activation(out=osl, in_=ps, func=ACT.Copy,
                                         scale=IWS)
                else:
                    nc.vector.tensor_scalar_mul(osl, ps, IWS)
            # early Q (own u=0 tile)
            for j in range(DTI):
                ps = psE.tile([P, 512], f32, name="pse", tag="pse")
                dr_acc(ps[:, 0:P],
                       [(wq1, xq8), (wq1, xq0lo), (wq1lo, xq8)],
                       lambda t, g: (t[:, 2 * g:2 * g + 2, 0:P]
                                     if t is xq8 else
                                     t[:, 2 * g:2 * g + 2, :]),
                       lambda t, g, j=j: t[:, 2 * g:2 * g + 2,
                                           j * P:(j + 1) * P])
                split3(drp, ps[:, 0:P], IWS, 0.0, ACT.Copy,
                       qeT[0][:, j, :], qeT[1][:, j, :], j % 2, P)

        # ==================== attention helper ============================
        def pump(gen, n=1):
            """Advance a filler emission generator n steps (no-op if None)."""
            for _ in range(n):
                if gen is None or next(gen, "END") == "END":
                    return

        def att_pools(stk, tagp, score_bufs, out_bufs=2):
            pss = stk.enter_context(tc.tile_pool(name=f"{tagp}pss",
                                                 bufs=score_bufs,
                                                 space="PSUM"))
            pso = stk.enter_context(tc.tile_pool(name=f"{tagp}pso",
                                                 bufs=out_bufs,
                                                 space="PSUM"))
            ep = stk.enter_context(tc.tile_pool(name=f"{tagp}ep", bufs=1))
            lnp = stk.enter_context(tc.tile_pool(name=f"{tagp}lnp", bufs=4))
            return tagp, pss, pso, ep, lnp

        def attention_half(ap_, c, qTt, kTt, vt, resid_sl, gb, bb, yout,
                           nout, masked, filler=None):
            tagp, pss, pso, ep, lnp = ap_
            nvis = 8 * (c + 1) if masked else KTI
            if True:
                e = ep.tile([P, nvis, 512], f8, name=f"e{c}", tag=f"e{c}")
                ee = None
                if masked and c == 0:
                    # early corrected scores/E for q-tile u=0, k-tiles 0..1
                    # (emitted first so its long drain chain overlaps the
                    # main score tiles)
                    ee = lnp.tile([P, 2, 2, P], f8, name="ee", tag="ee",
                                  bufs=1)
                    for t in range(2):
                        ps = pss.tile([P, 1024], f32, name="ps_s", tag="ps_s")
                        dr_acc(ps[:, 0:P],
                               [(keT[0], qeT[0]), (keT[0], qeT[1]),
                                (keT[1], qeT[0])],
                               lambda tt, g: tt[:, 2 * g:2 * g + 2, :],
                               lambda tt, g, t=t: tt[:, 2 * g:2 * g + 2,
                                                     t * P:(t + 1) * P])
                        tm = lnp.tile([P, P], f16, name="etm", tag="etm",
                                      bufs=2)
                        nc.scalar.activation(out=tm, in_=ps[:, 0:P],
                                             func=ACT.Exp, scale=SCALE)
                        nc.vector.tensor_mul(tm, tm, masks[:, t, :])
                        nc.vector.tensor_copy(out=ee[:, 0, t, :], in_=tm)
                        nc.gpsimd.tensor_sub(ee[:, 1, t, :], tm,
                                             ee[:, 0, t, :])
                # scores + exp, two k-tiles per psum tile / exp instruction;
                # causal mask only touches the boundary q-block of each tile
                # (hidden non-boundary blocks are never read downstream)
                for dual in range(nvis // 2):
                    ps = pss.tile([P, 1024], f32, name="ps_s", tag="ps_s")
                    for k in range(2):
                        t = 2 * dual + k
                        dr_acc(ps[:, k * 512:k * 512 + 512], [(kTt, qTt)],
                               lambda tt, g, c=c: tt[:, 2 * g:2 * g + 2,
                                                     c * 512:c * 512 + 512],
                               lambda tt, g, t=t: tt[:, 2 * g:2 * g + 2,
                                                     t * P:(t + 1) * P])
                    nc.scalar.activation(out=e[:, 2 * dual:2 * dual + 2, :],
                                         in_=ps, func=ACT.Exp, scale=SCALE)
                    if masked:
                        for k in range(2):
                            t = 2 * dual + k
                            if t < 8 * c:
                                continue
                            r = t - 8 * c
                            u4b = r // 2
                            esl = e[:, t, u4b * P:(u4b + 1) * P]
                            if r % 2 == 0:
                                nc.vector.tensor_mul(esl, esl, masks[:, r, :])
                            else:
                                nc.gpsimd.tensor_mul(esl, esl, masks[:, r, :])
                    pump(filler)
                # denominators: E^T @ ones -> [128 q, 1] per u4 column of a
                # psum tile (q on partitions; no DRAM transpose round-trip)
                pd = pso.tile([P, 1024], f32, name="pd", tag="po")
                for u4 in range(4):
                    if ee is not None and u4 == 0:
                        for hl in range(2):
                            nc.tensor.matmul(pd[:, 0:1],
                                             lhsT=ee[:, hl, :, :],
                                             rhs=ones8, perf_mode=DR,
                                             start=(hl == 0), stop=(hl == 1))
                        continue
                    np_ = (4 * c + u4 + 1) if masked else 8
                    for i in range(np_):
                        nc.tensor.matmul(
                            pd[:, u4:u4 + 1],
                            lhsT=e[:, 2 * i:2 * i + 2, u4 * P:(u4 + 1) * P],
                            rhs=ones8, perf_mode=DR,
                            start=(i == 0), stop=(i == np_ - 1))
                recT = lnp.tile([P, 4], f32, name="recT", tag="recT")
                nc.vector.reciprocal(recT, pd[:, 0:4])
                pump(filler)
                u4order = [1, 2, 3, 0] if ee is not None else range(4)
                for u4 in u4order:
                    u = c * 4 + u4
                    po = pso.tile([P, 1024], f32, name="po", tag="po")
                    if ee is not None and u == 0:
                        for half in range(2):
                            hsl = slice(half * 512, half * 512 + 512)
                            for ti, (el, vl) in enumerate(
                                    [(0, 0), (1, 0), (0, 1)]):
                                nc.tensor.matmul(
                                    po[:, hsl], lhsT=ee[:, el, :, :],
                                    rhs=ve[vl][:, :, hsl], perf_mode=DR,
                                    start=(ti == 0), stop=(ti == 2))
                    else:
                        np_ = (u + 1) if masked else 8
                        for half in range(2):
                            hsl = slice(half * 512, half * 512 + 512)
                            for i in range(np_):
                                nc.tensor.matmul(
                                    po[:, hsl],
                                    lhsT=e[:, 2 * i:2 * i + 2,
                                           u4 * P:(u4 + 1) * P],
                                    rhs=vt[:, 2 * i:2 * i + 2, hsl],
                                    perf_mode=DR, start=(i == 0),
                                    stop=(i == np_ - 1))
                    xr = lnp.tile([P, D], f16, name="xr", tag="xr", bufs=2)
                    nc.scalar.activation(out=xr, in_=po, func=ACT.Copy,
                                         scale=recT[:, u4:u4 + 1])
                    nc.vector.tensor_add(xr, xr, resid_sl(u))
                    pump(filler)
                    # LN core + affine
                    stats = lnp.tile([P, 2, 6], f32, name="stats",
                                     tag="stats")
                    nc.vector.bn_stats(out=stats[:, 0, :], in_=xr[:, 0:512])
                    nc.vector.bn_stats(out=stats[:, 1, :], in_=xr[:, 512:])
                    mv = lnp.tile([P, 2], f32, name="mv", tag="mv")
                    nc.vector.bn_aggr(out=mv, in_=stats)
                    # rstd = exp(-0.5*ln(var+eps)) -- stays in act table 6
                    lnv = lnp.tile([P, 1], f32, name="lnv", tag="lnv")
                    nc.scalar.activation(out=lnv, in_=mv[:, 1:2],
                                         func=ACT.Ln, bias=eps)
                    rstd = lnp.tile([P, 1], f32, name="rstd", tag="rstd")
                    nc.scalar.activation(out=rstd, in_=lnv, func=ACT.Exp,
                                         scale=-0.5)
                    nsl = nout[:, u, :]
                    nc.vector.tensor_scalar(out=nsl, in0=xr,
                                            scalar1=mv[:, 0:1], scalar2=rstd,
                                            op0=ALU.subtract, op1=ALU.mult)
                    t1 = lnp.tile([P, D], f16, name="lt0", tag="lt0", bufs=2)
                    nc.vector.tensor_mul(t1, nsl, gb)
                    nc.gpsimd.tensor_add(yout[:, u, :], t1, bb)
                    pump(filler, 2)

        # ==================== stage B: self-attention + LN1 ===============
        # Cross-attn K2/V2 projections are emitted as FILLER inside the
        # self-attention instruction stream: the PE chews them while the
        # Act/DVE engines work through exp + LayerNorm chains.
        with ExitStack() as stB:
            # prefetch stage-A2 inputs while attention runs
            zpB = stB.enter_context(tc.tile_pool(name="zpB", bufs=1))
            z8 = zpB.tile([P, DTI, S], f8, name="z8", tag="z8")
            nc.sync.dma_start(out=z8, in_=z8_d.ap())
            wk2 = zpB.tile([P, DTI, D], f8, name="wk2", tag="wk2")
            nc.sync.dma_start(out=wk2, in_=w_d["wk2"].ap())
            wv2 = zpB.tile([P, DTI, D], f8, name="wv2", tag="wv2")
            nc.sync.dma_start(out=wv2, in_=w_d["wv2"].ap())

            def emit_k2(pool, j, th, eng):
                ps = pool.tile([P, 1024], f32, name="psp2", tag="psp2")
                for sub in range(2):
                    tsl = slice(th * 1024 + sub * 512,
                                th * 1024 + sub * 512 + 512)
                    dr_acc(ps[:, sub * 512:sub * 512 + 512], [(wk2, z8)],
                           lambda t, g, tsl=tsl: t[:, 2 * g:2 * g + 2, tsl],
                           lambda t, g, j=j: t[:, 2 * g:2 * g + 2,
                                               j * P:(j + 1) * P])
                osl = kT2[:, j, th * 1024:(th + 1) * 1024]
                if eng == 0:
                    nc.vector.tensor_scalar_mul(osl, ps, IWS)
                else:
                    nc.scalar.activation(out=osl, in_=ps, func=ACT.Copy,
                                         scale=IWS)

            def emit_v2(pool, t, eng):
                ps = pool.tile([P, 1024], f32, name="psp2", tag="psp2")
                for half in range(2):
                    dr_acc(ps[:, half * 512:half * 512 + 512], [(z8, wv2)],
                           lambda tt, g, half=half: tt[:, 2 * g:2 * g + 2,
                                                       half * 512:half * 512 + 512],
                           lambda tt, g, t=t: tt[:, 2 * g:2 * g + 2,
                                                 t * P:(t + 1) * P])
                osl = v2[:, t, :]
                if eng == 0:
                    nc.vector.tensor_scalar_mul(osl, ps, IWS)
                else:
                    nc.scalar.activation(out=osl, in_=ps, func=ACT.Copy,
                                         scale=IWS)

            A2G = ([("k", j, th) for j in range(DTI) for th in range(2)]
                   + [("v", t, 0) for t in range(KTI)])

            def gen_a2(pool, groups):
                for gi, g in enumerate(groups):
                    eng = 0 if gi % 3 < 2 else 1  # drains mostly DVE
                    if g[0] == "k":
                        emit_k2(pool, g[1], g[2], eng)
                    else:
                        emit_v2(pool, g[1], eng)
                    yield

            NFILL = 20
            with ExitStack() as stB2:
                ap_ = att_pools(stB2, "sa_", score_bufs=1)
                psA2 = stB2.enter_context(tc.tile_pool(name="psA2", bufs=1,
                                                       space="PSUM"))
                attention_half(ap_, 0, qT, kT, v, lambda u: yres[:, u, :],
                               g1b, b1b, y1, n1, masked=True)
                earlyp.release()
                a2 = gen_a2(psA2, A2G[:NFILL])
                attention_half(ap_, 1, qT, kT, v, lambda u: yres[:, u, :],
                               g1b, b1b, y1, n1, masked=True, filler=a2)
                pump(a2, 99)
            # leftover A2 groups run dense with triple-buffered psum
            with ExitStack() as stA2t:
                psA2t = stA2t.enter_context(tc.tile_pool(name="psA2t",
                                                         bufs=3,
                                                         space="PSUM"))
                pump(gen_a2(psA2t, A2G[NFILL:]), 99)
            kvp.release()
            yresp.release()

            y2rp = tc.alloc_tile_pool(name="y2rp", bufs=1, side="right")
            y2r = y2rp.tile([P, QTI, D], f16, name="y2r", tag="y2r")
            n2p = tc.alloc_tile_pool(name="n2p", bufs=1, side="right")
            n2 = n2p.tile([P, QTI, D], f16, name="n2", tag="n2")

        # ======= stages T1/C0/cross/T2: pipelined with cross-attention ====
        # T1(c=0)+Q2(sub 0) run dense before cross; T1(c=1)+Q2(sub 1) fill
        # cross c0's exp/LN bubbles; T2(c=0) fills cross c1's; T2(c=1) is
        # pumped inside the FFN mm1 loop.
        n1Tp = tc.alloc_tile_pool(name="n1Tp", bufs=1)
        n1T = n1Tp.tile([P, DTI, NQ], f8, name="n1T", tag="n1T")
        wpC = tc.alloc_tile_pool(name="wpC", bufs=1)
        wq2 = wpC.tile([P, DTI, D], f8, name="wq2", tag="wq2")
        nc.sync.dma_start(out=wq2, in_=w_d["wq2"].ap())
        qT2p = tc.alloc_tile_pool(name="qT2p", bufs=1, side="right")
        qT2 = qT2p.tile([P, DTI, NQ], f8, name="qT2", tag="qT2")
        n2Tp = tc.alloc_tile_pool(name="n2Tp", bufs=1, side="right")
        n2T = [n2Tp.tile([P, DTI, NQ], f8, name=f"n2T{x}", tag=f"n2T{x}")
               for x in range(2)]

        def gen_t1(pst, c2, engs):
            for i in range(DTI):
                pt = pst.tile([P, 512], f16, name="pt", tag="pt")
                for u4 in range(4):
                    nc.tensor.transpose(
                        pt[:, u4 * P:(u4 + 1) * P],
                        in_=n1[:, c2 * 4 + u4, i * P:(i + 1) * P],
                        identity=ident)
                osl = n1T[:, i, c2 * 512:c2 * 512 + 512]
                if engs[i % len(engs)] == "a":
                    nc.scalar.activation(out=osl, in_=pt, func=ACT.Copy,
                                         scale=1.0)
                else:
                    nc.vector.tensor_copy(out=osl, in_=pt)
                yield

        def gen_q2(psC, sub, engs):
            ssl = slice(sub * 512, sub * 512 + 512)
            for j in range(DTI):
                ps = psC.tile([P, 512], f32, name="psq2", tag="psq2")
                dr_acc(ps, [(wq2, n1T)],
                       lambda t, g, ssl=ssl: t[:, 2 * g:2 * g + 2, ssl],
                       lambda t, g, j=j: t[:, 2 * g:2 * g + 2,
                                           j * P:(j + 1) * P])
                osl = qT2[:, j, ssl]
                if engs[j % len(engs)] == "a":
                    nc.scalar.activation(out=osl, in_=ps,
                                         func=ACT.Identity,
                                         bias=qb2sb[:, j:j + 1], scale=IWS)
                else:
                    nc.vector.tensor_scalar(out=osl, in0=ps, scalar1=IWS,
                                            scalar2=qb2sb[:, j:j + 1],
                                            op0=ALU.mult, op1=ALU.add)
                yield

        def gen_t2(pst, c2, engs):
            for i in range(DTI):
                pt = pst.tile([P, 512], f16, name="pt", tag="pt")
                for u4 in range(4):
                    nc.tensor.transpose(
                        pt[:, u4 * P:(u4 + 1) * P],
                        in_=n2[:, c2 * 4 + u4, i * P:(i + 1) * P],
                        identity=ident)
                csl = slice(c2 * 512, c2 * 512 + 512)
                if engs[i % len(engs)] == "a":
                    nc.scalar.activation(out=n2T[0][:, i, csl], in_=pt,
                                         func=ACT.Copy, scale=1.0)
                else:
                    nc.vector.tensor_copy(out=n2T[0][:, i, csl], in_=pt)
                nc.vector.tensor_sub(n2T[1][:, i, csl], pt,
                                     n2T[0][:, i, csl])
                yield

        def chain(*gens):
            for g in gens:
                yield from g

        with ExitStack() as stCC:
            pst = stCC.enter_context(tc.tile_pool(name="pstC", bufs=2,
                                                  space="PSUM"))
            psC = stCC.enter_context(tc.tile_pool(name="psC", bufs=2,
                                                  space="PSUM"))
            pump(gen_t1(pst, 0, "av"), 99)
            pump(gen_q2(psC, 0, "av"), 99)
            ap_ = att_pools(stCC, "ca_", score_bufs=1, out_bufs=1)
            fill0 = chain(gen_t1(pst, 1, "vg"), gen_q2(psC, 1, "v"))
            attention_half(ap_, 0, qT2, kT2, v2, lambda u: y1[:, u, :],
                           g2b, b2rb, y2r, n2, masked=False, filler=fill0)
            pump(fill0, 99)
            fill1 = gen_t2(pst, 0, "vg")
            attention_half(ap_, 1, qT2, kT2, v2, lambda u: y1[:, u, :],
                           g2b, b2rb, y2r, n2, masked=False, filler=fill1)
            pump(fill1, 99)
        wpC.release()
        n1Tp.release()
        n1p.release()
        y1p.release()
        kv2p.release()

        # ==================== stage D: FFN + LN3 + output =================
        with ExitStack() as stD:
            wf2p = stD.enter_context(tc.tile_pool(name="wf2p", bufs=1))
            wf1p = stD.enter_context(tc.tile_pool(name="wf1p", bufs=3))
            hp = stD.enter_context(tc.tile_pool(name="hp", bufs=1))
            psH = stD.enter_context(tc.tile_pool(name="psH", bufs=2,
                                                 space="PSUM"))
            psF = stD.enter_context(tc.tile_pool(name="psF", bufs=2,
                                                 space="PSUM"))
            pstD = stD.enter_context(tc.tile_pool(name="pstD", bufs=2,
                                                  space="PSUM"))
            drp = stD.enter_context(tc.tile_pool(name="drpD", bufs=1))
            lnp = stD.enter_context(tc.tile_pool(name="lnpD", bufs=4))
            outp = stD.enter_context(tc.tile_pool(name="outp", bufs=2))
            t2g1 = gen_t2(pstD, 1, "avv")
            # wf2/g3/b3 loads are chunked and interleaved between the
            # streamed w1t loads so they don't head-block the first FFN
            # matmuls on the DMA queue
            wf2h = wf2p.tile([P, FTI, D], f8, name="wf2h", tag="wf2h")
            wf2l = wf2p.tile([P, FTI, D], f8, name="wf2l", tag="wf2l")
            g3b = wf2p.tile([P, D], f32, name="g3b", tag="g3b")
            b3b = wf2p.tile([P, D], f32, name="b3b", tag="b3b")
            for c in range(2):
                csl = slice(c * 512, c * 512 + 512)
                hh = hp.tile([P, FTI, 512], f8, name="hh", tag="hh")
                hl = hp.tile([P, FTI, 512], f8, name="hl", tag="hl")
                for s in range(FTI):
                    w1t = wf1p.tile([P, 2, DTI, P], f8, name="w1t",
                                    tag="w1t")
                    nc.sync.dma_start(out=w1t,
                                      in_=wf1_d.ap()[s * P:(s + 1) * P])
                    if c == 0:
                        if s % 2 == 0 and s // 2 < 8:
                            ch = s // 2
                            nc.sync.dma_start(
                                out=wf2h[:, ch * 4:(ch + 1) * 4, :],
                                in_=wf2h_d.ap()[:, ch * 4:(ch + 1) * 4, :])
                        elif s == 1:
                            nc.sync.dma_start(out=g3b,
                                              in_=bc(v32_d["g3"].ap()))
                        elif s == 3:
                            nc.sync.dma_start(out=b3b,
                                              in_=bc(v32_d["be3"].ap()))
                        elif s % 2 == 1 and 5 <= s <= 19:
                            ch = (s - 5) // 2
                            nc.sync.dma_start(
                                out=wf2l[:, ch * 4:(ch + 1) * 4, :],
                                in_=wf2l_d.ap()[:, ch * 4:(ch + 1) * 4, :])
                    ps = psH.tile([P, 512], f32, name="ph", tag="ph")
                    i = 0
                    for wi, xi in [(0, 0), (0, 1), (1, 0)]:
                        for g in range(DTI // 2):
                            nc.tensor.matmul(
                                ps,
                                lhsT=w1t[:, wi, 2 * g:2 * g + 2, :],
                                rhs=n2T[xi][:, 2 * g:2 * g + 2, csl],
                                perf_mode=DR, start=(i == 0),
                                stop=(i == 3 * DTI // 2 - 1))
                            i += 1
                    split3(drp, ps, IWS, bf1sb[:, s:s + 1], ACT.Relu,
                           hh[:, s, :], hl[:, s, :], s % 2, 512)
                    if c == 0 and s % 2 == 1:
                        pump(t2g1)
                if c == 0:
                    pump(t2g1, 99)
                for u4 in range(4):
                    u = c * 4 + u4
                    pf = psF.tile([P, 1024], f32, name="pf", tag="pf")
                    usl = slice(u4 * P, (u4 + 1) * P)
                    for half in range(2):
                        hsl = slice(half * 512, half * 512 + 512)
                        i = 0
                        for ha, wb in [(hh, wf2h), (hl, wf2h), (hh, wf2l)]:
                            for sp in range(FTI // 2):
                                nc.tensor.matmul(
                                    pf[:, hsl],
                                    lhsT=ha[:, 2 * sp:2 * sp + 2, usl],
                                    rhs=wb[:, 2 * sp:2 * sp + 2, hsl],
                                    perf_mode=DR, start=(i == 0),
                                    stop=(i == 3 * FTI // 2 - 1))
                                i += 1
                    xr = lnp.tile([P, D], f16, name="xr3", tag="xr3",
                                  bufs=2)
                    nc.vector.tensor_scalar_mul(xr, pf, IWS)
                    nc.vector.tensor_add(xr, xr, y2r[:, u, :])
                    stats = lnp.tile([P, 2, 6], f32, name="st3", tag="st3")
                    nc.vector.bn_stats(out=stats[:, 0, :], in_=xr[:, 0:512])
                    nc.vector.bn_stats(out=stats[:, 1, :], in_=xr[:, 512:])
                    mv = lnp.tile([P, 2], f32, name="mv3", tag="mv3")
                    nc.vector.bn_aggr(out=mv, in_=stats)
                    lnv = lnp.tile([P, 1], f32, name="lnv3", tag="lnv3")
                    nc.scalar.activation(out=lnv, in_=mv[:, 1:2],
                                         func=ACT.Ln, bias=eps)
                    rstd = lnp.tile([P, 1], f32, name="rstd3", tag="rstd3")
                    nc.scalar.activation(out=rstd, in_=lnv, func=ACT.Exp,
                                         scale=-0.5)
                    n3 = lnp.tile([P, D], f16, name="n3", tag="n3", bufs=2)
                    nc.vector.tensor_scalar(out=n3, in0=xr,
                                            scalar1=mv[:, 0:1], scalar2=rstd,
                                            op0=ALU.subtract, op1=ALU.mult)
                    t1 = lnp.tile([P, D], f16, name="t13", tag="t13",
                                  bufs=2)
                    nc.vector.tensor_mul(t1, n3, g3b)
                    y3 = outp.tile([P, D], f32, name="y3", tag="y3")
                    nc.vector.tensor_add(y3, t1, b3b)
                    nc.sync.dma_start(out=out_d.ap()[u * P:(u + 1) * P, :],
                                      in_=y3)
        n2Tp.release()
        qT2p.release()
        n2p.release()
        y2rp.release()

    nc.compile()
    return nc


_CACHE = {}


def _get_nc():
    if "nc" not in _CACHE:
        _CACHE["nc"] = build_nc()
    return _CACHE["nc"]


def _q_indices(h):
    """Interleaved q-tile ownership: core-half h owns global tiles h, h+2..."""
    tiles = np.arange(h, 2 * QTI, 2)
    return (tiles[:, None] * P + np.arange(P)[None, :]).reshape(-1)


def _q8(x):
    return np.asarray(x, np.float32).astype(E4NP)


def _q8f(x):
    return _q8(x).astype(np.float32)


def _pack_dT(m):
    """[D, n] (d-major) -> [128, DTI, n] (partition, k-tile, col)."""
    return np.ascontiguousarray(
        m.reshape(DTI, P, -1).transpose(1, 0, 2))


def _hilo(m):
    hi = _q8(m)
    lo = _q8(np.asarray(m, np.float32) - hi.astype(np.float32))
    return hi, lo


def _prep_shared(inp):
    """Weight/vector arrays shared by all cores (host-side prep)."""
    f = lambda k: np.asarray(inp[k], np.float32)
    sh = {}
    for nm, key in [("wq1", "WQ1"), ("wk1", "WK1"), ("wv1", "WV1"),
                    ("wk2", "WK2"), ("wv2", "WV2")]:
        hi, lo = _hilo(WS * f(key))
        sh[nm] = _pack_dT(hi)
        if nm in ("wq1", "wk1", "wv1"):
            sh[nm + "lo"] = _pack_dT(lo)
    # wq2 with LN1 gamma folded; bias = be1 @ WQ2
    wq2p = WS * (f("g1")[:, None] * f("WQ2"))
    sh["wq2"] = _pack_dT(_q8(wq2p))
    sh["qb2"] = np.ascontiguousarray(
        (f("be1") @ f("WQ2")).reshape(DTI, P).T).astype(np.float32)
    # FFN weights: W1 with LN2 gamma folded, hi+lo interleaved; W2 hi+lo
    w1p = WS * (f("g2")[:, None] * f("W_ff1"))
    w1h, w1l = _hilo(w1p)
    w1h = w1h.reshape(DTI, P, FTI, P).transpose(2, 1, 0, 3)
    w1l = w1l.reshape(DTI, P, FTI, P).transpose(2, 1, 0, 3)
    sh["wf1"] = np.ascontiguousarray(
        np.stack([w1h, w1l], axis=2)).reshape(FTI * P, 2, DTI, P)
    w2h, w2l = _hilo(WS * f("W_ff2"))
    sh["wf2h"] = np.ascontiguousarray(
        w2h.reshape(FTI, P, D).transpose(1, 0, 2))
    sh["wf2l"] = np.ascontiguousarray(
        w2l.reshape(FTI, P, D).transpose(1, 0, 2))
    bh = f("be2") @ f("W_ff1") + f("b_ff1")
    sh["bf1"] = np.ascontiguousarray(bh.reshape(FTI, P).T).astype(np.float32)
    sh["g1"] = f("g1").astype(np.float16)
    sh["be1"] = f("be1").astype(np.float16)
    sh["g2"] = f("g2").astype(np.float16)
    sh["b2r"] = (f("be2") + f("b_ff2")).astype(np.float16)
    sh["g3"] = f("g3")
    sh["be3"] = f("be3")
    return sh


def _mask_blocks(h):
    """[128, 8, 128] fp8: boundary mask for self-attn score tile r=t-8c,
    applied to its q-block u4b=r//2 (the only block where the causal
    frontier can land).  r even: tri (h=0) / ones (h=1); r odd: zeros
    (h=0) / tri (h=1).  Hidden non-boundary blocks are never read."""
    tri = (np.arange(P)[:, None] <= np.arange(P)[None, :]).astype(np.float32)
    blocks = np.empty((DTI, P, P), np.float32)
    for r in range(DTI):
        cmp = 2 * (r // 2) + h - r
        blocks[r] = tri if cmp == 0 else (1.0 if cmp > 0 else 0.0)
    return np.ascontiguousarray(blocks.transpose(1, 0, 2)).astype(E4NP)


def _prep_core(c, y, Z, shared):
    b, h = c // 2, c % 2
    qi = _q_indices(h)
    yb16 = y[b].astype(np.float16)          # [S, D]
    yq16 = yb16[qi]                         # [NQ, D] own queries
    xkvT = yb16.T.astype(np.float32)        # [D, S]
    xqT = yq16.T.astype(np.float32)         # [D, NQ]
    zT = Z[b].astype(np.float16).T.astype(np.float32)
    m = {
        "xq8": _pack_dT(_q8(xqT)),
        "xq0lo": _pack_dT(_q8(xqT[:, 0:P] - _q8f(xqT[:, 0:P]))),
        "xkv8": _pack_dT(_q8(xkvT)),
        "xkvelo": _pack_dT(_q8(xkvT[:, 0:2 * P] - _q8f(xkvT[:, 0:2 * P]))),
        "z8": _pack_dT(_q8(zT)),
        "yres": np.ascontiguousarray(
            yq16.reshape(QTI, P, D).transpose(1, 0, 2)),
        "maskblk": _mask_blocks(h),
    }
    m.update(shared)
    return m


def kernel(**inputs):
    inp = {k: np.asarray(v) for k, v in inputs.items()}
    y = inp["y"].astype(np.float32)
    Z = inp["Z"].astype(np.float32)
    shared = _prep_shared(inp)
    in_maps = [_prep_core(c, y, Z, shared) for c in range(N_CORES)]
    res = run_bass_kernel_spmd(_get_nc(), in_maps, list(range(N_CORES)))
    out = np.zeros((4, 2048, 1024), np.float32)
    for c in range(N_CORES):
        b, h = c // 2, c % 2
        out[b, _q_indices(h)] = res.results[c]["out"]
    return out



# revision 52
# speedup vs baseline: 1.0912x; 1.0059x over previous
"""Trainium2 Bass kernel for a transformer decoder layer (self-attn +
cross-attn + FFN), fp8-e4m3 DoubleRow edition.

Sharding: 8 cores = 4 batches x 2 halves, no collectives. Core h of a batch
owns the interleaved query tiles {h, h+2, ..., h+14} (causal load balance) and
computes the FULL K/V projections for its batch locally (cheaper than the
pair-exchange collective at fp8 speeds).

Numerics: all matmuls run in fp8-e4m3 with DoubleRow perf mode (2 contraction
rows per partition).  Weights are pre-scaled x32 host-side so they sit in
e4m3's normal range; every PSUM drain folds the 1/32 back in.  Three
refinements keep absmax rel err ~3e-3 (gate is 2e-2):
  - FFN: both matmuls use hi+lo fp8 splits of activations AND weights
    (3 DoubleRow matmuls per logical matmul = fp16-level accuracy at 2x
    fp16 speed).
  - Early causal tokens (global positions 0..255, each core's local q-tile
    u=0) see few keys, so fp8 noise doesn't average out: their Q/K/V/E values
    are computed via the same hi+lo corrected path.
  - LayerNorm gammas/betas are folded into the next matmul's weights where
    possible (WQ2, W_ff1) and the residual carriers keep f16 precision.

Causal masking is via per-core precomputed [128 x 512] mask rows (tri/ones/
zeros blocks depending on core half), applied to each self-attn score tile,
so the SPMD program is uniform across cores.

Stage order: A1 (self K/V/Q + early) -> B (self-attn, prefetching A2 inputs)
-> A2 (cross K/V) -> T1 (n1 transpose) -> C0 (Q2) -> C (cross-attn) ->
T2 (n2 transpose hi/lo) -> D (FFN, token-chunked, streamed W_ff1).
"""

from contextlib import ExitStack

import ml_dtypes
import numpy as np

import concourse.bass as bass
import concourse.mybir as mybir
import concourse.tile as tile
from concourse import bacc
from concourse.bass_utils import run_bass_kernel_spmd
from concourse.masks import make_identity

f32 = mybir.dt.float32
f16 = mybir.dt.float16
f8 = mybir.dt.float8e4

P = 128
D = 1024          # d_model
S = 2048          # kv sequence length
NQ = 1024         # query tokens per core
DFF = 4096
DTI = D // P      # 8 d-model partition tiles
KTI = S // P      # 16 kv token tiles
QTI = NQ // P     # 8 query tiles
FTI = DFF // P    # 32 d_ff tiles
ACT = mybir.ActivationFunctionType
ALU = mybir.AluOpType
DR = mybir.MatmulPerfMode.DoubleRow
N_CORES = 8
WS = 32.0         # host-side weight pre-scale
IWS = 1.0 / WS
SCALE = 1.0 / 32.0  # 1/sqrt(D) softmax scale
E4NP = ml_dtypes.float8_e4m3


def build_nc():
    nc = bacc.Bacc("TRN2", target_bir_lowering=False, debug=False,
                   num_devices=N_CORES)

    def dp(name, shape, dt, out=False):
        return nc.declare_dram_parameter(name, shape, dt, isOutput=out)

    xq8_d = dp("xq8", [P, DTI, NQ], f8)
    xq0lo_d = dp("xq0lo", [P, DTI, P], f8)
    xkv8_d = dp("xkv8", [P, DTI, S], f8)
    xkvelo_d = dp("xkvelo", [P, DTI, 2 * P], f8)
    z8_d = dp("z8", [P, DTI, S], f8)
    yres_d = dp("yres", [P, QTI, D], f16)
    w_d = {n: dp(n, [P, DTI, D], f8)
           for n in ["wq1", "wk1", "wv1", "wq2", "wk2", "wv2",
                     "wq1lo", "wk1lo", "wv1lo"]}
    wf1_d = dp("wf1", [FTI * P, 2, DTI, P], f8)   # hi/lo interleaved
    wf2h_d = dp("wf2h", [P, FTI, D], f8)
    wf2l_d = dp("wf2l", [P, FTI, D], f8)
    bf1_d = dp("bf1", [P, FTI], f32)
    qb2_d = dp("qb2", [P, DTI], f32)
    mask_d = dp("maskblk", [P, DTI, P], f8)
    v16_d = {n: dp(n, [D], f16) for n in ["g1", "be1", "g2", "b2r"]}
    v32_d = {n: dp(n, [D], f32) for n in ["g3", "be3"]}
    out_d = dp("out", [NQ, D], f32, out=True)

    def bc(ap):  # broadcast a [n] dram vector across 128 partitions
        return bass.AP(tensor=ap.tensor, offset=ap.offset,
                       ap=[[0, P]] + [list(x) for x in ap.ap])

    with tile.TileContext(nc) as tc, ExitStack() as top:
        const = top.enter_context(tc.tile_pool(name="const", bufs=1))
        # one explicit act-table load (natural_log_exp_and_others: exp, ln,
        # copy, identity, relu, square) so every activation in the kernel is
        # servable without another table swap, regardless of how the
        # scheduler interleaves exp/rstd chains
        nc.scalar.add_instruction(mybir.InstLoadActFuncSet(
            name=f"I-{nc.next_id()}", act_func_set_id=6))
        ident = const.tile([P, P], f16, name="ident", tag="ident")
        make_identity(nc, ident)
        masks = const.tile([P, DTI, P], f8, name="masks", tag="masks")
        ones8t = const.tile([P, 2, 16], f8, name="ones8", tag="ones8")
        nc.vector.memset(ones8t, 1.0)
        ones8 = ones8t[:, :, 0:1]  # outer step 16B: dual-fp8 ldweights rule
        eps = const.tile([P, 1], f32, name="eps", tag="eps")
        nc.vector.memset(eps, 1e-5)
        bf1sb = const.tile([P, FTI], f32, name="bf1sb", tag="bf1sb")
        qb2sb = const.tile([P, DTI], f32, name="qb2sb", tag="qb2sb")

        def vload(name, dt, dram):
            return const.tile([P, D], dt, name=f"{name}b", tag=f"{name}b")

        g1b = vload("g1", f16, v16_d)
        b1b = vload("be1", f16, v16_d)
        g2b = vload("g2", f16, v16_d)
        b2rb = vload("b2r", f16, v16_d)

        def load_consts():
            # deferred off-critical-path constant loads
            nc.sync.dma_start(out=masks, in_=mask_d.ap())
            nc.sync.dma_start(out=bf1sb, in_=bf1_d.ap())
            nc.sync.dma_start(out=qb2sb, in_=qb2_d.ap())
            for t, nm, dd in [(g1b, "g1", v16_d), (b1b, "be1", v16_d),
                              (g2b, "g2", v16_d), (b2rb, "b2r", v16_d)]:
                nc.sync.dma_start(out=t, in_=bc(dd[nm].ap()))

        # ---- persistent pools; LIFO per side ----
        # left: kv2p (bottom; dies after cross), y1, n1, [zpB/wpB], [n1T],
        #       [n2T]
        # right: yres, kvp, earlyp | y2r, n2, qT2p
        kv2p = tc.alloc_tile_pool(name="kv2p", bufs=1)
        kT2 = kv2p.tile([P, DTI, S], f8, name="kT2", tag="kT2")
        v2 = kv2p.tile([P, KTI, D], f8, name="v2", tag="v2")
        y1p = tc.alloc_tile_pool(name="y1p", bufs=1)
        y1 = y1p.tile([P, QTI, D], f16, name="y1", tag="y1")
        n1p = tc.alloc_tile_pool(name="n1p", bufs=1)
        n1 = n1p.tile([P, QTI, D], f16, name="n1", tag="n1")

        yresp = tc.alloc_tile_pool(name="yresp", bufs=1, side="right")
        yres = yresp.tile([P, QTI, D], f16, name="yres", tag="yres")
        kvp = tc.alloc_tile_pool(name="kvp", bufs=1, side="right")
        kT = kvp.tile([P, DTI, S], f8, name="kT", tag="kT")
        v = kvp.tile([P, KTI, D], f8, name="v", tag="v")
        qT = kvp.tile([P, DTI, NQ], f8, name="qT", tag="qT")
        earlyp = tc.alloc_tile_pool(name="earlyp", bufs=1, side="right")
        keT = [earlyp.tile([P, DTI, 2 * P], f8, name=f"keT{x}", tag=f"keT{x}")
               for x in range(2)]  # hi, lo
        qeT = [earlyp.tile([P, DTI, P], f8, name=f"qeT{x}", tag=f"qeT{x}")
               for x in range(2)]
        ve = [earlyp.tile([P, 2, D], f8, name=f"ve{x}", tag=f"ve{x}")
              for x in range(2)]

        def dr_acc(ps, terms, rhs_sl, lhs_sl):
            """Accumulate sum of DoubleRow products into psum region ps.
            terms: list of (lhsT_tile, rhs_tile); contraction over DTI//2
            k-tile pairs per term. rhs_sl/lhs_sl: fn(tile, g) -> AP."""
            n = len(terms) * (DTI // 2)
            i = 0
            for lt, rt in terms:
                for g in range(DTI // 2):
                    nc.tensor.matmul(ps, lhsT=lhs_sl(lt, g),
                                     rhs=rhs_sl(rt, g), perf_mode=DR,
                                     start=(i == 0), stop=(i == n - 1))
                    i += 1

        def split3(pool, ps, scale, bias, func, hi_out, lo_out, eng, n):
            """3-op hi/lo drain: t16 = func(scale*ps + bias); hi = q8(t16);
            lo = q8(t16 - hi)."""
            t16 = pool.tile([P, n], f16, name="t16", tag="t16", bufs=3)
            nc.scalar.activation(out=t16, in_=ps, func=func, bias=bias,
                                 scale=scale)
            if eng == 0:
                nc.vector.tensor_copy(out=hi_out, in_=t16)
                nc.gpsimd.tensor_sub(lo_out, t16, hi_out)
            else:
                nc.gpsimd.tensor_copy(out=hi_out, in_=t16)
                nc.vector.tensor_sub(lo_out, t16, hi_out)

        # ==================== stage A1: self-attn projections =============
        with ExitStack() as stA:
            wpA = stA.enter_context(tc.tile_pool(name="wpA", bufs=1))
            xpA = stA.enter_context(tc.tile_pool(name="xpA", bufs=1))
            psA = stA.enter_context(tc.tile_pool(name="psA", bufs=3,
                                                 space="PSUM"))
            psE = stA.enter_context(tc.tile_pool(name="psE", bufs=2,
                                                 space="PSUM"))
            drp = stA.enter_context(tc.tile_pool(name="drpA", bufs=1))

            def wload(tag, name):
                t = wpA.tile([P, DTI, D], f8, name=name, tag=tag)
                nc.sync.dma_start(out=t, in_=w_d[name].ap())
                return t

            # first loads chunked so K1 (j=0, th=0) can start after ~1/4 of
            # the wk1+xkv8 bytes have landed
            wk1 = wpA.tile([P, DTI, D], f8, name="wk1", tag="wA0")
            xkv8 = xpA.tile([P, DTI, S], f8, name="xkv8", tag="xkv8")
            nc.sync.dma_start(out=wk1[:, :, 0:512],
                              in_=w_d["wk1"].ap()[:, :, 0:512])
            nc.sync.dma_start(out=xkv8[:, :, 0:1024],
                              in_=xkv8_d.ap()[:, :, 0:1024])
            nc.sync.dma_start(out=wk1[:, :, 512:1024],
                              in_=w_d["wk1"].ap()[:, :, 512:1024])
            nc.sync.dma_start(out=xkv8[:, :, 1024:2048],
                              in_=xkv8_d.ap()[:, :, 1024:2048])
            wv1 = wload("wA1", "wv1")
            wk1lo = wload("wA2", "wk1lo")
            wv1lo = wload("wA3", "wv1lo")
            xkvelo = xpA.tile([P, DTI, 2 * P], f8, name="xkvelo",
                              tag="xkvelo")
            nc.sync.dma_start(out=xkvelo, in_=xkvelo_d.ap())
            xq8 = xpA.tile([P, DTI, NQ], f8, name="xq8", tag="xq8")
            nc.sync.dma_start(out=xq8, in_=xq8_d.ap())
            xq0lo = xpA.tile([P, DTI, P], f8, name="xq0lo", tag="xq0lo")
            nc.sync.dma_start(out=xq0lo, in_=xq0lo_d.ap())
            nc.sync.dma_start(out=yres, in_=yres_d.ap())
            load_consts()

            # K1: kT[:, j, :] = (wk1.T @ xkv)/32, d_out on partitions
            for j in range(DTI):
                for th in range(2):
                    ps = psA.tile([P, 1024], f32, name="psp", tag="psp")
                    for sub in range(2):
                        tsl = slice(th * 1024 + sub * 512,
                                    th * 1024 + sub * 512 + 512)
                        dr_acc(ps[:, sub * 512:sub * 512 + 512],
                               [(wk1, xkv8)],
                               lambda t, g, tsl=tsl: t[:, 2 * g:2 * g + 2, tsl],
                               lambda t, g, j=j: t[:, 2 * g:2 * g + 2,
                                                   j * P:(j + 1) * P])
                    osl = kT[:, j, th * 1024:(th + 1) * 1024]
                    if (j + th) % 2 == 0:
                        nc.scalar.activation(out=osl, in_=ps, func=ACT.Copy,
                                             scale=IWS)
                    else:
                        nc.vector.tensor_scalar_mul(osl, ps, IWS)

            # early K (tokens 0:256), hi+lo corrected
            for j in range(DTI):
                ps = psE.tile([P, 512], f32, name="pse", tag="pse")
                dr_acc(ps[:, 0:256],
                       [(wk1, xkv8), (wk1, xkvelo), (wk1lo, xkv8)],
                       lambda t, g: (t[:, 2 * g:2 * g + 2, 0:256]
                                     if t is xkv8 else
                                     t[:, 2 * g:2 * g + 2, :]),
                       lambda t, g, j=j: t[:, 2 * g:2 * g + 2,
                                           j * P:(j + 1) * P])
                split3(drp, ps[:, 0:256], IWS, 0.0, ACT.Copy,
                       keT[0][:, j, :], keT[1][:, j, :], j % 2, 256)

            # V1: v[:, t, :] = (xkv.T @ wv1)/32, tokens on partitions
            for t in range(KTI):
                ps = psA.tile([P, 1024], f32, name="psp", tag="psp")
                for half in range(2):
                    dr_acc(ps[:, half * 512:half * 512 + 512],
                           [(xkv8, wv1)],
                           lambda tt, g, half=half: tt[:, 2 * g:2 * g + 2,
                                                       half * 512:half * 512 + 512],
                           lambda tt, g, t=t: tt[:, 2 * g:2 * g + 2,
                                                 t * P:(t + 1) * P])
                osl = v[:, t, :]
                if t % 2 == 0:
                    nc.scalar.activation(out=osl, in_=ps, func=ACT.Copy,
                                         scale=IWS)
                else:
                    nc.vector.tensor_scalar_mul(osl, ps, IWS)

            # early V (k-tiles 0..1), hi+lo corrected
            for t in range(2):
                for half in range(2):
                    ps = psE.tile([P, 512], f32, name="pse", tag="pse")
                    hsl = slice(half * 512, half * 512 + 512)
                    dr_acc(ps,
                           [(xkv8, wv1), (xkvelo, wv1), (xkv8, wv1lo)],
                           lambda tt, g, hsl=hsl: tt[:, 2 * g:2 * g + 2, hsl],
                           lambda tt, g, t=t: tt[:, 2 * g:2 * g + 2,
                                                 t * P:(t + 1) * P])
                    split3(drp, ps, IWS, 0.0, ACT.Copy,
                           ve[0][:, t, hsl], ve[1][:, t, hsl],
                           (t + half) % 2, 512)

            # Q1 (weights reuse the K1 buffers)
            wq1 = wload("wA0", "wq1")
            wq1lo = wload("wA2", "wq1lo")
            for j in range(DTI):
                ps = psA.tile([P, 1024], f32, name="psp", tag="psp")
                for sub in range(2):
                    dr_acc(ps[:, sub * 512:sub * 512 + 512],
                           [(wq1, xq8)],
                           lambda t, g, sub=sub: t[:, 2 * g:2 * g + 2,
                                                   sub * 512:sub * 512 + 512],
                           lambda t, g, j=j: t[:, 2 * g:2 * g + 2,
                                               j * P:(j + 1) * P])
                osl = qT[:, j, :]
                if j % 2 == 0:
                    nc.scalar.activation(out=osl, in_=ps, func=ACT.Copy,
                                         scale=IWS)
                else:
                    nc.vector.tensor_scalar_mul(osl, ps, IWS)
            # early Q (own u=0 tile)
            for j in range(DTI):
                ps = psE.tile([P, 512], f32, name="pse", tag="pse")
                dr_acc(ps[:, 0:P],
                       [(wq1, xq8), (wq1, xq0lo), (wq1lo, xq8)],
                       lambda t, g: (t[:, 2 * g:2 * g + 2, 0:P]
                                     if t is xq8 else
                                     t[:, 2 * g:2 * g + 2, :]),
                       lambda t, g, j=j: t[:, 2 * g:2 * g + 2,
                                           j * P:(j + 1) * P])
                split3(drp, ps[:, 0:P], IWS, 0.0, ACT.Copy,
                       qeT[0][:, j, :], qeT[1][:, j, :], j % 2, P)

        # ==================== attention helper ============================
        def pump(gen, n=1):
            """Advance a filler emission generator n steps (no-op if None)."""
            for _ in range(n):
                if gen is None or next(gen, "END") == "END":
                    return

        def att_pools(stk, tagp, score_bufs, out_bufs=2):
            pss = stk.enter_context(tc.tile_pool(name=f"{tagp}pss",
                                                 bufs=score_bufs,
                                                 space="PSUM"))
            pso = stk.enter_context(tc.tile_pool(name=f"{tagp}pso",
                                                 bufs=out_bufs,
                                                 space="PSUM"))
            ep = stk.enter_context(tc.tile_pool(name=f"{tagp}ep", bufs=1))
            lnp = stk.enter_context(tc.tile_pool(name=f"{tagp}lnp", bufs=4))
            return tagp, pss, pso, ep, lnp

        def attention_half(ap_, c, qTt, kTt, vt, resid_sl, gb, bb, yout,
                           nout, masked, filler=None, filler_late="same"):
            tagp, pss, pso, ep, lnp = ap_
            if filler_late == "same":
                filler_late = filler
            nvis = 8 * (c + 1) if masked else KTI
            if True:
                e = ep.tile([P, nvis, 512], f8, name=f"e{c}", tag=f"e{c}")
                ee = None
                if masked and c == 0:
                    # early corrected scores/E for q-tile u=0, k-tiles 0..1
                    # (emitted first so its long drain chain overlaps the
                    # main score tiles)
                    ee = lnp.tile([P, 2, 2, P], f8, name="ee", tag="ee",
                                  bufs=1)
                    for t in range(2):
                        ps = pss.tile([P, 1024], f32, name="ps_s", tag="ps_s")
                        dr_acc(ps[:, 0:P],
                               [(keT[0], qeT[0]), (keT[0], qeT[1]),
                                (keT[1], qeT[0])],
                               lambda tt, g: tt[:, 2 * g:2 * g + 2, :],
                               lambda tt, g, t=t: tt[:, 2 * g:2 * g + 2,
                                                     t * P:(t + 1) * P])
                        tm = lnp.tile([P, P], f16, name="etm", tag="etm",
                                      bufs=2)
                        nc.scalar.activation(out=tm, in_=ps[:, 0:P],
                                             func=ACT.Exp, scale=SCALE)
                        nc.vector.tensor_mul(tm, tm, masks[:, t, :])
                        nc.vector.tensor_copy(out=ee[:, 0, t, :], in_=tm)
                        nc.gpsimd.tensor_sub(ee[:, 1, t, :], tm,
                                             ee[:, 0, t, :])
                # scores + exp, two k-tiles per psum tile / exp instruction;
                # causal mask only touches the boundary q-block of each tile
                # (hidden non-boundary blocks are never read downstream)
                for dual in range(nvis // 2):
                    ps = pss.tile([P, 1024], f32, name="ps_s", tag="ps_s")
                    for k in range(2):
                        t = 2 * dual + k
                        dr_acc(ps[:, k * 512:k * 512 + 512], [(kTt, qTt)],
                               lambda tt, g, c=c: tt[:, 2 * g:2 * g + 2,
                                                     c * 512:c * 512 + 512],
                               lambda tt, g, t=t: tt[:, 2 * g:2 * g + 2,
                                                     t * P:(t + 1) * P])
                    nc.scalar.activation(out=e[:, 2 * dual:2 * dual + 2, :],
                                         in_=ps, func=ACT.Exp, scale=SCALE)
                    if masked:
                        for k in range(2):
                            t = 2 * dual + k
                            if t < 8 * c:
                                continue
                            r = t - 8 * c
                            u4b = r // 2
                            esl = e[:, t, u4b * P:(u4b + 1) * P]
                            if r % 2 == 0:
                                nc.vector.tensor_mul(esl, esl, masks[:, r, :])
                            else:
                                nc.gpsimd.tensor_mul(esl, esl, masks[:, r, :])
                    pump(filler)
                # denominators: E^T @ ones -> [128 q, 1] per u4 column of a
                # psum tile (q on partitions; no DRAM transpose round-trip)
                pd = pso.tile([P, 1024], f32, name="pd", tag="po")
                for u4 in range(4):
                    if ee is not None and u4 == 0:
                        for hl in range(2):
                            nc.tensor.matmul(pd[:, 0:1],
                                             lhsT=ee[:, hl, :, :],
                                             rhs=ones8, perf_mode=DR,
                                             start=(hl == 0), stop=(hl == 1))
                        continue
                    np_ = (4 * c + u4 + 1) if masked else 8
                    for i in range(np_):
                        nc.tensor.matmul(
                            pd[:, u4:u4 + 1],
                            lhsT=e[:, 2 * i:2 * i + 2, u4 * P:(u4 + 1) * P],
                            rhs=ones8, perf_mode=DR,
                            start=(i == 0), stop=(i == np_ - 1))
                recT = lnp.tile([P, 4], f32, name="recT", tag="recT")
                nc.vector.reciprocal(recT, pd[:, 0:4])
                pump(filler_late)
                u4order = [1, 2, 3, 0] if ee is not None else range(4)
                for u4 in u4order:
                    u = c * 4 + u4
                    po = pso.tile([P, 1024], f32, name="po", tag="po")
                    if ee is not None and u == 0:
                        for half in range(2):
                            hsl = slice(half * 512, half * 512 + 512)
                            for ti, (el, vl) in enumerate(
                                    [(0, 0), (1, 0), (0, 1)]):
                                nc.tensor.matmul(
                                    po[:, hsl], lhsT=ee[:, el, :, :],
                                    rhs=ve[vl][:, :, hsl], perf_mode=DR,
                                    start=(ti == 0), stop=(ti == 2))
                    else:
                        np_ = (u + 1) if masked else 8
                        for half in range(2):
                            hsl = slice(half * 512, half * 512 + 512)
                            for i in range(np_):
                                nc.tensor.matmul(
                                    po[:, hsl],
                                    lhsT=e[:, 2 * i:2 * i + 2,
                                           u4 * P:(u4 + 1) * P],
                                    rhs=vt[:, 2 * i:2 * i + 2, hsl],
                                    perf_mode=DR, start=(i == 0),
                                    stop=(i == np_ - 1))
                    xr = lnp.tile([P, D], f16, name="xr", tag="xr", bufs=2)
                    nc.scalar.activation(out=xr, in_=po, func=ACT.Copy,
                                         scale=recT[:, u4:u4 + 1])
                    nc.vector.tensor_add(xr, xr, resid_sl(u))
                    pump(filler_late)
                    # LN core + affine
                    stats = lnp.tile([P, 2, 6], f32, name="stats",
                                     tag="stats")
                    nc.vector.bn_stats(out=stats[:, 0, :], in_=xr[:, 0:512])
                    nc.vector.bn_stats(out=stats[:, 1, :], in_=xr[:, 512:])
                    mv = lnp.tile([P, 2], f32, name="mv", tag="mv")
                    nc.vector.bn_aggr(out=mv, in_=stats)
                    # rstd = exp(-0.5*ln(var+eps)) -- stays in act table 6
                    lnv = lnp.tile([P, 1], f32, name="lnv", tag="lnv")
                    nc.scalar.activation(out=lnv, in_=mv[:, 1:2],
                                         func=ACT.Ln, bias=eps)
                    rstd = lnp.tile([P, 1], f32, name="rstd", tag="rstd")
                    nc.scalar.activation(out=rstd, in_=lnv, func=ACT.Exp,
                                         scale=-0.5)
                    nsl = nout[:, u, :]
                    nc.vector.tensor_scalar(out=nsl, in0=xr,
                                            scalar1=mv[:, 0:1], scalar2=rstd,
                                            op0=ALU.subtract, op1=ALU.mult)
                    t1 = lnp.tile([P, D], f16, name="lt0", tag="lt0", bufs=2)
                    nc.vector.tensor_mul(t1, nsl, gb)
                    nc.gpsimd.tensor_add(yout[:, u, :], t1, bb)
                    pump(filler_late, 2)

        # ==================== stage B: self-attention + LN1 ===============
        # Cross-attn K2/V2 projections are emitted as FILLER inside the
        # self-attention instruction stream: the PE chews them while the
        # Act/DVE engines work through exp + LayerNorm chains.
        with ExitStack() as stB:
            # prefetch stage-A2 inputs while attention runs
            zpB = stB.enter_context(tc.tile_pool(name="zpB", bufs=1))
            wk2 = zpB.tile([P, DTI, D], f8, name="wk2", tag="wk2")
            nc.sync.dma_start(out=wk2, in_=w_d["wk2"].ap())
            z8 = zpB.tile([P, DTI, S], f8, name="z8", tag="z8")
            nc.sync.dma_start(out=z8[:, :, 0:1024],
                              in_=z8_d.ap()[:, :, 0:1024])
            nc.sync.dma_start(out=z8[:, :, 1024:2048],
                              in_=z8_d.ap()[:, :, 1024:2048])
            wv2 = zpB.tile([P, DTI, D], f8, name="wv2", tag="wv2")
            nc.sync.dma_start(out=wv2, in_=w_d["wv2"].ap())

            def emit_k2(pool, j, th, eng):
                ps = pool.tile([P, 1024], f32, name="psp2", tag="psp2")
                for sub in range(2):
                    tsl = slice(th * 1024 + sub * 512,
                                th * 1024 + sub * 512 + 512)
                    dr_acc(ps[:, sub * 512:sub * 512 + 512], [(wk2, z8)],
                           lambda t, g, tsl=tsl: t[:, 2 * g:2 * g + 2, tsl],
                           lambda t, g, j=j: t[:, 2 * g:2 * g + 2,
                                               j * P:(j + 1) * P])
                osl = kT2[:, j, th * 1024:(th + 1) * 1024]
                if eng == 0:
                    nc.vector.tensor_scalar_mul(osl, ps, IWS)
                else:
                    nc.scalar.activation(out=osl, in_=ps, func=ACT.Copy,
                                         scale=IWS)

            def emit_v2(pool, t, eng):
                ps = pool.tile([P, 1024], f32, name="psp2", tag="psp2")
                for half in range(2):
                    dr_acc(ps[:, half * 512:half * 512 + 512], [(z8, wv2)],
                           lambda tt, g, half=half: tt[:, 2 * g:2 * g + 2,
                                                       half * 512:half * 512 + 512],
                           lambda tt, g, t=t: tt[:, 2 * g:2 * g + 2,
                                                 t * P:(t + 1) * P])
                osl = v2[:, t, :]
                if eng == 0:
                    nc.vector.tensor_scalar_mul(osl, ps, IWS)
                else:
                    nc.scalar.activation(out=osl, in_=ps, func=ACT.Copy,
                                         scale=IWS)

            A2G = ([("k", j, 0) for j in range(DTI)]
                   + [("k", j, 1) for j in range(DTI)]
                   + [("v", t, 0) for t in range(KTI)])

            def gen_a2(pool, groups):
                for gi, g in enumerate(groups):
                    eng = gi % 2
                    if g[0] == "k":
                        emit_k2(pool, g[1], g[2], eng)
                    else:
                        emit_v2(pool, g[1], eng)
                    yield

            NFILL = 20
            with ExitStack() as stB2:
                ap_ = att_pools(stB2, "sa_", score_bufs=1)
                psA2 = stB2.enter_context(tc.tile_pool(name="psA2", bufs=1,
                                                       space="PSUM"))
                attention_half(ap_, 0, qT, kT, v, lambda u: yres[:, u, :],
                               g1b, b1b, y1, n1, masked=True)
                earlyp.release()
                a2 = gen_a2(psA2, A2G[:NFILL])
                attention_half(ap_, 1, qT, kT, v, lambda u: yres[:, u, :],
                               g1b, b1b, y1, n1, masked=True, filler=a2)
                pump(a2, 99)
            # leftover A2 groups run dense with triple-buffered psum
            with ExitStack() as stA2t:
                psA2t = stA2t.enter_context(tc.tile_pool(name="psA2t",
                                                         bufs=3,
                                                         space="PSUM"))
                pump(gen_a2(psA2t, A2G[NFILL:]), 99)
            kvp.release()
            yresp.release()

            y2rp = tc.alloc_tile_pool(name="y2rp", bufs=1, side="right")
            y2r = y2rp.tile([P, QTI, D], f16, name="y2r", tag="y2r")
            n2p = tc.alloc_tile_pool(name="n2p", bufs=1, side="right")
            n2 = n2p.tile([P, QTI, D], f16, name="n2", tag="n2")

        # ======= stages T1/C0/cross/T2: pipelined with cross-attention ====
        # T1(c=0)+Q2(sub 0) run dense before cross; T1(c=1)+Q2(sub 1) fill
        # cross c0's exp/LN bubbles; T2(c=0) fills cross c1's; T2(c=1) is
        # pumped inside the FFN mm1 loop.
        n1Tp = tc.alloc_tile_pool(name="n1Tp", bufs=1)
        n1T = n1Tp.tile([P, DTI, NQ], f8, name="n1T", tag="n1T")
        wpC = tc.alloc_tile_pool(name="wpC", bufs=1)
        wq2 = wpC.tile([P, DTI, D], f8, name="wq2", tag="wq2")
        nc.sync.dma_start(out=wq2, in_=w_d["wq2"].ap())
        qT2p = tc.alloc_tile_pool(name="qT2p", bufs=1, side="right")
        qT2 = qT2p.tile([P, DTI, NQ], f8, name="qT2", tag="qT2")
        n2Tp = tc.alloc_tile_pool(name="n2Tp", bufs=1, side="right")
        n2T = [n2Tp.tile([P, DTI, NQ], f8, name=f"n2T{x}", tag=f"n2T{x}")
               for x in range(2)]

        def gen_t1(pst, c2, engs):
            for i in range(DTI):
                pt = pst.tile([P, 512], f16, name="pt", tag="pt")
                for u4 in range(4):
                    nc.tensor.transpose(
                        pt[:, u4 * P:(u4 + 1) * P],
                        in_=n1[:, c2 * 4 + u4, i * P:(i + 1) * P],
                        identity=ident)
                osl = n1T[:, i, c2 * 512:c2 * 512 + 512]
                if engs[i % len(engs)] == "a":
                    nc.scalar.activation(out=osl, in_=pt, func=ACT.Copy,
                                         scale=1.0)
                else:
                    nc.vector.tensor_copy(out=osl, in_=pt)
                yield

        def gen_q2(psC, sub, engs):
            ssl = slice(sub * 512, sub * 512 + 512)
            for j in range(DTI):
                ps = psC.tile([P, 512], f32, name="psq2", tag="psq2")
                dr_acc(ps, [(wq2, n1T)],
                       lambda t, g, ssl=ssl: t[:, 2 * g:2 * g + 2, ssl],
                       lambda t, g, j=j: t[:, 2 * g:2 * g + 2,
                                           j * P:(j + 1) * P])
                osl = qT2[:, j, ssl]
                if engs[j % len(engs)] == "a":
                    nc.scalar.activation(out=osl, in_=ps,
                                         func=ACT.Identity,
                                         bias=qb2sb[:, j:j + 1], scale=IWS)
                else:
                    nc.vector.tensor_scalar(out=osl, in0=ps, scalar1=IWS,
                                            scalar2=qb2sb[:, j:j + 1],
                                            op0=ALU.mult, op1=ALU.add)
                yield

        def gen_t2(pst, c2, engs):
            for i in range(DTI):
                pt = pst.tile([P, 512], f16, name="pt", tag="pt")
                for u4 in range(4):
                    nc.tensor.transpose(
                        pt[:, u4 * P:(u4 + 1) * P],
                        in_=n2[:, c2 * 4 + u4, i * P:(i + 1) * P],
                        identity=ident)
                csl = slice(c2 * 512, c2 * 512 + 512)
                if engs[i % len(engs)] == "a":
                    nc.scalar.activation(out=n2T[0][:, i, csl], in_=pt,
                                         func=ACT.Copy, scale=1.0)
                else:
                    nc.vector.tensor_copy(out=n2T[0][:, i, csl], in_=pt)
                nc.vector.tensor_sub(n2T[1][:, i, csl], pt,
                                     n2T[0][:, i, csl])
                yield

        def chain(*gens):
            for g in gens:
                yield from g

        with ExitStack() as stC0:
            pst = stC0.enter_context(tc.tile_pool(name="pstC", bufs=2,
                                                  space="PSUM"))
            psC = stC0.enter_context(tc.tile_pool(name="psC", bufs=2,
                                                  space="PSUM"))
            pump(gen_t1(pst, 0, "av"), 99)
            pump(gen_q2(psC, 0, "av"), 99)
            ap_ = att_pools(stC0, "ca_", score_bufs=1, out_bufs=1)
            fill0 = chain(gen_t1(pst, 1, "va"), gen_q2(psC, 1, "va"))
            attention_half(ap_, 0, qT2, kT2, v2, lambda u: y1[:, u, :],
                           g2b, b2rb, y2r, n2, masked=False, filler=fill0)
            pump(fill0, 99)
        with ExitStack() as stC1:
            pst = stC1.enter_context(tc.tile_pool(name="pstC1", bufs=2,
                                                  space="PSUM"))
            ap_ = att_pools(stC1, "cb_", score_bufs=1, out_bufs=2)
            fill1 = gen_t2(pst, 0, "va")
            attention_half(ap_, 1, qT2, kT2, v2, lambda u: y1[:, u, :],
                           g2b, b2rb, y2r, n2, masked=False, filler=fill1)
            pump(fill1, 99)
        wpC.release()
        n1Tp.release()
        n1p.release()
        y1p.release()
        kv2p.release()

        # ==================== stage D: FFN + LN3 + output =================
        with ExitStack() as stD:
            wf2p = stD.enter_context(tc.tile_pool(name="wf2p", bufs=1))
            wf1p = stD.enter_context(tc.tile_pool(name="wf1p", bufs=3))
            hp = stD.enter_context(tc.tile_pool(name="hp", bufs=1))
            psH = stD.enter_context(tc.tile_pool(name="psH", bufs=2,
                                                 space="PSUM"))
            psF = stD.enter_context(tc.tile_pool(name="psF", bufs=2,
                                                 space="PSUM"))
            pstD = stD.enter_context(tc.tile_pool(name="pstD", bufs=2,
                                                  space="PSUM"))
            drp = stD.enter_context(tc.tile_pool(name="drpD", bufs=1))
            lnp = stD.enter_context(tc.tile_pool(name="lnpD", bufs=4))
            outp = stD.enter_context(tc.tile_pool(name="outp", bufs=2))
            t2g1 = gen_t2(pstD, 1, "avv")
            # wf2/g3/b3 loads are chunked and interleaved between the
            # streamed w1t loads so they don't head-block the first FFN
            # matmuls on the DMA queue
            wf2h = wf2p.tile([P, FTI, D], f8, name="wf2h", tag="wf2h")
            wf2l = wf2p.tile([P, FTI, D], f8, name="wf2l", tag="wf2l")
            g3b = wf2p.tile([P, D], f32, name="g3b", tag="g3b")
            b3b = wf2p.tile([P, D], f32, name="b3b", tag="b3b")
            for c in range(2):
                csl = slice(c * 512, c * 512 + 512)
                hh = hp.tile([P, FTI, 512], f8, name="hh", tag="hh")
                hl = hp.tile([P, FTI, 512], f8, name="hl", tag="hl")
                for s in range(FTI):
                    w1t = wf1p.tile([P, 2, DTI, P], f8, name="w1t",
                                    tag="w1t")
                    nc.sync.dma_start(out=w1t,
                                      in_=wf1_d.ap()[s * P:(s + 1) * P])
                    if c == 0:
                        if s % 2 == 0 and s // 2 < 8:
                            ch = s // 2
                            nc.sync.dma_start(
                                out=wf2h[:, ch * 4:(ch + 1) * 4, :],
                                in_=wf2h_d.ap()[:, ch * 4:(ch + 1) * 4, :])
                        elif s == 1:
                            nc.sync.dma_start(out=g3b,
                                              in_=bc(v32_d["g3"].ap()))
                        elif s == 3:
                            nc.sync.dma_start(out=b3b,
                                              in_=bc(v32_d["be3"].ap()))
                        elif s % 2 == 1 and 5 <= s <= 19:
                            ch = (s - 5) // 2
                            nc.sync.dma_start(
                                out=wf2l[:, ch * 4:(ch + 1) * 4, :],
                                in_=wf2l_d.ap()[:, ch * 4:(ch + 1) * 4, :])
                    ps = psH.tile([P, 512], f32, name="ph", tag="ph")
                    i = 0
                    for wi, xi in [(0, 0), (0, 1), (1, 0)]:
                        for g in range(DTI // 2):
                            nc.tensor.matmul(
                                ps,
                                lhsT=w1t[:, wi, 2 * g:2 * g + 2, :],
                                rhs=n2T[xi][:, 2 * g:2 * g + 2, csl],
                                perf_mode=DR, start=(i == 0),
                                stop=(i == 3 * DTI // 2 - 1))
                            i += 1
                    split3(drp, ps, IWS, bf1sb[:, s:s + 1], ACT.Relu,
                           hh[:, s, :], hl[:, s, :], s % 2, 512)
                    if c == 0 and s % 2 == 1:
                        pump(t2g1)
                if c == 0:
                    pump(t2g1, 99)
                for u4 in range(4):
                    u = c * 4 + u4
                    pf = psF.tile([P, 1024], f32, name="pf", tag="pf")
                    usl = slice(u4 * P, (u4 + 1) * P)
                    # drain each d-half as soon as its matmuls finish so the
                    # final u's exposed tail is ~half an LN chain
                    xr = lnp.tile([P, D], f16, name="xr3", tag="xr3",
                                  bufs=2)
                    stats = lnp.tile([P, 2, 6], f32, name="st3", tag="st3")
                    for half in range(2):
                        hsl = slice(half * 512, half * 512 + 512)
                        i = 0
                        for ha, wb in [(hh, wf2h), (hl, wf2h), (hh, wf2l)]:
                            for sp in range(FTI // 2):
                                nc.tensor.matmul(
                                    pf[:, hsl],
                                    lhsT=ha[:, 2 * sp:2 * sp + 2, usl],
                                    rhs=wb[:, 2 * sp:2 * sp + 2, hsl],
                                    perf_mode=DR, start=(i == 0),
                                    stop=(i == 3 * FTI // 2 - 1))
                                i += 1
                        if half == 0:
                            nc.scalar.activation(out=xr[:, hsl],
                                                 in_=pf[:, hsl],
                                                 func=ACT.Copy, scale=IWS)
                            nc.gpsimd.tensor_add(xr[:, hsl], xr[:, hsl],
                                                 y2r[:, u, hsl])
                        else:
                            nc.vector.tensor_scalar_mul(xr[:, hsl],
                                                        pf[:, hsl], IWS)
                            nc.vector.tensor_add(xr[:, hsl], xr[:, hsl],
                                                 y2r[:, u, hsl])
                        nc.vector.bn_stats(out=stats[:, half, :],
                                           in_=xr[:, hsl])
                    mv = lnp.tile([P, 2], f32, name="mv3", tag="mv3")
                    nc.vector.bn_aggr(out=mv, in_=stats)
                    lnv = lnp.tile([P, 1], f32, name="lnv3", tag="lnv3")
                    nc.scalar.activation(out=lnv, in_=mv[:, 1:2],
                                         func=ACT.Ln, bias=eps)
                    rstd = lnp.tile([P, 1], f32, name="rstd3", tag="rstd3")
                    nc.scalar.activation(out=rstd, in_=lnv, func=ACT.Exp,
                                         scale=-0.5)
                    n3 = lnp.tile([P, D], f16, name="n3", tag="n3", bufs=2)
                    t1 = lnp.tile([P, D], f16, name="t13", tag="t13",
                                  bufs=2)
                    y3 = outp.tile([P, D], f32, name="y3", tag="y3")
                    for half in range(2):
                        hsl = slice(half * 512, half * 512 + 512)
                        nc.vector.tensor_scalar(out=n3[:, hsl],
                                                in0=xr[:, hsl],
                                                scalar1=mv[:, 0:1],
                                                scalar2=rstd,
                                                op0=ALU.subtract,
                                                op1=ALU.mult)
                        if half == 0:
                            nc.gpsimd.tensor_mul(t1[:, hsl], n3[:, hsl],
                                                 g3b[:, hsl])
                            nc.vector.tensor_add(y3[:, hsl], t1[:, hsl],
                                                 b3b[:, hsl])
                        else:
                            nc.vector.tensor_mul(t1[:, hsl], n3[:, hsl],
                                                 g3b[:, hsl])
                            nc.gpsimd.tensor_add(y3[:, hsl], t1[:, hsl],
                                                 b3b[:, hsl])
                        nc.sync.dma_start(
                            out=out_d.ap()[u * P:(u + 1) * P, hsl],
                            in_=y3[:, hsl])
        n2Tp.release()
        qT2p.release()
        n2p.release()
        y2rp.release()

    nc.compile()
    return nc


_CACHE = {}


def _get_nc():
    if "nc" not in _CACHE:
        _CACHE["nc"] = build_nc()
    return _CACHE["nc"]


def _q_indices(h):
    """Interleaved q-tile ownership: core-half h owns global tiles h, h+2..."""
    tiles = np.arange(h, 2 * QTI, 2)
    return (tiles[:, None] * P + np.arange(P)[None, :]).reshape(-1)


def _q8(x):
    return np.asarray(x, np.float32).astype(E4NP)


def _q8f(x):
    return _q8(x).astype(np.float32)


def _pack_dT(m):
    """[D, n] (d-major) -> [128, DTI, n] (partition, k-tile, col)."""
    return np.ascontiguousarray(
        m.reshape(DTI, P, -1).transpose(1, 0, 2))


def _hilo(m):
    hi = _q8(m)
    lo = _q8(np.asarray(m, np.float32) - hi.astype(np.float32))
    return hi, lo


def _prep_shared(inp):
    """Weight/vector arrays shared by all cores (host-side prep)."""
    f = lambda k: np.asarray(inp[k], np.float32)
    sh = {}
    for nm, key in [("wq1", "WQ1"), ("wk1", "WK1"), ("wv1", "WV1"),
                    ("wk2", "WK2"), ("wv2", "WV2")]:
        hi, lo = _hilo(WS * f(key))
        sh[nm] = _pack_dT(hi)
        if nm in ("wq1", "wk1", "wv1"):
            sh[nm + "lo"] = _pack_dT(lo)
    # wq2 with LN1 gamma folded; bias = be1 @ WQ2
    wq2p = WS * (f("g1")[:, None] * f("WQ2"))
    sh["wq2"] = _pack_dT(_q8(wq2p))
    sh["qb2"] = np.ascontiguousarray(
        (f("be1") @ f("WQ2")).reshape(DTI, P).T).astype(np.float32)
    # FFN weights: W1 with LN2 gamma folded, hi+lo interleaved; W2 hi+lo
    w1p = WS * (f("g2")[:, None] * f("W_ff1"))
    w1h, w1l = _hilo(w1p)
    w1h = w1h.reshape(DTI, P, FTI, P).transpose(2, 1, 0, 3)
    w1l = w1l.reshape(DTI, P, FTI, P).transpose(2, 1, 0, 3)
    sh["wf1"] = np.ascontiguousarray(
        np.stack([w1h, w1l], axis=2)).reshape(FTI * P, 2, DTI, P)
    w2h, w2l = _hilo(WS * f("W_ff2"))
    sh["wf2h"] = np.ascontiguousarray(
        w2h.reshape(FTI, P, D).transpose(1, 0, 2))
    sh["wf2l"] = np.ascontiguousarray(
        w2l.reshape(FTI, P, D).transpose(1, 0, 2))
    bh = f("be2") @ f("W_ff1") + f("b_ff1")
    sh["bf1"] = np.ascontiguousarray(bh.reshape(FTI, P).T).astype(np.float32)
    sh["g1"] = f("g1").astype(np.float16)
    sh["be1"] = f("be1").astype(np.float16)
    sh["g2"] = f("g2").astype(np.float16)
    sh["b2r"] = (f("be2") + f("b_ff2")).astype(np.float16)
    sh["g3"] = f("g3")
    sh["be3"] = f("be3")
    return sh


def _mask_blocks(h):
    """[128, 8, 128] fp8: boundary mask for self-attn score tile r=t-8c,
    applied to its q-block u4b=r//2 (the only block where the causal
    frontier can land).  r even: tri (h=0) / ones (h=1); r odd: zeros
    (h=0) / tri (h=1).  Hidden non-boundary blocks are never read."""
    tri = (np.arange(P)[:, None] <= np.arange(P)[None, :]).astype(np.float32)
    blocks = np.empty((DTI, P, P), np.float32)
    for r in range(DTI):
        cmp = 2 * (r // 2) + h - r
        blocks[r] = tri if cmp == 0 else (1.0 if cmp > 0 else 0.0)
    return np.ascontiguousarray(blocks.transpose(1, 0, 2)).astype(E4NP)


def _prep_core(c, y, Z, shared):
    b, h = c // 2, c % 2
    qi = _q_indices(h)
    yb16 = y[b].astype(np.float16)          # [S, D]
    yq16 = yb16[qi]                         # [NQ, D] own queries
    xkvT = yb16.T.astype(np.float32)        # [D, S]
    xqT = yq16.T.astype(np.float32)         # [D, NQ]
    zT = Z[b].astype(np.float16).T.astype(np.float32)
    m = {
        "xq8": _pack_dT(_q8(xqT)),
        "xq0lo": _pack_dT(_q8(xqT[:, 0:P] - _q8f(xqT[:, 0:P]))),
        "xkv8": _pack_dT(_q8(xkvT)),
        "xkvelo": _pack_dT(_q8(xkvT[:, 0:2 * P] - _q8f(xkvT[:, 0:2 * P]))),
        "z8": _pack_dT(_q8(zT)),
        "yres": np.ascontiguousarray(
            yq16.reshape(QTI, P, D).transpose(1, 0, 2)),
        "maskblk": _mask_blocks(h),
    }
    m.update(shared)
    return m


def kernel(**inputs):
    inp = {k: np.asarray(v) for k, v in inputs.items()}
    y = inp["y"].astype(np.float32)
    Z = inp["Z"].astype(np.float32)
    shared = _prep_shared(inp)
    in_maps = [_prep_core(c, y, Z, shared) for c in range(N_CORES)]
    res = run_bass_kernel_spmd(_get_nc(), in_maps, list(range(N_CORES)))
    out = np.zeros((4, 2048, 1024), np.float32)
    for c in range(N_CORES):
        b, h = c // 2, c % 2
        out[b, _q_indices(h)] = res.results[c]["out"]
    return out



# revision 53
# speedup vs baseline: 1.1693x; 1.0715x over previous
"""Trainium2 Bass kernel for a transformer decoder layer (self-attn +
cross-attn + FFN), fp8-e4m3 DoubleRow edition.

Sharding: 8 cores = 4 batches x 2 halves, no collectives. Core h of a batch
owns the interleaved query tiles {h, h+2, ..., h+14} (causal load balance) and
computes the FULL K/V projections for its batch locally (cheaper than the
pair-exchange collective at fp8 speeds).

Numerics: all matmuls run in fp8-e4m3 with DoubleRow perf mode (2 contraction
rows per partition).  Weights are pre-scaled x32 host-side so they sit in
e4m3's normal range; every PSUM drain folds the 1/32 back in.  Three
refinements keep absmax rel err ~3e-3 (gate is 2e-2):
  - FFN: both matmuls use hi+lo fp8 splits of activations AND weights
    (3 DoubleRow matmuls per logical matmul = fp16-level accuracy at 2x
    fp16 speed).
  - Early causal tokens (global positions 0..255, each core's local q-tile
    u=0) see few keys, so fp8 noise doesn't average out: their Q/K/V/E values
    are computed via the same hi+lo corrected path.
  - LayerNorm gammas/betas are folded into the next matmul's weights where
    possible (WQ2, W_ff1) and the residual carriers keep f16 precision.

Causal masking is via per-core precomputed [128 x 512] mask rows (tri/ones/
zeros blocks depending on core half), applied to each self-attn score tile,
so the SPMD program is uniform across cores.

Stage order: A1 (self K/V/Q + early) -> B (self-attn, prefetching A2 inputs)
-> A2 (cross K/V) -> T1 (n1 transpose) -> C0 (Q2) -> C (cross-attn) ->
T2 (n2 transpose hi/lo) -> D (FFN, token-chunked, streamed W_ff1).
"""

from contextlib import ExitStack

import ml_dtypes
import numpy as np

import concourse.bass as bass
import concourse.mybir as mybir
import concourse.tile as tile
from concourse import bacc
from concourse.bass_utils import run_bass_kernel_spmd
from concourse.masks import make_identity

f32 = mybir.dt.float32
f16 = mybir.dt.float16
f8 = mybir.dt.float8e4

P = 128
D = 1024          # d_model
S = 2048          # kv sequence length
NQ = 1024         # query tokens per core
DFF = 4096
DTI = D // P      # 8 d-model partition tiles
KTI = S // P      # 16 kv token tiles
QTI = NQ // P     # 8 query tiles
FTI = DFF // P    # 32 d_ff tiles
ACT = mybir.ActivationFunctionType
ALU = mybir.AluOpType
DR = mybir.MatmulPerfMode.DoubleRow
N_CORES = 8
WS = 32.0         # host-side weight pre-scale
IWS = 1.0 / WS
SCALE = 1.0 / 32.0  # 1/sqrt(D) softmax scale
MM2_LO = False    # include (hh, wf2l) correction term in FFN mm2
E4NP = ml_dtypes.float8_e4m3


def build_nc():
    nc = bacc.Bacc("TRN2", target_bir_lowering=False, debug=False,
                   num_devices=N_CORES)

    def dp(name, shape, dt, out=False):
        return nc.declare_dram_parameter(name, shape, dt, isOutput=out)

    xq8_d = dp("xq8", [P, DTI, NQ], f8)
    xq0lo_d = dp("xq0lo", [P, DTI, P], f8)
    xkv8_d = dp("xkv8", [P, DTI, S], f8)
    xkvelo_d = dp("xkvelo", [P, DTI, 2 * P], f8)
    z8_d = dp("z8", [P, DTI, S], f8)
    yres_d = dp("yres", [P, QTI, D], f16)
    w_d = {n: dp(n, [P, DTI, D], f8)
           for n in ["wq1", "wk1", "wv1", "wq2", "wk2", "wv2",
                     "wq1lo", "wk1lo", "wv1lo"]}
    wf1_d = dp("wf1", [FTI * P, 2, DTI, P], f8)   # hi/lo interleaved
    wf2h_d = dp("wf2h", [P, FTI, D], f8)
    wf2l_d = dp("wf2l", [P, FTI, D], f8)
    bf1_d = dp("bf1", [P, FTI], f32)
    qb2_d = dp("qb2", [P, DTI], f32)
    mask_d = dp("maskblk", [P, DTI, P], f8)
    v16_d = {n: dp(n, [D], f16) for n in ["g1", "be1", "g2", "b2r"]}
    v32_d = {n: dp(n, [D], f32) for n in ["g3", "be3"]}
    out_d = dp("out", [NQ, D], f32, out=True)

    def bc(ap):  # broadcast a [n] dram vector across 128 partitions
        return bass.AP(tensor=ap.tensor, offset=ap.offset,
                       ap=[[0, P]] + [list(x) for x in ap.ap])

    with tile.TileContext(nc) as tc, ExitStack() as top:
        const = top.enter_context(tc.tile_pool(name="const", bufs=1))
        # one explicit act-table load (natural_log_exp_and_others: exp, ln,
        # copy, identity, relu, square) so every activation in the kernel is
        # servable without another table swap, regardless of how the
        # scheduler interleaves exp/rstd chains
        nc.scalar.add_instruction(mybir.InstLoadActFuncSet(
            name=f"I-{nc.next_id()}", act_func_set_id=6))
        ident = const.tile([P, P], f16, name="ident", tag="ident")
        make_identity(nc, ident)
        masks = const.tile([P, DTI, P], f8, name="masks", tag="masks")
        ones8t = const.tile([P, 2, 16], f8, name="ones8", tag="ones8")
        nc.vector.memset(ones8t, 1.0)
        ones8 = ones8t[:, :, 0:1]  # outer step 16B: dual-fp8 ldweights rule
        eps = const.tile([P, 1], f32, name="eps", tag="eps")
        nc.vector.memset(eps, 1e-5)
        bf1sb = const.tile([P, FTI], f32, name="bf1sb", tag="bf1sb")
        qb2sb = const.tile([P, DTI], f32, name="qb2sb", tag="qb2sb")

        def vload(name, dt, dram):
            return const.tile([P, D], dt, name=f"{name}b", tag=f"{name}b")

        g1b = vload("g1", f16, v16_d)
        b1b = vload("be1", f16, v16_d)
        g2b = vload("g2", f16, v16_d)
        b2rb = vload("b2r", f16, v16_d)

        def load_consts():
            # deferred off-critical-path constant loads
            nc.sync.dma_start(out=masks, in_=mask_d.ap())
            nc.sync.dma_start(out=bf1sb, in_=bf1_d.ap())
            nc.sync.dma_start(out=qb2sb, in_=qb2_d.ap())
            for t, nm, dd in [(g1b, "g1", v16_d), (b1b, "be1", v16_d),
                              (g2b, "g2", v16_d), (b2rb, "b2r", v16_d)]:
                nc.sync.dma_start(out=t, in_=bc(dd[nm].ap()))

        # ---- persistent pools; LIFO per side ----
        # left: kv2p (bottom; dies after cross), y1, n1, [zpB/wpB], [n1T],
        #       [n2T]
        # right: yres, kvp, earlyp | y2r, n2, qT2p
        kv2p = tc.alloc_tile_pool(name="kv2p", bufs=1)
        kT2 = kv2p.tile([P, DTI, S], f8, name="kT2", tag="kT2")
        v2 = kv2p.tile([P, KTI, D], f8, name="v2", tag="v2")
        y1p = tc.alloc_tile_pool(name="y1p", bufs=1)
        y1 = y1p.tile([P, QTI, D], f16, name="y1", tag="y1")
        n1p = tc.alloc_tile_pool(name="n1p", bufs=1)
        n1 = n1p.tile([P, QTI, D], f16, name="n1", tag="n1")

        yresp = tc.alloc_tile_pool(name="yresp", bufs=1, side="right")
        yres = yresp.tile([P, QTI, D], f16, name="yres", tag="yres")
        kvp = tc.alloc_tile_pool(name="kvp", bufs=1, side="right")
        kT = kvp.tile([P, DTI, S], f8, name="kT", tag="kT")
        v = kvp.tile([P, KTI, D], f8, name="v", tag="v")
        qT = kvp.tile([P, DTI, NQ], f8, name="qT", tag="qT")
        earlyp = tc.alloc_tile_pool(name="earlyp", bufs=1, side="right")
        keT = [earlyp.tile([P, DTI, 2 * P], f8, name=f"keT{x}", tag=f"keT{x}")
               for x in range(2)]  # hi, lo
        qeT = [earlyp.tile([P, DTI, P], f8, name=f"qeT{x}", tag=f"qeT{x}")
               for x in range(2)]
        ve = [earlyp.tile([P, 2, D], f8, name=f"ve{x}", tag=f"ve{x}")
              for x in range(2)]

        def dr_acc(ps, terms, rhs_sl, lhs_sl):
            """Accumulate sum of DoubleRow products into psum region ps.
            terms: list of (lhsT_tile, rhs_tile); contraction over DTI//2
            k-tile pairs per term. rhs_sl/lhs_sl: fn(tile, g) -> AP."""
            n = len(terms) * (DTI // 2)
            i = 0
            for lt, rt in terms:
                for g in range(DTI // 2):
                    nc.tensor.matmul(ps, lhsT=lhs_sl(lt, g),
                                     rhs=rhs_sl(rt, g), perf_mode=DR,
                                     start=(i == 0), stop=(i == n - 1))
                    i += 1

        def split3(pool, ps, scale, bias, func, hi_out, lo_out, eng, n):
            """3-op hi/lo drain: t16 = func(scale*ps + bias); hi = q8(t16);
            lo = q8(t16 - hi)."""
            t16 = pool.tile([P, n], f16, name="t16", tag="t16", bufs=3)
            nc.scalar.activation(out=t16, in_=ps, func=func, bias=bias,
                                 scale=scale)
            if eng == 0:
                nc.vector.tensor_copy(out=hi_out, in_=t16)
                nc.gpsimd.tensor_sub(lo_out, t16, hi_out)
            else:
                nc.gpsimd.tensor_copy(out=hi_out, in_=t16)
                nc.vector.tensor_sub(lo_out, t16, hi_out)

        # ==================== stage A1: self-attn projections =============
        with ExitStack() as stA:
            wpA = stA.enter_context(tc.tile_pool(name="wpA", bufs=1))
            xpA = stA.enter_context(tc.tile_pool(name="xpA", bufs=1))
            psA = stA.enter_context(tc.tile_pool(name="psA", bufs=3,
                                                 space="PSUM"))
            psE = stA.enter_context(tc.tile_pool(name="psE", bufs=2,
                                                 space="PSUM"))
            drp = stA.enter_context(tc.tile_pool(name="drpA", bufs=1))

            def wload(tag, name):
                t = wpA.tile([P, DTI, D], f8, name=name, tag=tag)
                nc.sync.dma_start(out=t, in_=w_d[name].ap())
                return t

            # first loads chunked so K1 (j=0, th=0) can start after ~1/4 of
            # the wk1+xkv8 bytes have landed
            wk1 = wpA.tile([P, DTI, D], f8, name="wk1", tag="wA0")
            xkv8 = xpA.tile([P, DTI, S], f8, name="xkv8", tag="xkv8")
            nc.sync.dma_start(out=wk1[:, :, 0:512],
                              in_=w_d["wk1"].ap()[:, :, 0:512])
            nc.sync.dma_start(out=xkv8[:, :, 0:1024],
                              in_=xkv8_d.ap()[:, :, 0:1024])
            nc.sync.dma_start(out=wk1[:, :, 512:1024],
                              in_=w_d["wk1"].ap()[:, :, 512:1024])
            nc.sync.dma_start(out=xkv8[:, :, 1024:2048],
                              in_=xkv8_d.ap()[:, :, 1024:2048])
            wv1 = wload("wA1", "wv1")
            wk1lo = wload("wA2", "wk1lo")
            wv1lo = wload("wA3", "wv1lo")
            xkvelo = xpA.tile([P, DTI, 2 * P], f8, name="xkvelo",
                              tag="xkvelo")
            nc.sync.dma_start(out=xkvelo, in_=xkvelo_d.ap())
            xq8 = xpA.tile([P, DTI, NQ], f8, name="xq8", tag="xq8")
            nc.sync.dma_start(out=xq8, in_=xq8_d.ap())
            xq0lo = xpA.tile([P, DTI, P], f8, name="xq0lo", tag="xq0lo")
            nc.sync.dma_start(out=xq0lo, in_=xq0lo_d.ap())
            nc.sync.dma_start(out=yres, in_=yres_d.ap())
            load_consts()

            # K1: kT[:, j, :] = (wk1.T @ xkv)/32, d_out on partitions
            for j in range(DTI):
                for th in range(2):
                    ps = psA.tile([P, 1024], f32, name="psp", tag="psp")
                    for sub in range(2):
                        tsl = slice(th * 1024 + sub * 512,
                                    th * 1024 + sub * 512 + 512)
                        dr_acc(ps[:, sub * 512:sub * 512 + 512],
                               [(wk1, xkv8)],
                               lambda t, g, tsl=tsl: t[:, 2 * g:2 * g + 2, tsl],
                               lambda t, g, j=j: t[:, 2 * g:2 * g + 2,
                                                   j * P:(j + 1) * P])
                    osl = kT[:, j, th * 1024:(th + 1) * 1024]
                    if (j + th) % 2 == 0:
                        nc.scalar.activation(out=osl, in_=ps, func=ACT.Copy,
                                             scale=IWS)
                    else:
                        nc.vector.tensor_scalar_mul(osl, ps, IWS)

            # early K (tokens 0:256), hi+lo corrected
            for j in range(DTI):
                ps = psE.tile([P, 512], f32, name="pse", tag="pse")
                dr_acc(ps[:, 0:256],
                       [(wk1, xkv8), (wk1, xkvelo), (wk1lo, xkv8)],
                       lambda t, g: (t[:, 2 * g:2 * g + 2, 0:256]
                                     if t is xkv8 else
                                     t[:, 2 * g:2 * g + 2, :]),
                       lambda t, g, j=j: t[:, 2 * g:2 * g + 2,
                                           j * P:(j + 1) * P])
                split3(drp, ps[:, 0:256], IWS, 0.0, ACT.Copy,
                       keT[0][:, j, :], keT[1][:, j, :], j % 2, 256)

            # V1: v[:, t, :] = (xkv.T @ wv1)/32, tokens on partitions
            for t in range(KTI):
                ps = psA.tile([P, 1024], f32, name="psp", tag="psp")
                for half in range(2):
                    dr_acc(ps[:, half * 512:half * 512 + 512],
                           [(xkv8, wv1)],
                           lambda tt, g, half=half: tt[:, 2 * g:2 * g + 2,
                                                       half * 512:half * 512 + 512],
                           lambda tt, g, t=t: tt[:, 2 * g:2 * g + 2,
                                                 t * P:(t + 1) * P])
                osl = v[:, t, :]
                if t % 2 == 0:
                    nc.scalar.activation(out=osl, in_=ps, func=ACT.Copy,
                                         scale=IWS)
                else:
                    nc.vector.tensor_scalar_mul(osl, ps, IWS)

            # early V (k-tiles 0..1), hi+lo corrected
            for t in range(2):
                for half in range(2):
                    ps = psE.tile([P, 512], f32, name="pse", tag="pse")
                    hsl = slice(half * 512, half * 512 + 512)
                    dr_acc(ps,
                           [(xkv8, wv1), (xkvelo, wv1), (xkv8, wv1lo)],
                           lambda tt, g, hsl=hsl: tt[:, 2 * g:2 * g + 2, hsl],
                           lambda tt, g, t=t: tt[:, 2 * g:2 * g + 2,
                                                 t * P:(t + 1) * P])
                    split3(drp, ps, IWS, 0.0, ACT.Copy,
                           ve[0][:, t, hsl], ve[1][:, t, hsl],
                           (t + half) % 2, 512)

            # Q1 (weights reuse the K1 buffers)
            wq1 = wload("wA0", "wq1")
            wq1lo = wload("wA2", "wq1lo")
            for j in range(DTI):
                ps = psA.tile([P, 1024], f32, name="psp", tag="psp")
                for sub in range(2):
                    dr_acc(ps[:, sub * 512:sub * 512 + 512],
                           [(wq1, xq8)],
                           lambda t, g, sub=sub: t[:, 2 * g:2 * g + 2,
                                                   sub * 512:sub * 512 + 512],
                           lambda t, g, j=j: t[:, 2 * g:2 * g + 2,
                                               j * P:(j + 1) * P])
                osl = qT[:, j, :]
                if j % 2 == 0:
                    nc.scalar.activation(out=osl, in_=ps, func=ACT.Copy,
                                         scale=IWS)
                else:
                    nc.vector.tensor_scalar_mul(osl, ps, IWS)
            # early Q (own u=0 tile)
            for j in range(DTI):
                ps = psE.tile([P, 512], f32, name="pse", tag="pse")
                dr_acc(ps[:, 0:P],
                       [(wq1, xq8), (wq1, xq0lo), (wq1lo, xq8)],
                       lambda t, g: (t[:, 2 * g:2 * g + 2, 0:P]
                                     if t is xq8 else
                                     t[:, 2 * g:2 * g + 2, :]),
                       lambda t, g, j=j: t[:, 2 * g:2 * g + 2,
                                           j * P:(j + 1) * P])
                split3(drp, ps[:, 0:P], IWS, 0.0, ACT.Copy,
                       qeT[0][:, j, :], qeT[1][:, j, :], j % 2, P)

        # ==================== attention helper ============================
        def pump(gen, n=1):
            """Advance a filler emission generator n steps (no-op if None)."""
            for _ in range(n):
                if gen is None or next(gen, "END") == "END":
                    return

        def att_pools(stk, tagp, score_bufs, out_bufs=2):
            pss = stk.enter_context(tc.tile_pool(name=f"{tagp}pss",
                                                 bufs=score_bufs,
                                                 space="PSUM"))
            pso = stk.enter_context(tc.tile_pool(name=f"{tagp}pso",
                                                 bufs=out_bufs,
                                                 space="PSUM"))
            ep = stk.enter_context(tc.tile_pool(name=f"{tagp}ep", bufs=1))
            lnp = stk.enter_context(tc.tile_pool(name=f"{tagp}lnp", bufs=4))
            return tagp, pss, pso, ep, lnp

        def attention_half(ap_, c, qTt, kTt, vt, resid_sl, gb, bb, yout,
                           nout, masked, filler=None, filler_late="same"):
            tagp, pss, pso, ep, lnp = ap_
            if filler_late == "same":
                filler_late = filler
            nvis = 8 * (c + 1) if masked else KTI
            if True:
                e = ep.tile([P, nvis, 512], f8, name=f"e{c}", tag=f"e{c}")
                ee = None
                if masked and c == 0:
                    # early corrected scores/E for q-tile u=0, k-tiles 0..1
                    # (emitted first so its long drain chain overlaps the
                    # main score tiles)
                    ee = lnp.tile([P, 2, 2, P], f8, name="ee", tag="ee",
                                  bufs=1)
                    for t in range(2):
                        ps = pss.tile([P, 1024], f32, name="ps_s", tag="ps_s")
                        dr_acc(ps[:, 0:P],
                               [(keT[0], qeT[0]), (keT[0], qeT[1]),
                                (keT[1], qeT[0])],
                               lambda tt, g: tt[:, 2 * g:2 * g + 2, :],
                               lambda tt, g, t=t: tt[:, 2 * g:2 * g + 2,
                                                     t * P:(t + 1) * P])
                        tm = lnp.tile([P, P], f16, name="etm", tag="etm",
                                      bufs=2)
                        nc.scalar.activation(out=tm, in_=ps[:, 0:P],
                                             func=ACT.Exp, scale=SCALE)
                        nc.vector.tensor_mul(tm, tm, masks[:, t, :])
                        nc.vector.tensor_copy(out=ee[:, 0, t, :], in_=tm)
                        nc.gpsimd.tensor_sub(ee[:, 1, t, :], tm,
                                             ee[:, 0, t, :])
                # scores + exp, two k-tiles per psum tile / exp instruction;
                # causal mask only touches the boundary q-block of each tile
                # (hidden non-boundary blocks are never read downstream)
                for dual in range(nvis // 2):
                    ps = pss.tile([P, 1024], f32, name="ps_s", tag="ps_s")
                    for k in range(2):
                        t = 2 * dual + k
                        dr_acc(ps[:, k * 512:k * 512 + 512], [(kTt, qTt)],
                               lambda tt, g, c=c: tt[:, 2 * g:2 * g + 2,
                                                     c * 512:c * 512 + 512],
                               lambda tt, g, t=t: tt[:, 2 * g:2 * g + 2,
                                                     t * P:(t + 1) * P])
                    nc.scalar.activation(out=e[:, 2 * dual:2 * dual + 2, :],
                                         in_=ps, func=ACT.Exp, scale=SCALE)
                    if masked:
                        for k in range(2):
                            t = 2 * dual + k
                            if t < 8 * c:
                                continue
                            r = t - 8 * c
                            u4b = r // 2
                            esl = e[:, t, u4b * P:(u4b + 1) * P]
                            if r % 2 == 0:
                                nc.vector.tensor_mul(esl, esl, masks[:, r, :])
                            else:
                                nc.gpsimd.tensor_mul(esl, esl, masks[:, r, :])
                    pump(filler)
                # denominators: E^T @ ones -> [128 q, 1] per u4 column of a
                # psum tile (q on partitions; no DRAM transpose round-trip)
                pd = pso.tile([P, 1024], f32, name="pd", tag="po")
                for u4 in range(4):
                    if ee is not None and u4 == 0:
                        for hl in range(2):
                            nc.tensor.matmul(pd[:, 0:1],
                                             lhsT=ee[:, hl, :, :],
                                             rhs=ones8, perf_mode=DR,
                                             start=(hl == 0), stop=(hl == 1))
                        continue
                    np_ = (4 * c + u4 + 1) if masked else 8
                    for i in range(np_):
                        nc.tensor.matmul(
                            pd[:, u4:u4 + 1],
                            lhsT=e[:, 2 * i:2 * i + 2, u4 * P:(u4 + 1) * P],
                            rhs=ones8, perf_mode=DR,
                            start=(i == 0), stop=(i == np_ - 1))
                recT = lnp.tile([P, 4], f32, name="recT", tag="recT")
                nc.vector.reciprocal(recT, pd[:, 0:4])
                pump(filler_late)
                u4order = [1, 2, 3, 0] if ee is not None else range(4)
                for u4 in u4order:
                    u = c * 4 + u4
                    po = pso.tile([P, 1024], f32, name="po", tag="po")
                    if ee is not None and u == 0:
                        for half in range(2):
                            hsl = slice(half * 512, half * 512 + 512)
                            for ti, (el, vl) in enumerate(
                                    [(0, 0), (1, 0), (0, 1)]):
                                nc.tensor.matmul(
                                    po[:, hsl], lhsT=ee[:, el, :, :],
                                    rhs=ve[vl][:, :, hsl], perf_mode=DR,
                                    start=(ti == 0), stop=(ti == 2))
                    else:
                        np_ = (u + 1) if masked else 8
                        for half in range(2):
                            hsl = slice(half * 512, half * 512 + 512)
                            for i in range(np_):
                                nc.tensor.matmul(
                                    po[:, hsl],
                                    lhsT=e[:, 2 * i:2 * i + 2,
                                           u4 * P:(u4 + 1) * P],
                                    rhs=vt[:, 2 * i:2 * i + 2, hsl],
                                    perf_mode=DR, start=(i == 0),
                                    stop=(i == np_ - 1))
                    xr = lnp.tile([P, D], f16, name="xr", tag="xr", bufs=2)
                    nc.scalar.activation(out=xr, in_=po, func=ACT.Copy,
                                         scale=recT[:, u4:u4 + 1])
                    nc.vector.tensor_add(xr, xr, resid_sl(u))
                    pump(filler_late)
                    # LN core + affine
                    stats = lnp.tile([P, 2, 6], f32, name="stats",
                                     tag="stats")
                    nc.vector.bn_stats(out=stats[:, 0, :], in_=xr[:, 0:512])
                    nc.vector.bn_stats(out=stats[:, 1, :], in_=xr[:, 512:])
                    mv = lnp.tile([P, 2], f32, name="mv", tag="mv")
                    nc.vector.bn_aggr(out=mv, in_=stats)
                    # rstd = exp(-0.5*ln(var+eps)) -- stays in act table 6
                    lnv = lnp.tile([P, 1], f32, name="lnv", tag="lnv")
                    nc.scalar.activation(out=lnv, in_=mv[:, 1:2],
                                         func=ACT.Ln, bias=eps)
                    rstd = lnp.tile([P, 1], f32, name="rstd", tag="rstd")
                    nc.scalar.activation(out=rstd, in_=lnv, func=ACT.Exp,
                                         scale=-0.5)
                    nsl = nout[:, u, :]
                    nc.vector.tensor_scalar(out=nsl, in0=xr,
                                            scalar1=mv[:, 0:1], scalar2=rstd,
                                            op0=ALU.subtract, op1=ALU.mult)
                    t1 = lnp.tile([P, D], f16, name="lt0", tag="lt0", bufs=2)
                    nc.vector.tensor_mul(t1, nsl, gb)
                    nc.gpsimd.tensor_add(yout[:, u, :], t1, bb)
                    pump(filler_late, 2)

        # ==================== stage B: self-attention + LN1 ===============
        # Cross-attn K2/V2 projections are emitted as FILLER inside the
        # self-attention instruction stream: the PE chews them while the
        # Act/DVE engines work through exp + LayerNorm chains.
        with ExitStack() as stB:
            # prefetch stage-A2 inputs while attention runs
            zpB = stB.enter_context(tc.tile_pool(name="zpB", bufs=1))
            wk2 = zpB.tile([P, DTI, D], f8, name="wk2", tag="wk2")
            nc.sync.dma_start(out=wk2, in_=w_d["wk2"].ap())
            z8 = zpB.tile([P, DTI, S], f8, name="z8", tag="z8")
            nc.sync.dma_start(out=z8[:, :, 0:1024],
                              in_=z8_d.ap()[:, :, 0:1024])
            nc.sync.dma_start(out=z8[:, :, 1024:2048],
                              in_=z8_d.ap()[:, :, 1024:2048])
            wv2 = zpB.tile([P, DTI, D], f8, name="wv2", tag="wv2")
            nc.sync.dma_start(out=wv2, in_=w_d["wv2"].ap())

            def emit_k2(pool, j, th, eng):
                ps = pool.tile([P, 1024], f32, name="psp2", tag="psp2")
                for sub in range(2):
                    tsl = slice(th * 1024 + sub * 512,
                                th * 1024 + sub * 512 + 512)
                    dr_acc(ps[:, sub * 512:sub * 512 + 512], [(wk2, z8)],
                           lambda t, g, tsl=tsl: t[:, 2 * g:2 * g + 2, tsl],
                           lambda t, g, j=j: t[:, 2 * g:2 * g + 2,
                                               j * P:(j + 1) * P])
                osl = kT2[:, j, th * 1024:(th + 1) * 1024]
                if eng == 0:
                    nc.vector.tensor_scalar_mul(osl, ps, IWS)
                else:
                    nc.scalar.activation(out=osl, in_=ps, func=ACT.Copy,
                                         scale=IWS)

            def emit_v2(pool, t, eng):
                ps = pool.tile([P, 1024], f32, name="psp2", tag="psp2")
                for half in range(2):
                    dr_acc(ps[:, half * 512:half * 512 + 512], [(z8, wv2)],
                           lambda tt, g, half=half: tt[:, 2 * g:2 * g + 2,
                                                       half * 512:half * 512 + 512],
                           lambda tt, g, t=t: tt[:, 2 * g:2 * g + 2,
                                                 t * P:(t + 1) * P])
                osl = v2[:, t, :]
                if eng == 0:
                    nc.vector.tensor_scalar_mul(osl, ps, IWS)
                else:
                    nc.scalar.activation(out=osl, in_=ps, func=ACT.Copy,
                                         scale=IWS)

            A2G = ([("k", j, 0) for j in range(DTI)]
                   + [("k", j, 1) for j in range(DTI)]
                   + [("v", t, 0) for t in range(KTI)])

            def gen_a2(pool, groups):
                for gi, g in enumerate(groups):
                    eng = gi % 2
                    if g[0] == "k":
                        emit_k2(pool, g[1], g[2], eng)
                    else:
                        emit_v2(pool, g[1], eng)
                    yield

            NFILL = 20
            with ExitStack() as stB2:
                ap_ = att_pools(stB2, "sa_", score_bufs=1)
                psA2 = stB2.enter_context(tc.tile_pool(name="psA2", bufs=1,
                                                       space="PSUM"))
                attention_half(ap_, 0, qT, kT, v, lambda u: yres[:, u, :],
                               g1b, b1b, y1, n1, masked=True)
                earlyp.release()
                a2 = gen_a2(psA2, A2G[:NFILL])
                attention_half(ap_, 1, qT, kT, v, lambda u: yres[:, u, :],
                               g1b, b1b, y1, n1, masked=True, filler=a2)
                pump(a2, 99)
            # leftover A2 groups run dense with triple-buffered psum
            with ExitStack() as stA2t:
                psA2t = stA2t.enter_context(tc.tile_pool(name="psA2t",
                                                         bufs=3,
                                                         space="PSUM"))
                pump(gen_a2(psA2t, A2G[NFILL:]), 99)
            kvp.release()
            yresp.release()

            y2rp = tc.alloc_tile_pool(name="y2rp", bufs=1, side="right")
            y2r = y2rp.tile([P, QTI, D], f16, name="y2r", tag="y2r")
            n2p = tc.alloc_tile_pool(name="n2p", bufs=1, side="right")
            n2 = n2p.tile([P, QTI, D], f16, name="n2", tag="n2")

        # ======= stages T1/C0/cross/T2: pipelined with cross-attention ====
        # T1(c=0)+Q2(sub 0) run dense before cross; T1(c=1)+Q2(sub 1) fill
        # cross c0's exp/LN bubbles; T2(c=0) fills cross c1's; T2(c=1) is
        # pumped inside the FFN mm1 loop.
        n1Tp = tc.alloc_tile_pool(name="n1Tp", bufs=1)
        n1T = n1Tp.tile([P, DTI, NQ], f8, name="n1T", tag="n1T")
        wpC = tc.alloc_tile_pool(name="wpC", bufs=1)
        wq2 = wpC.tile([P, DTI, D], f8, name="wq2", tag="wq2")
        nc.sync.dma_start(out=wq2, in_=w_d["wq2"].ap())
        qT2p = tc.alloc_tile_pool(name="qT2p", bufs=1, side="right")
        qT2 = qT2p.tile([P, DTI, NQ], f8, name="qT2", tag="qT2")
        n2Tp = tc.alloc_tile_pool(name="n2Tp", bufs=1, side="right")
        n2T = [n2Tp.tile([P, DTI, NQ], f8, name=f"n2T{x}", tag=f"n2T{x}")
               for x in range(2)]

        def gen_t1(pst, c2, engs):
            for i in range(DTI):
                pt = pst.tile([P, 512], f16, name="pt", tag="pt")
                for u4 in range(4):
                    nc.tensor.transpose(
                        pt[:, u4 * P:(u4 + 1) * P],
                        in_=n1[:, c2 * 4 + u4, i * P:(i + 1) * P],
                        identity=ident)
                osl = n1T[:, i, c2 * 512:c2 * 512 + 512]
                if engs[i % len(engs)] == "a":
                    nc.scalar.activation(out=osl, in_=pt, func=ACT.Copy,
                                         scale=1.0)
                else:
                    nc.vector.tensor_copy(out=osl, in_=pt)
                yield

        def gen_q2(psC, sub, engs):
            ssl = slice(sub * 512, sub * 512 + 512)
            for j in range(DTI):
                ps = psC.tile([P, 512], f32, name="psq2", tag="psq2")
                dr_acc(ps, [(wq2, n1T)],
                       lambda t, g, ssl=ssl: t[:, 2 * g:2 * g + 2, ssl],
                       lambda t, g, j=j: t[:, 2 * g:2 * g + 2,
                                           j * P:(j + 1) * P])
                osl = qT2[:, j, ssl]
                if engs[j % len(engs)] == "a":
                    nc.scalar.activation(out=osl, in_=ps,
                                         func=ACT.Identity,
                                         bias=qb2sb[:, j:j + 1], scale=IWS)
                else:
                    nc.vector.tensor_scalar(out=osl, in0=ps, scalar1=IWS,
                                            scalar2=qb2sb[:, j:j + 1],
                                            op0=ALU.mult, op1=ALU.add)
                yield

        def gen_t2(pst, c2, engs):
            for i in range(DTI):
                pt = pst.tile([P, 512], f16, name="pt", tag="pt")
                for u4 in range(4):
                    nc.tensor.transpose(
                        pt[:, u4 * P:(u4 + 1) * P],
                        in_=n2[:, c2 * 4 + u4, i * P:(i + 1) * P],
                        identity=ident)
                csl = slice(c2 * 512, c2 * 512 + 512)
                if engs[i % len(engs)] == "a":
                    nc.scalar.activation(out=n2T[0][:, i, csl], in_=pt,
                                         func=ACT.Copy, scale=1.0)
                else:
                    nc.vector.tensor_copy(out=n2T[0][:, i, csl], in_=pt)
                nc.vector.tensor_sub(n2T[1][:, i, csl], pt,
                                     n2T[0][:, i, csl])
                yield

        def chain(*gens):
            for g in gens:
                yield from g

        with ExitStack() as stC0:
            pst = stC0.enter_context(tc.tile_pool(name="pstC", bufs=2,
                                                  space="PSUM"))
            psC = stC0.enter_context(tc.tile_pool(name="psC", bufs=2,
                                                  space="PSUM"))
            pump(gen_t1(pst, 0, "av"), 99)
            pump(gen_q2(psC, 0, "av"), 99)
            ap_ = att_pools(stC0, "ca_", score_bufs=1, out_bufs=1)
            fill0 = chain(gen_t1(pst, 1, "va"), gen_q2(psC, 1, "va"))
            attention_half(ap_, 0, qT2, kT2, v2, lambda u: y1[:, u, :],
                           g2b, b2rb, y2r, n2, masked=False, filler=fill0)
            pump(fill0, 99)
        with ExitStack() as stC1:
            pst = stC1.enter_context(tc.tile_pool(name="pstC1", bufs=2,
                                                  space="PSUM"))
            ap_ = att_pools(stC1, "cb_", score_bufs=1, out_bufs=2)
            fill1 = gen_t2(pst, 0, "va")
            attention_half(ap_, 1, qT2, kT2, v2, lambda u: y1[:, u, :],
                           g2b, b2rb, y2r, n2, masked=False, filler=fill1)
            pump(fill1, 99)
        wpC.release()
        n1Tp.release()
        n1p.release()
        y1p.release()
        kv2p.release()

        # ==================== stage D: FFN + LN3 + output =================
        with ExitStack() as stD:
            wf2p = stD.enter_context(tc.tile_pool(name="wf2p", bufs=1))
            wf1p = stD.enter_context(tc.tile_pool(name="wf1p", bufs=3))
            hp = stD.enter_context(tc.tile_pool(name="hp", bufs=1))
            psH = stD.enter_context(tc.tile_pool(name="psH", bufs=2,
                                                 space="PSUM"))
            psF = stD.enter_context(tc.tile_pool(name="psF", bufs=2,
                                                 space="PSUM"))
            pstD = stD.enter_context(tc.tile_pool(name="pstD", bufs=2,
                                                  space="PSUM"))
            drp = stD.enter_context(tc.tile_pool(name="drpD", bufs=1))
            lnp = stD.enter_context(tc.tile_pool(name="lnpD", bufs=4))
            outp = stD.enter_context(tc.tile_pool(name="outp", bufs=2))
            t2g1 = gen_t2(pstD, 1, "avv")
            # wf2/g3/b3 loads are chunked and interleaved between the
            # streamed w1t loads so they don't head-block the first FFN
            # matmuls on the DMA queue
            wf2h = wf2p.tile([P, FTI, D], f8, name="wf2h", tag="wf2h")
            wf2l = (wf2p.tile([P, FTI, D], f8, name="wf2l", tag="wf2l")
                    if MM2_LO else None)
            g3b = wf2p.tile([P, D], f32, name="g3b", tag="g3b")
            b3b = wf2p.tile([P, D], f32, name="b3b", tag="b3b")
            for c in range(2):
                csl = slice(c * 512, c * 512 + 512)
                hh = hp.tile([P, FTI, 512], f8, name="hh", tag="hh")
                hl = hp.tile([P, FTI, 512], f8, name="hl", tag="hl")
                for s in range(FTI):
                    w1t = wf1p.tile([P, 2, DTI, P], f8, name="w1t",
                                    tag="w1t")
                    nc.sync.dma_start(out=w1t,
                                      in_=wf1_d.ap()[s * P:(s + 1) * P])
                    if c == 0:
                        if s % 2 == 0 and s // 2 < 8:
                            ch = s // 2
                            nc.sync.dma_start(
                                out=wf2h[:, ch * 4:(ch + 1) * 4, :],
                                in_=wf2h_d.ap()[:, ch * 4:(ch + 1) * 4, :])
                        elif s == 1:
                            nc.sync.dma_start(out=g3b,
                                              in_=bc(v32_d["g3"].ap()))
                        elif s == 3:
                            nc.sync.dma_start(out=b3b,
                                              in_=bc(v32_d["be3"].ap()))
                        elif (MM2_LO and s % 2 == 1 and 5 <= s <= 19):
                            ch = (s - 5) // 2
                            nc.sync.dma_start(
                                out=wf2l[:, ch * 4:(ch + 1) * 4, :],
                                in_=wf2l_d.ap()[:, ch * 4:(ch + 1) * 4, :])
                    ps = psH.tile([P, 512], f32, name="ph", tag="ph")
                    i = 0
                    for wi, xi in [(0, 0), (0, 1), (1, 0)]:
                        for g in range(DTI // 2):
                            nc.tensor.matmul(
                                ps,
                                lhsT=w1t[:, wi, 2 * g:2 * g + 2, :],
                                rhs=n2T[xi][:, 2 * g:2 * g + 2, csl],
                                perf_mode=DR, start=(i == 0),
                                stop=(i == 3 * DTI // 2 - 1))
                            i += 1
                    split3(drp, ps, IWS, bf1sb[:, s:s + 1], ACT.Relu,
                           hh[:, s, :], hl[:, s, :], s % 2, 512)
                    if c == 0 and s % 2 == 1:
                        pump(t2g1)
                if c == 0:
                    pump(t2g1, 99)
                for u4 in range(4):
                    u = c * 4 + u4
                    pf = psF.tile([P, 1024], f32, name="pf", tag="pf")
                    usl = slice(u4 * P, (u4 + 1) * P)
                    # drain each d-half as soon as its matmuls finish so the
                    # final u's exposed tail is ~half an LN chain
                    xr = lnp.tile([P, D], f16, name="xr3", tag="xr3",
                                  bufs=2)
                    stats = lnp.tile([P, 2, 6], f32, name="st3", tag="st3")
                    for half in range(2):
                        hsl = slice(half * 512, half * 512 + 512)
                        combos = [(hh, wf2h), (hl, wf2h)]
                        if MM2_LO:
                            combos.append((hh, wf2l))
                        nmm = len(combos) * (FTI // 2)
                        i = 0
                        for ha, wb in combos:
                            for sp in range(FTI // 2):
                                nc.tensor.matmul(
                                    pf[:, hsl],
                                    lhsT=ha[:, 2 * sp:2 * sp + 2, usl],
                                    rhs=wb[:, 2 * sp:2 * sp + 2, hsl],
                                    perf_mode=DR, start=(i == 0),
                                    stop=(i == nmm - 1))
                                i += 1
                        if half == 0:
                            nc.scalar.activation(out=xr[:, hsl],
                                                 in_=pf[:, hsl],
                                                 func=ACT.Copy, scale=IWS)
                            nc.gpsimd.tensor_add(xr[:, hsl], xr[:, hsl],
                                                 y2r[:, u, hsl])
                        else:
                            nc.vector.tensor_scalar_mul(xr[:, hsl],
                                                        pf[:, hsl], IWS)
                            nc.vector.tensor_add(xr[:, hsl], xr[:, hsl],
                                                 y2r[:, u, hsl])
                        nc.vector.bn_stats(out=stats[:, half, :],
                                           in_=xr[:, hsl])
                    mv = lnp.tile([P, 2], f32, name="mv3", tag="mv3")
                    nc.vector.bn_aggr(out=mv, in_=stats)
                    lnv = lnp.tile([P, 1], f32, name="lnv3", tag="lnv3")
                    nc.scalar.activation(out=lnv, in_=mv[:, 1:2],
                                         func=ACT.Ln, bias=eps)
                    rstd = lnp.tile([P, 1], f32, name="rstd3", tag="rstd3")
                    nc.scalar.activation(out=rstd, in_=lnv, func=ACT.Exp,
                                         scale=-0.5)
                    n3 = lnp.tile([P, D], f16, name="n3", tag="n3", bufs=2)
                    t1 = lnp.tile([P, D], f16, name="t13", tag="t13",
                                  bufs=2)
                    y3 = outp.tile([P, D], f32, name="y3", tag="y3")
                    for half in range(2):
                        hsl = slice(half * 512, half * 512 + 512)
                        nc.vector.tensor_scalar(out=n3[:, hsl],
                                                in0=xr[:, hsl],
                                                scalar1=mv[:, 0:1],
                                                scalar2=rstd,
                                                op0=ALU.subtract,
                                                op1=ALU.mult)
                        if half == 0:
                            nc.gpsimd.tensor_mul(t1[:, hsl], n3[:, hsl],
                                                 g3b[:, hsl])
                            nc.vector.tensor_add(y3[:, hsl], t1[:, hsl],
                                                 b3b[:, hsl])
                        else:
                            nc.vector.tensor_mul(t1[:, hsl], n3[:, hsl],
                                                 g3b[:, hsl])
                            nc.gpsimd.tensor_add(y3[:, hsl], t1[:, hsl],
                                                 b3b[:, hsl])
                        nc.sync.dma_start(
                            out=out_d.ap()[u * P:(u + 1) * P, hsl],
                            in_=y3[:, hsl])
        n2Tp.release()
        qT2p.release()
        n2p.release()
        y2rp.release()

    nc.compile()
    return nc


_CACHE = {}


def _get_nc():
    if "nc" not in _CACHE:
        _CACHE["nc"] = build_nc()
    return _CACHE["nc"]


def _q_indices(h):
    """Interleaved q-tile ownership: core-half h owns global tiles h, h+2..."""
    tiles = np.arange(h, 2 * QTI, 2)
    return (tiles[:, None] * P + np.arange(P)[None, :]).reshape(-1)


def _q8(x):
    return np.asarray(x, np.float32).astype(E4NP)


def _q8f(x):
    return _q8(x).astype(np.float32)


def _pack_dT(m):
    """[D, n] (d-major) -> [128, DTI, n] (partition, k-tile, col)."""
    return np.ascontiguousarray(
        m.reshape(DTI, P, -1).transpose(1, 0, 2))


def _hilo(m):
    hi = _q8(m)
    lo = _q8(np.asarray(m, np.float32) - hi.astype(np.float32))
    return hi, lo


def _prep_shared(inp):
    """Weight/vector arrays shared by all cores (host-side prep)."""
    f = lambda k: np.asarray(inp[k], np.float32)
    sh = {}
    for nm, key in [("wq1", "WQ1"), ("wk1", "WK1"), ("wv1", "WV1"),
                    ("wk2", "WK2"), ("wv2", "WV2")]:
        hi, lo = _hilo(WS * f(key))
        sh[nm] = _pack_dT(hi)
        if nm in ("wq1", "wk1", "wv1"):
            sh[nm + "lo"] = _pack_dT(lo)
    # wq2 with LN1 gamma folded; bias = be1 @ WQ2
    wq2p = WS * (f("g1")[:, None] * f("WQ2"))
    sh["wq2"] = _pack_dT(_q8(wq2p))
    sh["qb2"] = np.ascontiguousarray(
        (f("be1") @ f("WQ2")).reshape(DTI, P).T).astype(np.float32)
    # FFN weights: W1 with LN2 gamma folded, hi+lo interleaved; W2 hi+lo
    w1p = WS * (f("g2")[:, None] * f("W_ff1"))
    w1h, w1l = _hilo(w1p)
    w1h = w1h.reshape(DTI, P, FTI, P).transpose(2, 1, 0, 3)
    w1l = w1l.reshape(DTI, P, FTI, P).transpose(2, 1, 0, 3)
    sh["wf1"] = np.ascontiguousarray(
        np.stack([w1h, w1l], axis=2)).reshape(FTI * P, 2, DTI, P)
    w2h, w2l = _hilo(WS * f("W_ff2"))
    sh["wf2h"] = np.ascontiguousarray(
        w2h.reshape(FTI, P, D).transpose(1, 0, 2))
    sh["wf2l"] = np.ascontiguousarray(
        w2l.reshape(FTI, P, D).transpose(1, 0, 2))
    bh = f("be2") @ f("W_ff1") + f("b_ff1")
    sh["bf1"] = np.ascontiguousarray(bh.reshape(FTI, P).T).astype(np.float32)
    sh["g1"] = f("g1").astype(np.float16)
    sh["be1"] = f("be1").astype(np.float16)
    sh["g2"] = f("g2").astype(np.float16)
    sh["b2r"] = (f("be2") + f("b_ff2")).astype(np.float16)
    sh["g3"] = f("g3")
    sh["be3"] = f("be3")
    return sh


def _mask_blocks(h):
    """[128, 8, 128] fp8: boundary mask for self-attn score tile r=t-8c,
    applied to its q-block u4b=r//2 (the only block where the causal
    frontier can land).  r even: tri (h=0) / ones (h=1); r odd: zeros
    (h=0) / tri (h=1).  Hidden non-boundary blocks are never read."""
    tri = (np.arange(P)[:, None] <= np.arange(P)[None, :]).astype(np.float32)
    blocks = np.empty((DTI, P, P), np.float32)
    for r in range(DTI):
        cmp = 2 * (r // 2) + h - r
        blocks[r] = tri if cmp == 0 else (1.0 if cmp > 0 else 0.0)
    return np.ascontiguousarray(blocks.transpose(1, 0, 2)).astype(E4NP)


def _prep_core(c, y, Z, shared):
    b, h = c // 2, c % 2
    qi = _q_indices(h)
    yb16 = y[b].astype(np.float16)          # [S, D]
    yq16 = yb16[qi]                         # [NQ, D] own queries
    xkvT = yb16.T.astype(np.float32)        # [D, S]
    xqT = yq16.T.astype(np.float32)         # [D, NQ]
    zT = Z[b].astype(np.float16).T.astype(np.float32)
    m = {
        "xq8": _pack_dT(_q8(xqT)),
        "xq0lo": _pack_dT(_q8(xqT[:, 0:P] - _q8f(xqT[:, 0:P]))),
        "xkv8": _pack_dT(_q8(xkvT)),
        "xkvelo": _pack_dT(_q8(xkvT[:, 0:2 * P] - _q8f(xkvT[:, 0:2 * P]))),
        "z8": _pack_dT(_q8(zT)),
        "yres": np.ascontiguousarray(
            yq16.reshape(QTI, P, D).transpose(1, 0, 2)),
        "maskblk": _mask_blocks(h),
    }
    m.update(shared)
    return m


def kernel(**inputs):
    inp = {k: np.asarray(v) for k, v in inputs.items()}
    y = inp["y"].astype(np.float32)
    Z = inp["Z"].astype(np.float32)
    shared = _prep_shared(inp)
    in_maps = [_prep_core(c, y, Z, shared) for c in range(N_CORES)]
    res = run_bass_kernel_spmd(_get_nc(), in_maps, list(range(N_CORES)))
    out = np.zeros((4, 2048, 1024), np.float32)
    for c in range(N_CORES):
        b, h = c // 2, c % 2
        out[b, _q_indices(h)] = res.results[c]["out"]
    return out

